# revision 1
# baseline (speedup 1.0000x reference)
"""Trainium2 Bass kernel for multi-head causal attention + output projection.

Problem (hardcoded): B=4, T=2048, E=1024, H=16, D=64, float32.
  q = einsum('bte,hed->bhtd', data, Wq)   (same k, v)
  scores = q@k.T / sqrt(D), causal mask, softmax
  out = (attn @ v) concat-heads @ Wp + bp

Sharding across 8 NeuronCores: core c -> (batch b=c//2, head-group g=c%2).
Each core computes 8 heads of one batch:
  - QKV projections from x.T (E-major layout, fed transposed from host)
  - attention with scores kept TRANSPOSED (scoresT[tk, tq]) so softmax's
    cross-key reduction is done by the TensorEngine: V is augmented with a
    ones-column so attn@V also yields sum(exp) as output row 64.
  - no max-subtraction in softmax (scores ~ N(0,1) after the 1/8 scale;
    exp cannot overflow f32)
  - causal masking: whole key-tiles above the diagonal are skipped; the 4
    diagonal tiles per query-block get an additive -1e30 mask
  - partial output projection with this core's 512-row slice of Wp
Host side: out[b] = core(2b) + core(2b+1) partials, + bias.

Matmuls run as float32r (full-rate fp32 mode, 1 cycle/row for moving dim
>= 256); softmax weights and V are bf16 for the attn@V matmul.
"""

import numpy as np

import concourse.bass as bass
import concourse.mybir as mybir
import concourse.tile as tile
from contextlib import ExitStack

F32 = mybir.dt.float32
F32R = mybir.dt.float32r
BF16 = mybir.dt.bfloat16

NEG = -1.0e30

# Full-problem constants
B, T, E, H, D = 4, 2048, 1024, 16, 64
N_CORES = 8
H_LOC = H // 2          # heads per core
HP = H_LOC // 2         # head pairs per core
SCALE = float(D) ** -0.5

# Tunables
USE_F32R = True         # bitcast f32 operands to float32r for matmuls
EXP_BF16 = True         # softmax weights in bf16
CAUSAL = True           # skip fully-masked key tiles
AV_LAG = 5              # software pipeline depth between exp and attn@V
FLUSH_T = 4             # t-step at which the previous block's norm flushes


MMDT = F32R if USE_F32R else F32


def _mm_dt(ap):
    # operands reach matmuls already typed as MMDT
    return ap


def build_program(nc, *, T=T, E=E, HP=HP, causal=CAUSAL):
    """Emit the whole per-core program into `nc`. Shapes parameterized only
    so a scaled-down version can be validated in CoreSim.

    Emission order (PE executes in order; ACT/DVE hang off it):
      v, q(all pairs), k(0), k(1), attn(0), k(2), attn(1), k(3), attn(2),
      attn(3) [+ projection per query-block inside the last pair's attn]
    so the PE always has dense independent work while ACT runs exp, and
    pools are closed (ExitStack) as their tensors die to fit SBUF.
    """
    HL = 2 * HP                    # local heads
    C = HL * D                     # local concat width (512 full-size)
    ET = E // 128                  # e (embedding) 128-tiles
    TT = T // 128                  # token 128-tiles
    TQB = 512                      # query-block width
    NJB = T // TQB                 # query blocks
    DIAG = TQB // 128              # diagonal key-tiles per query block
    EW = min(512, E)               # projection output block width
    NE = E // EW
    CT = C // 128                  # concat 128-tiles (== HP)
    W2 = 2 * TQB                   # 2-bank psum slot width
    assert CT == HP and C <= TQB
    exp_dt = BF16 if EXP_BF16 else MMDT

    AL = mybir.AluOpType
    AF = mybir.ActivationFunctionType

    xTd = nc.dram_tensor("xT", [E, T], MMDT, kind="ExternalInput").ap()
    wqd = nc.dram_tensor("wq", [E, C], MMDT, kind="ExternalInput").ap()
    wkd = nc.dram_tensor("wk", [E, C], MMDT, kind="ExternalInput").ap()
    wvd = nc.dram_tensor("wv", [E, C], MMDT, kind="ExternalInput").ap()
    wpd = nc.dram_tensor("wp", [C, E], exp_dt, kind="ExternalInput").ap()
    # mask o duplicated into both halves: [mask_o | mask_o], o = 0..DIAG-1
    masks = nc.dram_tensor("masks", [128, DIAG * W2], exp_dt,
                           kind="ExternalInput").ap()
    out = nc.dram_tensor("out", [T, E], F32, kind="ExternalOutput").ap()

    with tile.TileContext(nc) as tc, ExitStack() as ctx:
        const = ctx.enter_context(tc.tile_pool(name="const", bufs=1))
        qk_pool = ctx.enter_context(tc.tile_pool(name="qk", bufs=2 * HP))
        vaug_pool = ctx.enter_context(tc.tile_pool(name="vaug", bufs=HL))
        # all PSUM through one pool of 4 two-bank slots
        psum = ctx.enter_context(tc.tile_pool(name="ps", bufs=4, space="PSUM"))

        mask_sb = const.tile([128, DIAG * W2], exp_dt, name="mask_sb")
        ones_sb = const.tile([1, 64], exp_dt, name="ones_sb")
        nc.vector.memset(ones_sb[:], 1.0)
        # ACT-touch scratch (see norm tail)
        tch = const.tile([1, 2], F32, name="tch")


        qT = [qk_pool.tile([128, T], MMDT, tag="qk", name=f"qT{i}")
              for i in range(HP)]
        kT = [qk_pool.tile([128, T], MMDT, tag="qk", name=f"kT{i}")
              for i in range(HP)]
        vaug = [vaug_pool.tile([128, TT * 65], exp_dt, tag="vaug",
                               name=f"vaug{i}") for i in range(HL)]

        def ps_tile(parts, name):
            return psum.tile([parts, W2], F32, tag="ps", name=name,
                             padded_shape=[128, W2])

        # input pools, manually released LIFO as their tensors die:
        # stack order (bottom->top): wk, xt, wq, wv
        wk_pool = tc.alloc_tile_pool(name="wkt", bufs=ET)
        xt_pool = tc.alloc_tile_pool(name="xt", bufs=ET)
        wq_pool = tc.alloc_tile_pool(name="wqt", bufs=ET)
        wv_pool = tc.alloc_tile_pool(name="wvt", bufs=ET)

        # ---------------- inputs (DMA order = consumption order) -----------
        xt, wqt, wkt, wvt = [], [], [], []
        for e in range(ET):
            wt = wv_pool.tile([128, C], MMDT, tag="wv", name=f"wvt{e}")
            nc.sync.dma_start(wt[:], wvd[e * 128:(e + 1) * 128, :])
            wvt.append(wt)
            xe = xt_pool.tile([128, T], MMDT, tag="xt", name=f"xt{e}")
            nc.sync.dma_start(xe[:], xTd[e * 128:(e + 1) * 128, :])
            xt.append(xe)
        for e in range(ET):
            wt = wq_pool.tile([128, C], MMDT, tag="wq", name=f"wqt{e}")
            nc.sync.dma_start(wt[:], wqd[e * 128:(e + 1) * 128, :])
            wqt.append(wt)
        for e in range(ET):
            wt = wk_pool.tile([128, C], MMDT, tag="wk", name=f"wkt{e}")
            nc.sync.dma_start(wt[:], wkd[e * 128:(e + 1) * 128, :])
            wkt.append(wt)
        nc.sync.dma_start(mask_sb[:], masks)

        # ---------------- v (natural [t, d] layout, e-outer groups) --------
        for h in range(HL):
            nc.vector.memset(vaug[h][:], 1.0)
        VG = 2 if TT % 2 == 0 else TT

        def emit_v_group(tg):
            psv = [ps_tile(128, f"psv{i}") for i in range(VG // 2)]
            for e in range(ET):
                for ti in range(VG):
                    t = tg + ti
                    nc.tensor.matmul(
                        psv[ti // 2][:, (ti % 2) * TQB:(ti % 2) * TQB + C],
                        _mm_dt(xt[e][:, t * 128:(t + 1) * 128]),
                        _mm_dt(wvt[e][:]),
                        start=(e == 0), stop=(e == ET - 1))
            for ti in range(VG):
                t = tg + ti
                for h in range(HL):
                    eng = nc.vector.tensor_copy if h % 2 == 0 else \
                        nc.scalar.copy
                    eng(vaug[h][:, t * 65:t * 65 + 64],
                        psv[ti // 2][:, (ti % 2) * TQB + h * 64:
                                     (ti % 2) * TQB + (h + 1) * 64])

        # ---------------- q/k projections ([d, t] layout, pairs packed) ----
        def emit_qk(p, wlist, dst, only_jbp=None):
            for jbp in range(0, NJB, 2):
                if only_jbp is not None and jbp != only_jbp:
                    continue
                pq = ps_tile(128, "psqk")
                nhalf = min(2, NJB - jbp)
                for e in range(ET):
                    for j in range(nhalf):
                        jb = jbp + j
                        sl = slice(j * TQB, (j + 1) * TQB)
                        nc.tensor.matmul(
                            pq[:, sl],
                            _mm_dt(wlist[e][:, p * 128:(p + 1) * 128]),
                            _mm_dt(xt[e][:, jb * TQB:(jb + 1) * TQB]),
                            start=(e == 0), stop=(e == ET - 1))
                w = nhalf * TQB
                nc.vector.tensor_copy(
                    dst[p][:, jbp * TQB:jbp * TQB + w], pq[:, 0:w])

        # v groups interleaved with q projections: q matmuls fill the PE
        # stalls while v psum groups wait on their evictions.
        vgroups = list(range(0, TT, VG))
        qpairs = list(range(HP))
        emit_v_group(vgroups[0])
        for i, tg in enumerate(vgroups[1:]):
            emit_v_group(tg)
            if i < len(qpairs):
                emit_qk(qpairs[i], wqt, qT)
        for p in qpairs[max(0, len(vgroups) - 1):]:
            emit_qk(p, wqt, qT)
        wv_pool.release()
        wq_pool.release()

        # attention pools on the RIGHT side so input pools can release
        # underneath them while attention overlaps the k projections.
        n_early = max(0, HP - 2)
        olt = [None] * CT
        olt_early = None
        if n_early:
            olt_early = tc.alloc_tile_pool(name="olt01", bufs=n_early,
                                           side="right")
            for i in range(n_early):
                olt[i] = olt_early.tile([128, T], exp_dt, tag="olt",
                                        name=f"olt{i}")
        exp_pool = tc.alloc_tile_pool(name="exp", bufs=7, side="right")
        sab_pool = tc.alloc_tile_pool(name="sab", bufs=2, side="right")

        norm_queue = []

        def emit_norm(p, jb, psAB):
            """Normalization + olt write for a finished (p, jb) block,
            deferred so it overlaps the next block's compute."""
            qsl = slice(jb * TQB, (jb + 1) * TQB)
            # evict out_av rows to SBUF; 1/sumexp straight to a bf16 row
            # (borrowed exp slot) feeding the broadcast matmul.
            sab = sab_pool.tile([64, W2], F32, tag="sab", name="sab")
            nc.vector.tensor_copy(sab[:], psAB[0:64, :])
            rr = exp_pool.tile([1, W2], exp_dt, tag="exp", name="rr")
            with nc.allow_low_precision(reason="softmax recip in bf16"):
                nc.vector.reciprocal(rr[:], psAB[64:65, :])
            psb = ps_tile(64, "psbc")
            nc.tensor.matmul(psb[:, 0:TQB], ones_sb[:], rr[:, 0:TQB],
                             start=True, stop=True)
            nc.tensor.matmul(psb[:, TQB:W2], ones_sb[:], rr[:, TQB:W2],
                             start=True, stop=True)
            for hh in range(2):
                nc.vector.scalar_tensor_tensor(
                    olt[p][64 * hh:64 * hh + 64, qsl],
                    sab[:, hh * TQB:(hh + 1) * TQB], 1.0,
                    psb[0:64, hh * TQB:(hh + 1) * TQB], AL.mult, AL.mult)
            # make ACT the slot's last reader: WAR waits then merge into the
            # ACT wait every PE matmul already carries (LW allows 1 wait)
            nc.scalar.copy(tch[:], psb[0:1, 0:2])
            if p == HP - 1:
                for it in range(DIAG * jb, DIAG * (jb + 1)):
                    emit_proj(it)

        def flush_norms():
            while norm_queue:
                emit_norm(*norm_queue.pop(0))

        def emit_attn_block(p, jb):
                n_tk = DIAG * (jb + 1) if causal else TT
                # heads A|B side by side: A cols 0:TQB, B cols TQB:2TQB
                psAB = ps_tile(65, "psAB")
                qsl = slice(jb * TQB, (jb + 1) * TQB)
                pend = []

                def flush_av(psAB=psAB, p=p, n_tk=n_tk, pend=pend):
                    t, ee = pend.pop(0)
                    last = t == n_tk - 1
                    nc.tensor.matmul(
                        psAB[:, 0:TQB],
                        _bf(vaug[2 * p][:, t * 65:t * 65 + 65]),
                        _bf(ee[:, 0:TQB]), start=(t == 0), stop=last)
                    nc.tensor.matmul(
                        psAB[:, TQB:W2],
                        _bf(vaug[2 * p + 1][:, t * 65:t * 65 + 65]),
                        _bf(ee[:, TQB:W2]), start=(t == 0), stop=last)

                for t in range(n_tk):
                    ksl = slice(t * 128, (t + 1) * 128)
                    psS = ps_tile(128, "psS")
                    nc.tensor.matmul(psS[:, 0:TQB],
                                     _mm_dt(kT[p][0:64, ksl]),
                                     _mm_dt(qT[p][0:64, qsl]),
                                     start=True, stop=True)
                    nc.tensor.matmul(psS[:, TQB:W2],
                                     _mm_dt(kT[p][64:128, ksl]),
                                     _mm_dt(qT[p][64:128, qsl]),
                                     start=True, stop=True)
                    ee = exp_pool.tile([128, W2], exp_dt, tag="exp",
                                       name="ee")
                    nc.scalar.activation(ee[:], psS[:], AF.Exp, scale=SCALE)
                    o = t - DIAG * jb
                    if o >= 0:
                        # zero above-diagonal entries (bf16 SBUF fast path)
                        nc.vector.tensor_mul(
                            ee[:], ee[:], mask_sb[:, o * W2:(o + 1) * W2])
                    pend.append((t, ee))
                    if t == FLUSH_T:
                        # previous block's deferred normalization, overlapped
                        # with this block's compute
                        flush_norms()
                    if len(pend) > AV_LAG:
                        flush_av()
                while pend:
                    flush_av()
                norm_queue.append((p, jb, psAB))

        def emit_proj(it):
            ps = ps_tile(128, "psp")
            for nb in range(NE):
                for c in range(CT):
                    nc.tensor.matmul(
                        ps[:, nb * EW:(nb + 1) * EW],
                        _bf(olt[c][:, it * 128:(it + 1) * 128]),
                        _bf(wpt[c][:, nb * EW:(nb + 1) * EW]),
                        start=(c == 0), stop=(c == CT - 1))
            ot = po_pool.tile([128, E], F32, tag="po", name="po")
            nc.vector.tensor_copy(ot[:], ps[:, 0:E])
            nc.sync.dma_start(out[it * 128:(it + 1) * 128, :], ot[:])

        # k projections interleaved with attention, one pair of lag
        for p in range(HP):
            emit_qk(p, wkt, kT)
            if 1 <= p <= HP - 2:
                for jb in range(NJB):
                    emit_attn_block(p - 1, jb)
        xt_pool.release()
        wk_pool.release()

        # late pools (space freed by xt/wk): remaining olt tiles, Wp, out
        olt_late = tc.alloc_tile_pool(name="olt23", bufs=CT - n_early,
                                      side="right")
        for i in range(n_early, CT):
            olt[i] = olt_late.tile([128, T], exp_dt, tag="olt",
                                   name=f"olt{i}")
        wp_pool = tc.alloc_tile_pool(name="wp", bufs=CT, side="right")
        po_pool = tc.alloc_tile_pool(name="po", bufs=3, side="right")
        wpt = []
        for c in range(CT):
            w = wp_pool.tile([128, E], exp_dt, tag="wp", name=f"wpt{c}")
            nc.sync.dma_start(w[:], wpd[c * 128:(c + 1) * 128, :])
            wpt.append(w)
        for p in range(max(0, HP - 2), HP):
            for jb in range(NJB):
                emit_attn_block(p, jb)
        flush_norms()
        # release right-side pools LIFO
        po_pool.release()
        wp_pool.release()
        olt_late.release()
        sab_pool.release()
        exp_pool.release()
        if olt_early is not None:
            olt_early.release()
    return nc


def _bf(ap):
    # attn@V operands are typed exp_dt (bf16 or MMDT) at allocation
    return ap


_WV_CACHE = {}


def wv_row(nc, wv_pool, wv, e, C):
    """Load (once) and return the e-th 128-row slice of Wv as an SBUF tile."""
    key = (id(nc), e)
    if key not in _WV_CACHE:
        wt = wv_pool.tile([128, C], MMDT, tag="wvt", name=f"wvt{e}")
        nc.sync.dma_start(wt[:], wv[e * 128:(e + 1) * 128, :])
        _WV_CACHE[key] = wt
    return _WV_CACHE[key][:]


def _wp_cast(a):
    if EXP_BF16:
        import ml_dtypes
        return a.astype(ml_dtypes.bfloat16)
    return a


def make_masks(diag=4, tqb=512):
    import ml_dtypes
    m = np.empty((128, diag * 2 * tqb), np.float32)
    p = np.arange(128)[:, None]
    f = np.arange(tqb)[None, :]
    for o in range(diag):
        blk = np.where(f >= p + 128 * o, 1.0, 0.0)
        m[:, o * 2 * tqb:o * 2 * tqb + tqb] = blk
        m[:, o * 2 * tqb + tqb:(o + 1) * 2 * tqb] = blk
    if EXP_BF16:
        return m.astype(ml_dtypes.bfloat16)
    return m


def shard_inputs(data, Wq, Wk, Wv, Wp):
    """Build the 8 per-core input maps from full inputs."""
    data = np.asarray(data, np.float32)
    Wq = np.asarray(Wq, np.float32)
    Wk = np.asarray(Wk, np.float32)
    Wv = np.asarray(Wv, np.float32)
    Wp = np.asarray(Wp, np.float32)
    masks = make_masks()
    in_maps = []
    for c in range(N_CORES):
        b, g = c // 2, c % 2
        hs = slice(g * H_LOC, (g + 1) * H_LOC)
        in_maps.append({
            "xT": np.ascontiguousarray(data[b].T),
            "wq": np.ascontiguousarray(
                Wq[hs].transpose(1, 0, 2).reshape(E, H_LOC * D)),
            "wk": np.ascontiguousarray(
                Wk[hs].transpose(1, 0, 2).reshape(E, H_LOC * D)),
            "wv": np.ascontiguousarray(
                Wv[hs].transpose(1, 0, 2).reshape(E, H_LOC * D)),
            "wp": _wp_cast(
                np.ascontiguousarray(Wp[g * H_LOC * D:(g + 1) * H_LOC * D, :])),
            "masks": masks,
        })
    return in_maps


_NC_CACHE = {}


def legalize_single_wait(nc):
    """This toolchain's walrus accepts at most ONE sync wait per engine
    instruction; Tile freely emits more. Split extra waits onto preceding
    same-engine NoOps (engine FIFOs make that equivalent)."""
    import bass_rust
    cnt = 0
    for f in nc.m.functions:
        for blk in f.blocks:
            new = []
            changed = False
            for inst in blk.instructions:
                si = inst.sync_info
                if si is not None and len(si.on_wait) > 1:
                    waits = list(si.on_wait)
                    for w in waits[:-1]:
                        nop = bass_rust.InstNoOp(name=f"legal_nop_{cnt}")
                        cnt += 1
                        nop.engine = inst.engine
                        nop.sync_info = bass_rust.SyncInfo(on_wait=[w],
                                                           on_update=[])
                        new.append(nop)
                    inst.sync_info = bass_rust.SyncInfo(
                        on_wait=[waits[-1]], on_update=list(si.on_update))
                    changed = True
                new.append(inst)
            if changed:
                blk.instructions = new
    return cnt


def get_nc():
    if "nc" not in _NC_CACHE:
        nc = bass.Bass("TRN2", target_bir_lowering=False, debug=False,
                       num_devices=N_CORES)
        build_program(nc)
        legalize_single_wait(nc)
        _NC_CACHE["nc"] = nc
    return _NC_CACHE["nc"]


def run(inputs, trace=False, **kw):
    """Run on the 8 NeuronCores; returns (full_output, BassKernelResults)."""
    from concourse.bass_utils import run_bass_kernel_spmd
    nc = get_nc()
    in_maps = shard_inputs(inputs["data"], inputs["Wq"], inputs["Wk"],
                           inputs["Wv"], inputs["Wp"])
    res = run_bass_kernel_spmd(nc, in_maps, core_ids=list(range(N_CORES)),
                               trace=trace, **kw)
    bp = np.asarray(inputs["bp"], np.float32)
    outf = np.empty((B, T, E), np.float32)
    for b in range(B):
        outf[b] = res.results[2 * b]["out"] + res.results[2 * b + 1]["out"] + bp
    return outf, res


def kernel(**inputs):
    out, _ = run(inputs)
    return out



# revision 53
# speedup vs baseline: 1.3525x; 1.3525x over previous
"""Trainium2 Bass kernel for multi-head causal attention + output projection.

Problem (hardcoded): B=4, T=2048, E=1024, H=16, D=64, float32.
  q = einsum('bte,hed->bhtd', data, Wq)   (same k, v)
  scores = q@k.T / sqrt(D), causal mask, softmax
  out = (attn @ v) concat-heads @ Wp + bp

Sharding across 8 NeuronCores: core c -> (batch b=c//2, head-group g=c%2).
Each core computes 8 heads of one batch and a partial projection with its
512-row slice of Wp; host sums the two partials per batch and adds bias.

Kernel structure (v2 — transposed attn@V):
  - all matmul operands bf16 (validated ~5e-3 rel err end to end)
  - scores kept transposed: psS[key, query] per key tile, 2 heads side by
    side; diagonal key tiles only compute query cols >= o*128; exp on ACT
    (bf16 out); within-tile triangle masked via a [128,256] tri mask (DVE)
  - attn@V with queries on PSUM partitions: stationary = exp weights
    [k, q-chunk], moving = V-augmented [k, 64+1] -> out [q, 65] per chunk
    (65-col moving beats the 512-col orientation ~2x in PE time); the ones
    column of V yields sum(exp) at col 64
  - normalization: DVE/GPSIMD tensor_scalar with per-partition 1/sumexp
    during the PSUM->SBUF eviction, then a PE transpose ([q,c]->[c,q])
    rebuilds olt[c, t] for the output projection
  - ACT (exp) is the attention-phase bottleneck: remaining v/q/k
    projections, transposes and the output projection are drip-fed
    between key tiles by an emission pacer so the PE never starves.

PSUM discipline (8 banks = 4 slots of [128,1024]):
  tag "pss"  x2: score tiles (ping-pong), also head-phase v/q/k groups
  tag "av"   x1: per-block attn@V accumulator (A: cols 0:260, B: 512:772)
  tag "misc" x1: strictly-FIFO fillers (v pass B, q/k blocks, transposes,
                 projection rounds) — each holds the slot until done
"""

from collections import deque

import numpy as np

import concourse.bass as bass
import concourse.mybir as mybir
import concourse.tile as tile
from contextlib import ExitStack

F32 = mybir.dt.float32
BF16 = mybir.dt.bfloat16

# Full-problem constants
B, T, E, H, D = 4, 2048, 1024, 16, 64
N_CORES = 8
H_LOC = H // 2          # heads per core
HP = H_LOC // 2         # head pairs per core
SCALE = float(D) ** -0.5

C = H_LOC * D           # local concat width (512)
ET = E // 128           # embedding 128-tiles (8)
TT = T // 128           # token 128-tiles (16)
TQB = 512               # query-block width
NJB = T // TQB          # query blocks (4)
VW = 65                 # vaug per-head width (64 + ones col)
LAG = 2                 # tiles between exp and attn@V consumption (the
                        # loop emits AV one tile later -> effective 3)

# pacing constants (ns estimates mirroring the cost model)
PE_C = 1.0 / 2.4
ACT_C = 1.0 / 1.2


def _exp_ns(cols, nops=1):
    return cols * ACT_C + 185.0 * nops


def build_program(nc):
    AF = mybir.ActivationFunctionType
    AL = mybir.AluOpType

    xTd = nc.dram_tensor("xT", [E, T], BF16, kind="ExternalInput").ap()
    wqd = nc.dram_tensor("wq", [E, C], BF16, kind="ExternalInput").ap()
    wkd = nc.dram_tensor("wk", [E, C], BF16, kind="ExternalInput").ap()
    wvd = nc.dram_tensor("wv", [E, C], BF16, kind="ExternalInput").ap()
    wpd = nc.dram_tensor("wp", [C, E], BF16, kind="ExternalInput").ap()
    # ntri[r, g] = -1e30 where g < r else 0 (strict lower triangle)
    maskd = nc.dram_tensor("masks", [128, 128], BF16, kind="ExternalInput").ap()
    identd = nc.dram_tensor("ident", [128, 128], BF16, kind="ExternalInput").ap()
    out = nc.dram_tensor("out", [T, E], F32, kind="ExternalOutput").ap()

    with tile.TileContext(nc) as tc, ExitStack() as ctx:
        const = ctx.enter_context(tc.tile_pool(name="const", bufs=1))
        xt_pool = ctx.enter_context(tc.tile_pool(name="xt", bufs=ET))
        wv_pool = ctx.enter_context(tc.tile_pool(name="wvp", bufs=ET))
        wq_pool = ctx.enter_context(tc.tile_pool(name="wqp", bufs=ET))
        wk_pool = ctx.enter_context(tc.tile_pool(name="wkp", bufs=ET))
        vaug_pool = ctx.enter_context(tc.tile_pool(name="vaugp", bufs=1))
        qk_pool = ctx.enter_context(tc.tile_pool(name="qkp", bufs=2 * HP))
        ee_pool = ctx.enter_context(tc.tile_pool(name="eep", bufs=6))
        usb_pool = ctx.enter_context(tc.tile_pool(name="usbp", bufs=8))
        r_pool = ctx.enter_context(tc.tile_pool(name="rp", bufs=4))
        olt_pool = ctx.enter_context(tc.tile_pool(name="oltp", bufs=HP))
        wp_pool = ctx.enter_context(tc.tile_pool(name="wpp", bufs=HP))
        out_pool = ctx.enter_context(tc.tile_pool(name="outp", bufs=4))
        psum = ctx.enter_context(tc.tile_pool(name="ps", bufs=4, space="PSUM"))

        mask_sb = const.tile([128, 128], BF16, name="mask_sb")
        ident_sb = const.tile([128, 128], BF16, name="ident_sb")

        vaug = vaug_pool.tile([128, TT * H_LOC * VW], BF16, name="vaug")
        xt = [xt_pool.tile([128, T], BF16, tag="xt", name=f"xt{e}")
              for e in range(ET)]
        wvt = [wv_pool.tile([128, C], BF16, tag="wv", name=f"wvt{e}")
               for e in range(ET)]
        wqt = [wq_pool.tile([128, C], BF16, tag="wq", name=f"wqt{e}")
               for e in range(ET)]
        wkt = [wk_pool.tile([128, C], BF16, tag="wk", name=f"wkt{e}")
               for e in range(ET)]
        qT = [qk_pool.tile([128, T], BF16, tag="qk", name=f"qT{p}")
              for p in range(HP)]
        kT = [qk_pool.tile([128, T], BF16, tag="qk", name=f"kT{p}")
              for p in range(HP)]
        olt = [olt_pool.tile([128, T], BF16, tag="olt", name=f"olt{c}")
               for c in range(HP)]
        wpt = [wp_pool.tile([128, E], BF16, tag="wp", name=f"wpt{c}")
               for c in range(HP)]

        TAG_BUFS = {"pss": 2, "av": 1, "misc": 2}

        def ps_tile(tag, name):
            # pss/av slots are 2 banks ([128,1024] f32); misc slots 1 bank
            shape = [128, 512] if tag == "misc" else [128, 1024]
            return psum.tile(shape, F32, tag=tag, name=name,
                             bufs=TAG_BUFS[tag])

        # ---------------- input DMAs (order = consumption order) ----------
        # first v matmul needs only xt0[:,0:128]+wvt0: tiny first transfers
        nc.sync.dma_start(xt[0][:, 0:128], xTd[0:128, 0:128])
        nc.sync.dma_start(wvt[0][:], wvd[0:128, :])
        nc.sync.dma_start(xt[0][:, 128:T // 2], xTd[0:128, 128:T // 2])
        for e in range(1, ET):
            nc.sync.dma_start(wvt[e][:], wvd[e * 128:(e + 1) * 128, :])
            nc.sync.dma_start(xt[e][:, 0:T // 2],
                              xTd[e * 128:(e + 1) * 128, 0:T // 2])
        for e in range(ET):
            nc.sync.dma_start(wqt[e][:], wqd[e * 128:(e + 1) * 128, :])
        for e in range(ET):
            nc.sync.dma_start(xt[e][:, T // 2:T],
                              xTd[e * 128:(e + 1) * 128, T // 2:T])
        for e in range(ET):
            nc.sync.dma_start(wkt[e][:], wkd[e * 128:(e + 1) * 128, :])
        nc.sync.dma_start(mask_sb[:], maskd)
        nc.sync.dma_start(ident_sb[:], identd)
        for c in range(HP):
            nc.sync.dma_start(wpt[c][:], wpd[c * 128:(c + 1) * 128, :])

        # ones columns of vaug (data cols are fully overwritten by evicts)
        nc.vector.memset(vaug[:, 64:TT * H_LOC * VW:VW], 1.0)

        def evict_engine(i, with_act=False):
            # GPSIMD cannot access PSUM on this target: evictions are
            # DVE-only during attention, DVE/ACT alternating in phases
            # where the ACT (exp) is idle.
            engs = [nc.vector, nc.scalar] if with_act else [nc.vector]
            eng = engs[i % len(engs)]

            def copy(out_ap, in_ap, _eng=eng):
                if _eng is nc.scalar:
                    return _eng.copy(out_ap, in_ap)
                return _eng.tensor_copy(out_ap, in_ap)
            return type("E", (), {"tensor_copy": staticmethod(copy),
                                  "tensor_scalar": getattr(eng, "tensor_scalar",
                                                           None)})

        # ---------------- v projection ------------------------------------
        def v_mm(pv, slot, e, t):
            nc.tensor.matmul(
                pv[:, slot * C:(slot + 1) * C],
                xt[e][:, t * 128:(t + 1) * 128],
                wvt[e][:],
                start=(e == 0), stop=(e == ET - 1))

        def v_evict(pv, slot, t, eng):
            base = t * H_LOC * VW
            dst = vaug[:, base:base + H_LOC * VW].rearrange(
                "p (h c) -> p h c", c=VW)[:, :, 0:64]
            src = pv[:, slot * C:(slot + 1) * C].rearrange(
                "p (h c) -> p h c", c=64)
            eng.tensor_copy(dst, src)

        # head phase: t 0..5 in 4 interleaved groups, then t 6..7
        def emit_v_pass_a():
            g0 = ps_tile("pss", "psv_a0")
            g1 = ps_tile("pss", "psv_a1")
            g2 = ps_tile("misc", "psv_a2")
            g3 = ps_tile("misc", "psv_a3")
            gs = [(g0, 0), (g0, 1), (g1, 0), (g1, 1), (g2, 0), (g3, 0)]
            for e in range(ET):
                for t in range(6):
                    v_mm(gs[t][0], gs[t][1], e, t)
            for t in range(6):
                v_evict(gs[t][0], gs[t][1], t, evict_engine(t, with_act=True))
            g4 = ps_tile("av", "psv_a4")
            for e in range(ET):
                for t in (6, 7):
                    v_mm(g4, t % 2, e, t)
            for t in (6, 7):
                v_evict(g4, t % 2, t, evict_engine(t, with_act=True))

        # ---------------- q/k projections (head phase, pss tag) -----------
        def emit_qk_block(wlist, dst, p, jbp, eng, tag="pss"):
            pq = ps_tile(tag, "psqk")
            for e in range(ET):
                for j in range(2):
                    jb = jbp + j
                    nc.tensor.matmul(
                        pq[:, j * TQB:(j + 1) * TQB],
                        wlist[e][:, p * 128:(p + 1) * 128],
                        xt[e][:, jb * TQB:(jb + 1) * TQB],
                        start=(e == 0), stop=(e == ET - 1))
            eng.tensor_copy(dst[p][:, jbp * TQB:(jbp + 2) * TQB], pq[:])

        # ---------------- pacer / filler machinery ------------------------
        # Two queues: `urgent` (per-chunk transposes — tiny, gate olt) and
        # `background` (v pass B, q/k projections, output projection —
        # clock-paced against the ACT (exp) bottleneck). Entries:
        # (key, gen, min_tick): min_tick delays emission until the DVE work
        # they depend on has had time to execute (avtick = AV emissions).
        clock = {"pe": 0.0, "act": 0.0}
        avtick = [0]
        urgent = deque()
        background = deque()

        def gen_v_group_b(t):
            pv = ps_tile("misc", f"psv_b{t}")
            for e in range(ET):
                v_mm(pv, 0, e, t)
                yield TQB * PE_C
            v_evict(pv, 0, t, evict_engine(t))

        def gen_qk_fill(wlist, dst, p, jb):
            pq = ps_tile("misc", "psqkf")
            for e in range(ET):
                nc.tensor.matmul(
                    pq[:],
                    wlist[e][:, p * 128:(p + 1) * 128],
                    xt[e][:, jb * TQB:(jb + 1) * TQB],
                    start=(e == 0), stop=(e == ET - 1))
                yield TQB * PE_C
            evict_engine(p + jb).tensor_copy(
                dst[p][:, jb * TQB:(jb + 1) * TQB], pq[:])

        def gen_transp_chunk(p, jb, c, usb_c):
            mt = psum.tile([128, 128], BF16, tag="misc", name="pstr",
                           bufs=TAG_BUFS["misc"])
            nc.tensor.transpose(mt[:], usb_c[:], ident_sb[:])
            evict_engine(c).tensor_copy(
                olt[p][:, jb * TQB + c * 128:jb * TQB + (c + 1) * 128],
                mt[:])
            yield 128 * PE_C

        def gen_proj_tile(t):
            ot = out_pool.tile([128, E], F32, tag="out", name=f"ot{t}")
            for nb in range(2):
                mp = ps_tile("misc", "psproj")
                h = nb * TQB
                for cc in range(HP):
                    nc.tensor.matmul(
                        mp[:],
                        olt[cc][:, t * 128:(t + 1) * 128],
                        wpt[cc][:, h:h + TQB],
                        start=(cc == 0), stop=(cc == HP - 1))
                    yield TQB * PE_C
                nc.vector.tensor_copy(ot[:, h:h + TQB], mp[:])
                nc.sync.dma_start(out[t * 128:(t + 1) * 128, h:h + TQB],
                                  ot[:, h:h + TQB])

        def step_q(q):
            entry = q[0]
            try:
                clock["pe"] += next(entry[1])
                if len(entry) == 5:
                    entry[4][0] -= 1
                return True
            except StopIteration:
                q.popleft()
                return False

        quota = [0.0]

        def pace():
            while urgent and urgent[0][2] <= avtick[0]:
                step_q(urgent)
            if not background:
                return
            # EDF rationing: per tick emit just enough background steps that
            # every entry finishes by its deadline, spread uniformly
            tick = avtick[0]
            cum, rate = 0.0, 0.0
            for entry in background:
                cum += entry[4][0]
                rate = max(rate, cum / max(entry[3] - tick, 1.0))
            quota[0] = min(quota[0] + rate, 8.0)
            while (background and quota[0] >= 1.0
                   and background[0][2] <= avtick[0]):
                if step_q(background):
                    quota[0] -= 1.0

        def force_drain(q, pred):
            """Fully emit all entries of q matching pred (FIFO order, so
            everything queued before them drains too)."""
            while any(pred(e[0]) for e in q):
                step_q(q)

        def drain_fillers():
            while urgent:
                step_q(urgent)
            while background:
                step_q(background)

        # ---------------- attention block ---------------------------------
        def emit_attn_block(p, jb, bi):
            n_tk = NJB * (jb + 1)
            av = ps_tile("av", "psav")
            r_t = [r_pool.tile([128, NJB], F32, tag="r", name=f"r{h}")
                   for h in range(2)]
            usb = [None] * NJB
            pend = deque()

            def emit_av():
                avtick[0] += 1
                t, ee = pend.popleft()
                o = t - NJB * jb
                for h in range(2):
                    hb = h * 512
                    vcol = t * H_LOC * VW + (2 * p + h) * VW
                    for cch in range(max(o, 0), NJB):
                        # one accumulation group per PSUM bank per block:
                        # start only on the round's first matmul (start
                        # marks the whole 2KB zero-region pending; later
                        # chunks are lazily zeroed on first write), stop
                        # only on the last (tile n_tk-1 touches chunk 3
                        # alone)
                        nc.tensor.matmul(
                            av[:, hb + cch * VW:hb + cch * VW + VW],
                            ee[:, hb + cch * 128:hb + (cch + 1) * 128],
                            vaug[:, vcol:vcol + VW],
                            start=(t == 0 and cch == max(o, 0)),
                            stop=(t == n_tk - 1 and cch == NJB - 1))
                        clock["pe"] += VW * PE_C
                # after the bank groups stop (last tile): normalize + evict
                # all chunks, then queue their transposes / projections
                if t == n_tk - 1:
                    for h in range(2):
                        hb = h * 512
                        nc.vector.reciprocal(
                            r_t[h][:],
                            av[:, hb + 64:hb + NJB * VW:VW])
                    for cch in range(NJB):
                        usb[cch] = usb_pool.tile(
                            [128, 128], BF16, tag="usb", name=f"usb{cch}")
                        for h in range(2):
                            hb = h * 512
                            eng = evict_engine(cch + h,
                                               with_act=(p == HP - 1))
                            eng.tensor_scalar(
                                usb[cch][:, h * 64:(h + 1) * 64],
                                av[:, hb + cch * VW:hb + cch * VW + 64],
                                r_t[h][:, cch:cch + 1], None, AL.mult)
                        urgent.append((("t", bi),
                                       gen_transp_chunk(p, jb, cch, usb[cch]),
                                       avtick[0] + 3 + cch))
                        if p == HP - 1:
                            # drain each block's proj within the following
                            # pair-3 block (last: by end)
                            dl = {0: 132, 1: 144, 2: 160, 3: 160}[jb]
                            background.append(
                                (("proj", jb),
                                 gen_proj_tile(NJB * jb + cch),
                                 avtick[0] + 2 + cch, dl, [9]))

            for t in range(n_tk):
                if len(pend) > LAG:
                    emit_av()
                o = t - NJB * jb
                psS = ps_tile("pss", "psS")
                lo = max(o, 0) * 128
                for h in range(2):
                    hb = h * 512
                    kc = kT[p][h * 64:(h + 1) * 64, t * 128:(t + 1) * 128]
                    if o < 0:
                        nc.tensor.matmul(
                            psS[:, hb:hb + 512], kc,
                            qT[p][h * 64:(h + 1) * 64,
                                  jb * TQB:(jb + 1) * TQB],
                            start=True, stop=True)
                    else:
                        # diagonal tile: -1e30 mask bias folded into the
                        # accumulation, then scores on top. One group per
                        # bank: start only on the first matmul (the rest
                        # region is lazily zeroed on write), stop on last.
                        nc.tensor.matmul(
                            psS[:, hb + lo:hb + lo + 128],
                            ident_sb[:], mask_sb[:],
                            start=True, stop=False)
                        nc.tensor.matmul(
                            psS[:, hb + lo:hb + lo + 128], kc,
                            qT[p][h * 64:(h + 1) * 64,
                                  jb * TQB + lo:jb * TQB + lo + 128],
                            start=False, stop=(o == NJB - 1))
                        clock["pe"] += 256 * PE_C
                        if o < NJB - 1:
                            nc.tensor.matmul(
                                psS[:, hb + lo + 128:hb + 512], kc,
                                qT[p][h * 64:(h + 1) * 64,
                                      jb * TQB + lo + 128:(jb + 1) * TQB],
                                start=False, stop=True)
                    clock["pe"] += (512 - lo - (128 if o >= 0 else 0)) * PE_C
                ee = ee_pool.tile([128, 1024], BF16, tag="ee", name="ee")
                if o <= 0:
                    nc.scalar.activation(ee[:], psS[:], AF.Exp, scale=SCALE)
                    clock["act"] += _exp_ns(1024)
                else:
                    for h in range(2):
                        nc.scalar.activation(
                            ee[:, h * 512 + lo:h * 512 + 512],
                            psS[:, h * 512 + lo:h * 512 + 512],
                            AF.Exp, scale=SCALE)
                    clock["act"] += _exp_ns(2 * (512 - lo), nops=2)
                pend.append((t, ee))
                pace()
            while pend:
                emit_av()

        # ---------------- emission ----------------------------------------
        emit_v_pass_a()
        emit_qk_block(wqt, qT, 0, 0, evict_engine(0, True))
        emit_qk_block(wqt, qT, 0, 2, evict_engine(1, True))
        emit_qk_block(wkt, kT, 0, 0, evict_engine(0, True))
        emit_qk_block(wkt, kT, 0, 2, evict_engine(1, True))

        for tg in range(8, TT):
            jbn = tg // 4
            dl = max(2 * jbn * (jbn + 1) - 2, 1)
            background.append((("vb", tg), gen_v_group_b(tg), 0, dl, [9]))
        for p in range(1, HP):
            for jb in range(NJB):
                s_blk = 40 * p + 2 * jb * (jb + 1)
                background.append(
                    (("qk", p, jb), gen_qk_fill(wqt, qT, p, jb), 0,
                     max(s_blk - 2, 1), [9]))
                background.append(
                    (("qk", p, jb), gen_qk_fill(wkt, kT, p, jb), 0,
                     max(s_blk + 4 * jb - 2, 1), [9]))

        bi = 0
        for p in range(HP):
            for jb in range(NJB):
                # correctness: everything this block consumes must already
                # be emitted (Tile deps follow emission order) — vaug tiles
                # for its key range, q/k of this pair; plus recycle old
                # transposes (usb pool depth) before new norms allocate.
                n_tk = NJB * (jb + 1)
                force_drain(background, lambda k, n=n_tk, p=p: (
                    (k[0] == "vb" and k[1] < n)
                    or (k[0] == "qk" and k[1] <= p)))
                force_drain(urgent, lambda k, bi=bi: (
                    k[0] == "t" and k[1] <= bi - 2))
                emit_attn_block(p, jb, bi)
                bi += 1
        drain_fillers()
    return nc


def make_host_inputs():
    import ml_dtypes
    ntri = np.where(np.arange(128)[None, :] < np.arange(128)[:, None],
                    -1.0e30, 0.0).astype(np.float32)
    masks = ntri.astype(ml_dtypes.bfloat16)
    ident = np.eye(128, dtype=np.float32).astype(ml_dtypes.bfloat16)
    return masks, ident


def shard_inputs(data, Wq, Wk, Wv, Wp):
    """Build the 8 per-core input maps from full inputs."""
    import ml_dtypes
    BF = ml_dtypes.bfloat16
    data = np.asarray(data, np.float32)
    Wq = np.asarray(Wq, np.float32)
    Wk = np.asarray(Wk, np.float32)
    Wv = np.asarray(Wv, np.float32)
    Wp = np.asarray(Wp, np.float32)
    masks, ident = make_host_inputs()
    in_maps = []
    for c in range(N_CORES):
        b, g = c // 2, c % 2
        hs = slice(g * H_LOC, (g + 1) * H_LOC)
        in_maps.append({
            "xT": np.ascontiguousarray(data[b].T).astype(BF),
            "wq": np.ascontiguousarray(
                Wq[hs].transpose(1, 0, 2).reshape(E, H_LOC * D)).astype(BF),
            "wk": np.ascontiguousarray(
                Wk[hs].transpose(1, 0, 2).reshape(E, H_LOC * D)).astype(BF),
            "wv": np.ascontiguousarray(
                Wv[hs].transpose(1, 0, 2).reshape(E, H_LOC * D)).astype(BF),
            "wp": np.ascontiguousarray(
                Wp[g * H_LOC * D:(g + 1) * H_LOC * D, :]).astype(BF),
            "masks": masks,
            "ident": ident,
        })
    return in_maps


_NC_CACHE = {}


def legalize_single_wait(nc):
    """This toolchain's walrus accepts at most ONE sync wait per engine
    instruction; Tile freely emits more. Split extra waits onto preceding
    same-engine NoOps (engine FIFOs make that equivalent)."""
    import bass_rust
    cnt = 0
    for f in nc.m.functions:
        for blk in f.blocks:
            new = []
            changed = False
            for inst in blk.instructions:
                si = inst.sync_info
                if si is not None and len(si.on_wait) > 1:
                    waits = list(si.on_wait)
                    for w in waits[:-1]:
                        nop = bass_rust.InstNoOp(name=f"legal_nop_{cnt}")
                        cnt += 1
                        nop.engine = inst.engine
                        nop.sync_info = bass_rust.SyncInfo(on_wait=[w],
                                                           on_update=[])
                        new.append(nop)
                    inst.sync_info = bass_rust.SyncInfo(
                        on_wait=[waits[-1]], on_update=list(si.on_update))
                    changed = True
                new.append(inst)
            if changed:
                blk.instructions = new
    return cnt


def get_nc():
    if "nc" not in _NC_CACHE:
        nc = bass.Bass("TRN2", target_bir_lowering=False, debug=False,
                       num_devices=N_CORES)
        build_program(nc)
        legalize_single_wait(nc)
        _NC_CACHE["nc"] = nc
    return _NC_CACHE["nc"]


def run(inputs, trace=False, **kw):
    """Run on the 8 NeuronCores; returns (full_output, BassKernelResults)."""
    from concourse.bass_utils import run_bass_kernel_spmd
    nc = get_nc()
    in_maps = shard_inputs(inputs["data"], inputs["Wq"], inputs["Wk"],
                           inputs["Wv"], inputs["Wp"])
    res = run_bass_kernel_spmd(nc, in_maps, core_ids=list(range(N_CORES)),
                               trace=trace, **kw)
    bp = np.asarray(inputs["bp"], np.float32)
    outf = np.empty((B, T, E), np.float32)
    for b in range(B):
        outf[b] = res.results[2 * b]["out"] + res.results[2 * b + 1]["out"] + bp
    return outf, res


def kernel(**inputs):
    out, _ = run(inputs)
    return out


# revision 62
# speedup vs baseline: 1.3600x; 1.0055x over previous
"""Trainium2 Bass kernel for multi-head causal attention + output projection.

Problem (hardcoded): B=4, T=2048, E=1024, H=16, D=64, float32.
  q = einsum('bte,hed->bhtd', data, Wq)   (same k, v)
  scores = q@k.T / sqrt(D), causal mask, softmax
  out = (attn @ v) concat-heads @ Wp + bp

Sharding across 8 NeuronCores: core c -> (batch b=c//2, head-group g=c%2).
Each core computes 8 heads of one batch and a partial projection with its
512-row slice of Wp; host sums the two partials per batch and adds bias.

Kernel structure (v2 — transposed attn@V):
  - all matmul operands bf16 (validated ~5e-3 rel err end to end)
  - scores kept transposed: psS[key, query] per key tile, 2 heads side by
    side; diagonal key tiles only compute query cols >= o*128, with the
    -1e30 causal mask folded into the PSUM accumulation as an extra
    ident.T@ntri matmul (no DVE masking, one PE->ACT->PE hop per tile)
  - attn@V with queries on PSUM partitions: stationary = exp weights
    [k, q-chunk], moving = V-augmented [k, 64+1] -> out [q, 65] per chunk
    (65-col moving beats the 512-col orientation ~2x in PE time); the ones
    column of V yields sum(exp) at col 64
  - PSUM zero-region rule: start=True lazily zeroes the whole 2KB bank, so
    each (bank, block) accumulation round has exactly one start (first
    matmul) and one stop (last matmul); PSUM is only read after the stop
  - normalization at block end: DVE reciprocal + per-partition
    tensor_scalar eviction, then a PE transpose ([q,c]->[c,q]) rebuilds
    olt[c, t] for the output projection
  - the PE is the overall bottleneck (~213us busy): remaining v/q/k
    projections, transposes and the output projection are drip-fed
    between key tiles by an EDF-rationed emission pacer so filler work
    interleaves with the ACT-paced attention cadence instead of bursting

PSUM discipline (8 banks):
  tag "pss"  x2 [128,1024]: score tiles (ping-pong) + head-phase groups
  tag "av"   x1 [128,1024]: per-block attn@V (A: cols 0:260, B: 512:772)
  tag "misc" x2 [128,512]:  fillers (v pass B, q/k blocks, transposes,
                            projection rounds), ping-pong hides WAR
"""

from collections import deque

import numpy as np

import concourse.bass as bass
import concourse.mybir as mybir
import concourse.tile as tile
from contextlib import ExitStack

F32 = mybir.dt.float32
BF16 = mybir.dt.bfloat16

# Full-problem constants
B, T, E, H, D = 4, 2048, 1024, 16, 64
N_CORES = 8
H_LOC = H // 2          # heads per core
HP = H_LOC // 2         # head pairs per core
SCALE = float(D) ** -0.5

C = H_LOC * D           # local concat width (512)
ET = E // 128           # embedding 128-tiles (8)
TT = T // 128           # token 128-tiles (16)
TQB = 512               # query-block width
NJB = T // TQB          # query blocks (4)
VW = 65                 # vaug per-head width (64 + ones col)
LAG = 2                 # tiles between exp and attn@V consumption (the
                        # loop emits AV one tile later -> effective 3)

# pacing constants (ns estimates mirroring the cost model)
PE_C = 1.0 / 2.4
ACT_C = 1.0 / 1.2


def _exp_ns(cols, nops=1):
    return cols * ACT_C + 185.0 * nops


def build_program(nc):
    AF = mybir.ActivationFunctionType
    AL = mybir.AluOpType

    xTd = nc.dram_tensor("xT", [E, T], BF16, kind="ExternalInput").ap()
    wqd = nc.dram_tensor("wq", [E, C], BF16, kind="ExternalInput").ap()
    wkd = nc.dram_tensor("wk", [E, C], BF16, kind="ExternalInput").ap()
    wvd = nc.dram_tensor("wv", [E, C], BF16, kind="ExternalInput").ap()
    wpd = nc.dram_tensor("wp", [C, E], BF16, kind="ExternalInput").ap()
    # ntri[r, g] = -1e30 where g < r else 0 (strict lower triangle)
    maskd = nc.dram_tensor("masks", [128, 128], BF16, kind="ExternalInput").ap()
    identd = nc.dram_tensor("ident", [128, 128], BF16, kind="ExternalInput").ap()
    out = nc.dram_tensor("out", [T, E], F32, kind="ExternalOutput").ap()

    with tile.TileContext(nc) as tc, ExitStack() as ctx:
        const = ctx.enter_context(tc.tile_pool(name="const", bufs=1))
        xt_pool = ctx.enter_context(tc.tile_pool(name="xt", bufs=ET))
        wv_pool = ctx.enter_context(tc.tile_pool(name="wvp", bufs=ET))
        wq_pool = ctx.enter_context(tc.tile_pool(name="wqp", bufs=ET))
        wk_pool = ctx.enter_context(tc.tile_pool(name="wkp", bufs=ET))
        vaug_pool = ctx.enter_context(tc.tile_pool(name="vaugp", bufs=1))
        qk_pool = ctx.enter_context(tc.tile_pool(name="qkp", bufs=2 * HP))
        ee_pool = ctx.enter_context(tc.tile_pool(name="eep", bufs=6))
        usb_pool = ctx.enter_context(tc.tile_pool(name="usbp", bufs=8))
        r_pool = ctx.enter_context(tc.tile_pool(name="rp", bufs=4))
        olt_pool = ctx.enter_context(tc.tile_pool(name="oltp", bufs=HP))
        wp_pool = ctx.enter_context(tc.tile_pool(name="wpp", bufs=HP))
        out_pool = ctx.enter_context(tc.tile_pool(name="outp", bufs=4))
        psum = ctx.enter_context(tc.tile_pool(name="ps", bufs=4, space="PSUM"))

        mask_sb = const.tile([128, 128], BF16, name="mask_sb")
        ident_sb = const.tile([128, 128], BF16, name="ident_sb")

        vaug = vaug_pool.tile([128, TT * H_LOC * VW], BF16, name="vaug")
        xt = [xt_pool.tile([128, T], BF16, tag="xt", name=f"xt{e}")
              for e in range(ET)]
        wvt = [wv_pool.tile([128, C], BF16, tag="wv", name=f"wvt{e}")
               for e in range(ET)]
        wqt = [wq_pool.tile([128, C], BF16, tag="wq", name=f"wqt{e}")
               for e in range(ET)]
        wkt = [wk_pool.tile([128, C], BF16, tag="wk", name=f"wkt{e}")
               for e in range(ET)]
        qT = [qk_pool.tile([128, T], BF16, tag="qk", name=f"qT{p}")
              for p in range(HP)]
        kT = [qk_pool.tile([128, T], BF16, tag="qk", name=f"kT{p}")
              for p in range(HP)]
        olt = [olt_pool.tile([128, T], BF16, tag="olt", name=f"olt{c}")
               for c in range(HP)]
        wpt = [wp_pool.tile([128, E], BF16, tag="wp", name=f"wpt{c}")
               for c in range(HP)]

        TAG_BUFS = {"pss": 2, "av": 1, "misc": 2}

        def ps_tile(tag, name):
            # pss/av slots are 2 banks ([128,1024] f32); misc slots 1 bank
            shape = [128, 512] if tag == "misc" else [128, 1024]
            return psum.tile(shape, F32, tag=tag, name=name,
                             bufs=TAG_BUFS[tag])

        # ---------------- input DMAs (order = consumption order) ----------
        # first v matmul needs only xt0[:,0:128]+wvt0: tiny first transfers
        nc.sync.dma_start(xt[0][:, 0:128], xTd[0:128, 0:128])
        nc.sync.dma_start(wvt[0][:], wvd[0:128, :])
        nc.sync.dma_start(xt[0][:, 128:T // 2], xTd[0:128, 128:T // 2])
        for e in range(1, ET):
            nc.sync.dma_start(wvt[e][:], wvd[e * 128:(e + 1) * 128, :])
            nc.sync.dma_start(xt[e][:, 0:T // 2],
                              xTd[e * 128:(e + 1) * 128, 0:T // 2])
        for e in range(ET):
            nc.sync.dma_start(wqt[e][:], wqd[e * 128:(e + 1) * 128, :])
        for e in range(ET):
            nc.sync.dma_start(xt[e][:, T // 2:T],
                              xTd[e * 128:(e + 1) * 128, T // 2:T])
        for e in range(ET):
            nc.sync.dma_start(wkt[e][:], wkd[e * 128:(e + 1) * 128, :])
        nc.sync.dma_start(mask_sb[:], maskd)
        nc.sync.dma_start(ident_sb[:], identd)
        for c in range(HP):
            nc.sync.dma_start(wpt[c][:], wpd[c * 128:(c + 1) * 128, :])

        # ones columns of vaug (data cols are fully overwritten by evicts)
        nc.vector.memset(vaug[:, 64:TT * H_LOC * VW:VW], 1.0)

        def evict_engine(i, with_act=False):
            # GPSIMD cannot access PSUM on this target: evictions are
            # DVE-only during attention, DVE/ACT alternating in phases
            # where the ACT (exp) is idle.
            engs = [nc.vector, nc.scalar] if with_act else [nc.vector]
            eng = engs[i % len(engs)]

            def copy(out_ap, in_ap, _eng=eng):
                if _eng is nc.scalar:
                    return _eng.copy(out_ap, in_ap)
                return _eng.tensor_copy(out_ap, in_ap)

            def ts(out_ap, in_ap, scalar, _unused, _op, _eng=eng):
                if _eng is nc.scalar:
                    return _eng.mul(out_ap, in_ap, scalar)
                return _eng.tensor_scalar(out_ap, in_ap, scalar, None, _op)
            return type("E", (), {"tensor_copy": staticmethod(copy),
                                  "tensor_scalar": staticmethod(ts)})

        # ---------------- v projection ------------------------------------
        def v_mm(pv, slot, e, t):
            nc.tensor.matmul(
                pv[:, slot * C:(slot + 1) * C],
                xt[e][:, t * 128:(t + 1) * 128],
                wvt[e][:],
                start=(e == 0), stop=(e == ET - 1))

        def v_evict(pv, slot, t, eng):
            base = t * H_LOC * VW
            dst = vaug[:, base:base + H_LOC * VW].rearrange(
                "p (h c) -> p h c", c=VW)[:, :, 0:64]
            src = pv[:, slot * C:(slot + 1) * C].rearrange(
                "p (h c) -> p h c", c=64)
            eng.tensor_copy(dst, src)

        # head phase: t 0..5 in 4 interleaved groups, then t 6..7
        def emit_v_pass_a():
            g0 = ps_tile("pss", "psv_a0")
            g1 = ps_tile("pss", "psv_a1")
            g2 = ps_tile("misc", "psv_a2")
            g3 = ps_tile("misc", "psv_a3")
            gs = [(g0, 0), (g0, 1), (g1, 0), (g1, 1), (g2, 0), (g3, 0)]
            for e in range(ET):
                for t in range(6):
                    v_mm(gs[t][0], gs[t][1], e, t)
            for t in range(6):
                v_evict(gs[t][0], gs[t][1], t, evict_engine(t, with_act=True))
            g4 = ps_tile("av", "psv_a4")
            for e in range(ET):
                for t in (6, 7):
                    v_mm(g4, t % 2, e, t)
            for t in (6, 7):
                v_evict(g4, t % 2, t, evict_engine(t, with_act=True))

        # ---------------- q/k projections (head phase, pss tag) -----------
        def emit_qk_block(wlist, dst, p, jbp, eng, tag="pss"):
            pq = ps_tile(tag, "psqk")
            for e in range(ET):
                for j in range(2):
                    jb = jbp + j
                    nc.tensor.matmul(
                        pq[:, j * TQB:(j + 1) * TQB],
                        wlist[e][:, p * 128:(p + 1) * 128],
                        xt[e][:, jb * TQB:(jb + 1) * TQB],
                        start=(e == 0), stop=(e == ET - 1))
            eng.tensor_copy(dst[p][:, jbp * TQB:(jbp + 2) * TQB], pq[:])

        # ---------------- pacer / filler machinery ------------------------
        # Two queues: `urgent` (per-chunk transposes — tiny, gate olt) and
        # `background` (v pass B, q/k projections, output projection —
        # clock-paced against the ACT (exp) bottleneck). Entries:
        # (key, gen, min_tick): min_tick delays emission until the DVE work
        # they depend on has had time to execute (avtick = AV emissions).
        clock = {"pe": 0.0, "act": 0.0}
        avtick = [0]
        urgent = deque()
        background = deque()

        def gen_v_group_b(t):
            pv = ps_tile("misc", f"psv_b{t}")
            for e in range(ET):
                v_mm(pv, 0, e, t)
                yield TQB * PE_C
            v_evict(pv, 0, t, evict_engine(t))

        def gen_qk_fill(wlist, dst, p, jb):
            pq = ps_tile("misc", "psqkf")
            for e in range(ET):
                nc.tensor.matmul(
                    pq[:],
                    wlist[e][:, p * 128:(p + 1) * 128],
                    xt[e][:, jb * TQB:(jb + 1) * TQB],
                    start=(e == 0), stop=(e == ET - 1))
                yield TQB * PE_C
            evict_engine(p + jb).tensor_copy(
                dst[p][:, jb * TQB:(jb + 1) * TQB], pq[:])

        def gen_transp_chunk(p, jb, c, usb_c):
            mt = psum.tile([128, 128], BF16, tag="misc", name="pstr",
                           bufs=TAG_BUFS["misc"])
            nc.tensor.transpose(mt[:], usb_c[:], ident_sb[:])
            evict_engine(c).tensor_copy(
                olt[p][:, jb * TQB + c * 128:jb * TQB + (c + 1) * 128],
                mt[:])
            yield 128 * PE_C

        def gen_proj_tile(t, act_evict=False):
            ot = out_pool.tile([128, E], F32, tag="out", name=f"ot{t}")
            for nb in range(2):
                mp = ps_tile("misc", "psproj")
                h = nb * TQB
                for cc in range(HP):
                    nc.tensor.matmul(
                        mp[:],
                        olt[cc][:, t * 128:(t + 1) * 128],
                        wpt[cc][:, h:h + TQB],
                        start=(cc == 0), stop=(cc == HP - 1))
                    yield TQB * PE_C
                # ACT is idle at the very end: offload half the evictions
                if act_evict and nb == 1:
                    nc.scalar.copy(ot[:, h:h + TQB], mp[:])
                else:
                    nc.vector.tensor_copy(ot[:, h:h + TQB], mp[:])
                nc.sync.dma_start(out[t * 128:(t + 1) * 128, h:h + TQB],
                                  ot[:, h:h + TQB])

        def step_q(q):
            entry = q[0]
            try:
                clock["pe"] += next(entry[1])
                if len(entry) == 5:
                    entry[4][0] -= 1
                return True
            except StopIteration:
                q.popleft()
                return False

        quota = [0.0]

        def pace():
            while urgent and urgent[0][2] <= avtick[0]:
                step_q(urgent)
            if not background:
                return
            # EDF rationing: per tick emit just enough background steps that
            # every entry finishes by its deadline, spread uniformly
            tick = avtick[0]
            cum, rate = 0.0, 0.0
            for entry in background:
                cum += entry[4][0]
                rate = max(rate, cum / max(entry[3] - tick, 1.0))
            quota[0] = min(quota[0] + rate, 8.0)
            while (background and quota[0] >= 1.0
                   and background[0][2] <= avtick[0]):
                if step_q(background):
                    quota[0] -= 1.0

        def force_drain(q, pred):
            """Fully emit all entries of q matching pred (FIFO order, so
            everything queued before them drains too)."""
            while any(pred(e[0]) for e in q):
                step_q(q)

        def drain_fillers():
            while urgent:
                step_q(urgent)
            while background:
                step_q(background)

        # ---------------- attention block ---------------------------------
        def emit_attn_block(p, jb, bi):
            n_tk = NJB * (jb + 1)
            av = ps_tile("av", "psav")
            r_t = [r_pool.tile([128, NJB], F32, tag="r", name=f"r{h}")
                   for h in range(2)]
            usb = [None] * NJB
            pend = deque()

            def emit_av():
                avtick[0] += 1
                t, ee = pend.popleft()
                o = t - NJB * jb
                for h in range(2):
                    hb = h * 512
                    vcol = t * H_LOC * VW + (2 * p + h) * VW
                    for cch in range(max(o, 0), NJB):
                        # one accumulation group per PSUM bank per block:
                        # start only on the round's first matmul (start
                        # marks the whole 2KB zero-region pending; later
                        # chunks are lazily zeroed on first write), stop
                        # only on the last (tile n_tk-1 touches chunk 3
                        # alone)
                        nc.tensor.matmul(
                            av[:, hb + cch * VW:hb + cch * VW + VW],
                            ee[:, hb + cch * 128:hb + (cch + 1) * 128],
                            vaug[:, vcol:vcol + VW],
                            start=(t == 0 and cch == max(o, 0)),
                            stop=(t == n_tk - 1 and cch == NJB - 1))
                        clock["pe"] += VW * PE_C
                # after the bank groups stop (last tile): normalize + evict
                # all chunks, then queue their transposes / projections
                if t == n_tk - 1:
                    for h in range(2):
                        hb = h * 512
                        nc.vector.reciprocal(
                            r_t[h][:],
                            av[:, hb + 64:hb + NJB * VW:VW])
                    for cch in range(NJB):
                        usb[cch] = usb_pool.tile(
                            [128, 128], BF16, tag="usb", name=f"usb{cch}")
                        for h in range(2):
                            hb = h * 512
                            eng = evict_engine(cch + h,
                                               with_act=(p == HP - 1))
                            eng.tensor_scalar(
                                usb[cch][:, h * 64:(h + 1) * 64],
                                av[:, hb + cch * VW:hb + cch * VW + 64],
                                r_t[h][:, cch:cch + 1], None, AL.mult)
                        urgent.append((("t", bi),
                                       gen_transp_chunk(p, jb, cch, usb[cch]),
                                       avtick[0] + 3 + cch))
                        if p == HP - 1:
                            # drain each block's proj within the following
                            # pair-3 block (last: by end)
                            dl = {0: 132, 1: 144, 2: 160, 3: 160}[jb]
                            background.append(
                                (("proj", jb),
                                 gen_proj_tile(NJB * jb + cch, jb == NJB - 1),
                                 avtick[0] + 4 + cch, dl, [9]))

            for t in range(n_tk):
                if len(pend) > LAG:
                    emit_av()
                o = t - NJB * jb
                psS = ps_tile("pss", "psS")
                lo = max(o, 0) * 128
                for h in range(2):
                    hb = h * 512
                    kc = kT[p][h * 64:(h + 1) * 64, t * 128:(t + 1) * 128]
                    if o < 0:
                        nc.tensor.matmul(
                            psS[:, hb:hb + 512], kc,
                            qT[p][h * 64:(h + 1) * 64,
                                  jb * TQB:(jb + 1) * TQB],
                            start=True, stop=True)
                    else:
                        # diagonal tile: -1e30 mask bias folded into the
                        # accumulation, then scores on top. One group per
                        # bank: start only on the first matmul (the rest
                        # region is lazily zeroed on write), stop on last.
                        nc.tensor.matmul(
                            psS[:, hb + lo:hb + lo + 128],
                            ident_sb[:], mask_sb[:],
                            start=True, stop=False)
                        nc.tensor.matmul(
                            psS[:, hb + lo:hb + lo + 128], kc,
                            qT[p][h * 64:(h + 1) * 64,
                                  jb * TQB + lo:jb * TQB + lo + 128],
                            start=False, stop=(o == NJB - 1))
                        clock["pe"] += 256 * PE_C
                        if o < NJB - 1:
                            nc.tensor.matmul(
                                psS[:, hb + lo + 128:hb + 512], kc,
                                qT[p][h * 64:(h + 1) * 64,
                                      jb * TQB + lo + 128:(jb + 1) * TQB],
                                start=False, stop=True)
                    clock["pe"] += (512 - lo - (128 if o >= 0 else 0)) * PE_C
                ee = ee_pool.tile([128, 1024], BF16, tag="ee", name="ee")
                if o <= 0:
                    nc.scalar.activation(ee[:], psS[:], AF.Exp, scale=SCALE)
                    clock["act"] += _exp_ns(1024)
                else:
                    for h in range(2):
                        nc.scalar.activation(
                            ee[:, h * 512 + lo:h * 512 + 512],
                            psS[:, h * 512 + lo:h * 512 + 512],
                            AF.Exp, scale=SCALE)
                    clock["act"] += _exp_ns(2 * (512 - lo), nops=2)
                pend.append((t, ee))
                pace()
            while pend:
                emit_av()

        # ---------------- emission ----------------------------------------
        emit_v_pass_a()
        emit_qk_block(wqt, qT, 0, 0, evict_engine(0, True))
        emit_qk_block(wqt, qT, 0, 2, evict_engine(1, True))
        emit_qk_block(wkt, kT, 0, 0, evict_engine(0, True))
        emit_qk_block(wkt, kT, 0, 2, evict_engine(1, True))

        for tg in range(8, TT):
            jbn = tg // 4
            dl = max(2 * jbn * (jbn + 1) - 2, 1)
            background.append((("vb", tg), gen_v_group_b(tg), 0, dl, [9]))
        for p in range(1, HP):
            for jb in range(NJB):
                s_blk = 40 * p + 2 * jb * (jb + 1)
                background.append(
                    (("qk", p, jb), gen_qk_fill(wqt, qT, p, jb), 0,
                     max(s_blk - 2, 1), [9]))
                background.append(
                    (("qk", p, jb), gen_qk_fill(wkt, kT, p, jb), 0,
                     max(s_blk + 4 * jb - 2, 1), [9]))

        bi = 0
        for p in range(HP):
            for jb in range(NJB):
                # correctness: everything this block consumes must already
                # be emitted (Tile deps follow emission order) — vaug tiles
                # for its key range, q/k of this pair; plus recycle old
                # transposes (usb pool depth) before new norms allocate.
                n_tk = NJB * (jb + 1)
                force_drain(background, lambda k, n=n_tk, p=p: (
                    (k[0] == "vb" and k[1] < n)
                    or (k[0] == "qk" and k[1] <= p)))
                force_drain(urgent, lambda k, bi=bi: (
                    k[0] == "t" and k[1] <= bi - 2))
                emit_attn_block(p, jb, bi)
                bi += 1
        drain_fillers()
    return nc


def make_host_inputs():
    import ml_dtypes
    ntri = np.where(np.arange(128)[None, :] < np.arange(128)[:, None],
                    -1.0e30, 0.0).astype(np.float32)
    masks = ntri.astype(ml_dtypes.bfloat16)
    ident = np.eye(128, dtype=np.float32).astype(ml_dtypes.bfloat16)
    return masks, ident


def shard_inputs(data, Wq, Wk, Wv, Wp):
    """Build the 8 per-core input maps from full inputs."""
    import ml_dtypes
    BF = ml_dtypes.bfloat16
    data = np.asarray(data, np.float32)
    Wq = np.asarray(Wq, np.float32)
    Wk = np.asarray(Wk, np.float32)
    Wv = np.asarray(Wv, np.float32)
    Wp = np.asarray(Wp, np.float32)
    masks, ident = make_host_inputs()
    in_maps = []
    for c in range(N_CORES):
        b, g = c // 2, c % 2
        hs = slice(g * H_LOC, (g + 1) * H_LOC)
        in_maps.append({
            "xT": np.ascontiguousarray(data[b].T).astype(BF),
            "wq": np.ascontiguousarray(
                Wq[hs].transpose(1, 0, 2).reshape(E, H_LOC * D)).astype(BF),
            "wk": np.ascontiguousarray(
                Wk[hs].transpose(1, 0, 2).reshape(E, H_LOC * D)).astype(BF),
            "wv": np.ascontiguousarray(
                Wv[hs].transpose(1, 0, 2).reshape(E, H_LOC * D)).astype(BF),
            "wp": np.ascontiguousarray(
                Wp[g * H_LOC * D:(g + 1) * H_LOC * D, :]).astype(BF),
            "masks": masks,
            "ident": ident,
        })
    return in_maps


_NC_CACHE = {}


def legalize_single_wait(nc):
    """This toolchain's walrus accepts at most ONE sync wait per engine
    instruction; Tile freely emits more. Split extra waits onto preceding
    same-engine NoOps (engine FIFOs make that equivalent)."""
    import bass_rust
    cnt = 0
    for f in nc.m.functions:
        for blk in f.blocks:
            new = []
            changed = False
            for inst in blk.instructions:
                si = inst.sync_info
                if si is not None and len(si.on_wait) > 1:
                    waits = list(si.on_wait)
                    for w in waits[:-1]:
                        nop = bass_rust.InstNoOp(name=f"legal_nop_{cnt}")
                        cnt += 1
                        nop.engine = inst.engine
                        nop.sync_info = bass_rust.SyncInfo(on_wait=[w],
                                                           on_update=[])
                        new.append(nop)
                    inst.sync_info = bass_rust.SyncInfo(
                        on_wait=[waits[-1]], on_update=list(si.on_update))
                    changed = True
                new.append(inst)
            if changed:
                blk.instructions = new
    return cnt


def get_nc():
    if "nc" not in _NC_CACHE:
        nc = bass.Bass("TRN2", target_bir_lowering=False, debug=False,
                       num_devices=N_CORES)
        build_program(nc)
        legalize_single_wait(nc)
        _NC_CACHE["nc"] = nc
    return _NC_CACHE["nc"]


def run(inputs, trace=False, **kw):
    """Run on the 8 NeuronCores; returns (full_output, BassKernelResults)."""
    from concourse.bass_utils import run_bass_kernel_spmd
    nc = get_nc()
    in_maps = shard_inputs(inputs["data"], inputs["Wq"], inputs["Wk"],
                           inputs["Wv"], inputs["Wp"])
    res = run_bass_kernel_spmd(nc, in_maps, core_ids=list(range(N_CORES)),
                               trace=trace, **kw)
    bp = np.asarray(inputs["bp"], np.float32)
    outf = np.empty((B, T, E), np.float32)
    for b in range(B):
        outf[b] = res.results[2 * b]["out"] + res.results[2 * b + 1]["out"] + bp
    return outf, res


def kernel(**inputs):
    out, _ = run(inputs)
    return out


# revision 64
# speedup vs baseline: 1.3709x; 1.0080x over previous
"""Trainium2 Bass kernel for multi-head causal attention + output projection.

Problem (hardcoded): B=4, T=2048, E=1024, H=16, D=64, float32.
  q = einsum('bte,hed->bhtd', data, Wq)   (same k, v)
  scores = q@k.T / sqrt(D), causal mask, softmax
  out = (attn @ v) concat-heads @ Wp + bp

Sharding across 8 NeuronCores: core c -> (batch b=c//2, head-group g=c%2).
Each core computes 8 heads of one batch and a partial projection with its
512-row slice of Wp; host sums the two partials per batch and adds bias.

Kernel structure (v2 — transposed attn@V):
  - all matmul operands bf16 (validated ~5e-3 rel err end to end)
  - scores kept transposed: psS[key, query] per key tile, 2 heads side by
    side; diagonal key tiles only compute query cols >= o*128, with the
    -1e30 causal mask folded into the PSUM accumulation as an extra
    ident.T@ntri matmul (no DVE masking, one PE->ACT->PE hop per tile)
  - attn@V with queries on PSUM partitions: stationary = exp weights
    [k, q-chunk], moving = V-augmented [k, 64+1] -> out [q, 65] per chunk
    (65-col moving beats the 512-col orientation ~2x in PE time); the ones
    column of V yields sum(exp) at col 64
  - PSUM zero-region rule: start=True lazily zeroes the whole 2KB bank, so
    each (bank, block) accumulation round has exactly one start (first
    matmul) and one stop (last matmul); PSUM is only read after the stop
  - normalization at block end: DVE reciprocal + per-partition
    tensor_scalar eviction, then a PE transpose ([q,c]->[c,q]) rebuilds
    olt[c, t] for the output projection
  - the PE is the overall bottleneck (~213us busy): remaining v/q/k
    projections, transposes and the output projection are drip-fed
    between key tiles by an EDF-rationed emission pacer so filler work
    interleaves with the ACT-paced attention cadence instead of bursting

PSUM discipline (8 banks):
  tag "pss"  x2 [128,1024]: score tiles (ping-pong) + head-phase groups
  tag "av"   x1 [128,1024]: per-block attn@V (A: cols 0:260, B: 512:772)
  tag "misc" x2 [128,512]:  fillers (v pass B, q/k blocks, transposes,
                            projection rounds), ping-pong hides WAR
"""

from collections import deque

import numpy as np

import concourse.bass as bass
import concourse.mybir as mybir
import concourse.tile as tile
from contextlib import ExitStack

F32 = mybir.dt.float32
BF16 = mybir.dt.bfloat16

# Full-problem constants
B, T, E, H, D = 4, 2048, 1024, 16, 64
N_CORES = 8
H_LOC = H // 2          # heads per core
HP = H_LOC // 2         # head pairs per core
SCALE = float(D) ** -0.5

C = H_LOC * D           # local concat width (512)
ET = E // 128           # embedding 128-tiles (8)
TT = T // 128           # token 128-tiles (16)
TQB = 512               # query-block width
NJB = T // TQB          # query blocks (4)
VW = 65                 # vaug per-head width (64 + ones col)
LAG = 2                 # tiles between exp and attn@V consumption (the
                        # loop emits AV one tile later -> effective 3)

# pacing constants (ns estimates mirroring the cost model)
PE_C = 1.0 / 2.4
ACT_C = 1.0 / 1.2


def _exp_ns(cols, nops=1):
    return cols * ACT_C + 185.0 * nops


def build_program(nc):
    AF = mybir.ActivationFunctionType
    AL = mybir.AluOpType

    xTd = nc.dram_tensor("xT", [E, T], BF16, kind="ExternalInput").ap()
    wqd = nc.dram_tensor("wq", [E, C], BF16, kind="ExternalInput").ap()
    wkd = nc.dram_tensor("wk", [E, C], BF16, kind="ExternalInput").ap()
    wvd = nc.dram_tensor("wv", [E, C], BF16, kind="ExternalInput").ap()
    wpd = nc.dram_tensor("wp", [C, E], BF16, kind="ExternalInput").ap()
    # ntri[r, g] = -1e30 where g < r else 0 (strict lower triangle)
    maskd = nc.dram_tensor("masks", [128, 128], BF16, kind="ExternalInput").ap()
    identd = nc.dram_tensor("ident", [128, 128], BF16, kind="ExternalInput").ap()
    out = nc.dram_tensor("out", [T, E], F32, kind="ExternalOutput").ap()

    with tile.TileContext(nc) as tc, ExitStack() as ctx:
        const = ctx.enter_context(tc.tile_pool(name="const", bufs=1))
        xt_pool = ctx.enter_context(tc.tile_pool(name="xt", bufs=ET))
        wv_pool = ctx.enter_context(tc.tile_pool(name="wvp", bufs=ET))
        wq_pool = ctx.enter_context(tc.tile_pool(name="wqp", bufs=ET))
        wk_pool = ctx.enter_context(tc.tile_pool(name="wkp", bufs=ET))
        vaug_pool = ctx.enter_context(tc.tile_pool(name="vaugp", bufs=1))
        qk_pool = ctx.enter_context(tc.tile_pool(name="qkp", bufs=2 * HP))
        ee_pool = ctx.enter_context(tc.tile_pool(name="eep", bufs=8))
        usb_pool = ctx.enter_context(tc.tile_pool(name="usbp", bufs=8))
        r_pool = ctx.enter_context(tc.tile_pool(name="rp", bufs=4))
        olt_pool = ctx.enter_context(tc.tile_pool(name="oltp", bufs=HP))
        wp_pool = ctx.enter_context(tc.tile_pool(name="wpp", bufs=HP))
        out_pool = ctx.enter_context(tc.tile_pool(name="outp", bufs=4))
        psum = ctx.enter_context(tc.tile_pool(name="ps", bufs=4, space="PSUM"))

        mask_sb = const.tile([128, 128], BF16, name="mask_sb")
        ident_sb = const.tile([128, 128], BF16, name="ident_sb")

        vaug = vaug_pool.tile([128, TT * H_LOC * VW], BF16, name="vaug")
        xt = [xt_pool.tile([128, T], BF16, tag="xt", name=f"xt{e}")
              for e in range(ET)]
        wvt = [wv_pool.tile([128, C], BF16, tag="wv", name=f"wvt{e}")
               for e in range(ET)]
        wqt = [wq_pool.tile([128, C], BF16, tag="wq", name=f"wqt{e}")
               for e in range(ET)]
        wkt = [wk_pool.tile([128, C], BF16, tag="wk", name=f"wkt{e}")
               for e in range(ET)]
        qT = [qk_pool.tile([128, T], BF16, tag="qk", name=f"qT{p}")
              for p in range(HP)]
        kT = [qk_pool.tile([128, T], BF16, tag="qk", name=f"kT{p}")
              for p in range(HP)]
        olt = [olt_pool.tile([128, T], BF16, tag="olt", name=f"olt{c}")
               for c in range(HP)]
        wpt = [wp_pool.tile([128, E], BF16, tag="wp", name=f"wpt{c}")
               for c in range(HP)]

        TAG_BUFS = {"pss": 2, "av": 1, "misc": 2}

        def ps_tile(tag, name):
            # pss/av slots are 2 banks ([128,1024] f32); misc slots 1 bank
            shape = [128, 512] if tag == "misc" else [128, 1024]
            return psum.tile(shape, F32, tag=tag, name=name,
                             bufs=TAG_BUFS[tag])

        # ---------------- input DMAs (order = consumption order) ----------
        # first v matmul needs only xt0[:,0:128]+wvt0: tiny first transfers
        nc.sync.dma_start(xt[0][:, 0:128], xTd[0:128, 0:128])
        nc.sync.dma_start(wvt[0][:], wvd[0:128, :])
        nc.sync.dma_start(xt[0][:, 128:T // 2], xTd[0:128, 128:T // 2])
        for e in range(1, ET):
            nc.sync.dma_start(wvt[e][:], wvd[e * 128:(e + 1) * 128, :])
            nc.sync.dma_start(xt[e][:, 0:T // 2],
                              xTd[e * 128:(e + 1) * 128, 0:T // 2])
        for e in range(ET):
            nc.sync.dma_start(wqt[e][:], wqd[e * 128:(e + 1) * 128, :])
        for e in range(ET):
            nc.sync.dma_start(xt[e][:, T // 2:T],
                              xTd[e * 128:(e + 1) * 128, T // 2:T])
        for e in range(ET):
            nc.sync.dma_start(wkt[e][:], wkd[e * 128:(e + 1) * 128, :])
        nc.sync.dma_start(mask_sb[:], maskd)
        nc.sync.dma_start(ident_sb[:], identd)
        for c in range(HP):
            nc.sync.dma_start(wpt[c][:], wpd[c * 128:(c + 1) * 128, :])

        # ones columns of vaug (data cols are fully overwritten by evicts)
        nc.vector.memset(vaug[:, 64:TT * H_LOC * VW:VW], 1.0)

        def evict_engine(i, with_act=False):
            # GPSIMD cannot access PSUM on this target: evictions are
            # DVE-only during attention, DVE/ACT alternating in phases
            # where the ACT (exp) is idle.
            engs = [nc.vector, nc.scalar] if with_act else [nc.vector]
            eng = engs[i % len(engs)]

            def copy(out_ap, in_ap, _eng=eng):
                if _eng is nc.scalar:
                    return _eng.copy(out_ap, in_ap)
                return _eng.tensor_copy(out_ap, in_ap)

            def ts(out_ap, in_ap, scalar, _unused, _op, _eng=eng):
                if _eng is nc.scalar:
                    return _eng.mul(out_ap, in_ap, scalar)
                return _eng.tensor_scalar(out_ap, in_ap, scalar, None, _op)
            return type("E", (), {"tensor_copy": staticmethod(copy),
                                  "tensor_scalar": staticmethod(ts)})

        # ---------------- v projection ------------------------------------
        def v_mm(pv, slot, e, t):
            nc.tensor.matmul(
                pv[:, slot * C:(slot + 1) * C],
                xt[e][:, t * 128:(t + 1) * 128],
                wvt[e][:],
                start=(e == 0), stop=(e == ET - 1))

        def v_evict(pv, slot, t, eng):
            base = t * H_LOC * VW
            dst = vaug[:, base:base + H_LOC * VW].rearrange(
                "p (h c) -> p h c", c=VW)[:, :, 0:64]
            src = pv[:, slot * C:(slot + 1) * C].rearrange(
                "p (h c) -> p h c", c=64)
            eng.tensor_copy(dst, src)

        # head phase: t 0..5 in 4 interleaved groups, then t 6..7
        def emit_v_pass_a():
            g0 = ps_tile("pss", "psv_a0")
            g1 = ps_tile("pss", "psv_a1")
            g2 = ps_tile("misc", "psv_a2")
            g3 = ps_tile("misc", "psv_a3")
            gs = [(g0, 0), (g0, 1), (g1, 0), (g1, 1), (g2, 0), (g3, 0)]
            for e in range(ET):
                for t in range(6):
                    v_mm(gs[t][0], gs[t][1], e, t)
            for t in range(6):
                v_evict(gs[t][0], gs[t][1], t, evict_engine(t, with_act=True))
            g4 = ps_tile("av", "psv_a4")
            for e in range(ET):
                for t in (6, 7):
                    v_mm(g4, t % 2, e, t)
            for t in (6, 7):
                v_evict(g4, t % 2, t, evict_engine(t, with_act=True))

        # ---------------- q/k projections (head phase, pss tag) -----------
        def emit_qk_block(wlist, dst, p, jbp, eng, tag="pss"):
            pq = ps_tile(tag, "psqk")
            for e in range(ET):
                for j in range(2):
                    jb = jbp + j
                    nc.tensor.matmul(
                        pq[:, j * TQB:(j + 1) * TQB],
                        wlist[e][:, p * 128:(p + 1) * 128],
                        xt[e][:, jb * TQB:(jb + 1) * TQB],
                        start=(e == 0), stop=(e == ET - 1))
            eng.tensor_copy(dst[p][:, jbp * TQB:(jbp + 2) * TQB], pq[:])

        # ---------------- pacer / filler machinery ------------------------
        # Two queues: `urgent` (per-chunk transposes — tiny, gate olt) and
        # `background` (v pass B, q/k projections, output projection —
        # clock-paced against the ACT (exp) bottleneck). Entries:
        # (key, gen, min_tick): min_tick delays emission until the DVE work
        # they depend on has had time to execute (avtick = AV emissions).
        clock = {"pe": 0.0, "act": 0.0}
        avtick = [0]
        urgent = deque()
        background = deque()

        def gen_v_group_b(t):
            pv = ps_tile("misc", f"psv_b{t}")
            for e in range(ET):
                v_mm(pv, 0, e, t)
                yield TQB * PE_C
            v_evict(pv, 0, t, evict_engine(t))

        def gen_qk_fill(wlist, dst, p, jb):
            pq = ps_tile("misc", "psqkf")
            for e in range(ET):
                nc.tensor.matmul(
                    pq[:],
                    wlist[e][:, p * 128:(p + 1) * 128],
                    xt[e][:, jb * TQB:(jb + 1) * TQB],
                    start=(e == 0), stop=(e == ET - 1))
                yield TQB * PE_C
            evict_engine(p + jb).tensor_copy(
                dst[p][:, jb * TQB:(jb + 1) * TQB], pq[:])

        def gen_transp_chunk(p, jb, c, usb_c):
            mt = psum.tile([128, 128], BF16, tag="misc", name="pstr",
                           bufs=TAG_BUFS["misc"])
            nc.tensor.transpose(mt[:], usb_c[:], ident_sb[:])
            evict_engine(c).tensor_copy(
                olt[p][:, jb * TQB + c * 128:jb * TQB + (c + 1) * 128],
                mt[:])
            yield 128 * PE_C

        def gen_proj_tile(t, act_evict=False):
            ot = out_pool.tile([128, E], F32, tag="out", name=f"ot{t}")
            for nb in range(2):
                mp = ps_tile("misc", "psproj")
                h = nb * TQB
                for cc in range(HP):
                    nc.tensor.matmul(
                        mp[:],
                        olt[cc][:, t * 128:(t + 1) * 128],
                        wpt[cc][:, h:h + TQB],
                        start=(cc == 0), stop=(cc == HP - 1))
                    yield TQB * PE_C
                # ACT is idle at the very end: offload half the evictions
                if act_evict and nb == 1:
                    nc.scalar.copy(ot[:, h:h + TQB], mp[:])
                else:
                    nc.vector.tensor_copy(ot[:, h:h + TQB], mp[:])
                nc.sync.dma_start(out[t * 128:(t + 1) * 128, h:h + TQB],
                                  ot[:, h:h + TQB])

        def step_q(q):
            entry = q[0]
            try:
                clock["pe"] += next(entry[1])
                if len(entry) == 5:
                    entry[4][0] -= 1
                return True
            except StopIteration:
                q.popleft()
                return False

        quota = [0.0]

        def pace():
            while urgent and urgent[0][2] <= avtick[0]:
                step_q(urgent)
            if not background:
                return
            # EDF rationing: per tick emit just enough background steps that
            # every entry finishes by its deadline, spread uniformly
            tick = avtick[0]
            cum, rate = 0.0, 0.0
            for entry in background:
                cum += entry[4][0]
                rate = max(rate, cum / max(entry[3] - tick, 1.0))
            quota[0] = min(quota[0] + max(rate, 1.5), 8.0)
            while (background and quota[0] >= 1.0
                   and background[0][2] <= avtick[0]):
                if step_q(background):
                    quota[0] -= 1.0

        def force_drain(q, pred):
            """Fully emit all entries of q matching pred (FIFO order, so
            everything queued before them drains too)."""
            while any(pred(e[0]) for e in q):
                step_q(q)

        def drain_fillers():
            while urgent:
                step_q(urgent)
            while background:
                step_q(background)

        # ---------------- attention block ---------------------------------
        def emit_attn_block(p, jb, bi):
            n_tk = NJB * (jb + 1)
            av = ps_tile("av", "psav")
            r_t = [r_pool.tile([128, NJB], F32, tag="r", name=f"r{h}")
                   for h in range(2)]
            usb = [None] * NJB
            pend = deque()

            def emit_av():
                avtick[0] += 1
                t, ee = pend.popleft()
                o = t - NJB * jb
                for h in range(2):
                    hb = h * 512
                    vcol = t * H_LOC * VW + (2 * p + h) * VW
                    for cch in range(max(o, 0), NJB):
                        # one accumulation group per PSUM bank per block:
                        # start only on the round's first matmul (start
                        # marks the whole 2KB zero-region pending; later
                        # chunks are lazily zeroed on first write), stop
                        # only on the last (tile n_tk-1 touches chunk 3
                        # alone)
                        nc.tensor.matmul(
                            av[:, hb + cch * VW:hb + cch * VW + VW],
                            ee[:, hb + cch * 128:hb + (cch + 1) * 128],
                            vaug[:, vcol:vcol + VW],
                            start=(t == 0 and cch == max(o, 0)),
                            stop=(t == n_tk - 1 and cch == NJB - 1))
                        clock["pe"] += VW * PE_C
                # after the bank groups stop (last tile): normalize + evict
                # all chunks, then queue their transposes / projections
                if t == n_tk - 1:
                    for h in range(2):
                        hb = h * 512
                        nc.vector.reciprocal(
                            r_t[h][:],
                            av[:, hb + 64:hb + NJB * VW:VW])
                    for cch in range(NJB):
                        usb[cch] = usb_pool.tile(
                            [128, 128], BF16, tag="usb", name=f"usb{cch}")
                        for h in range(2):
                            hb = h * 512
                            eng = evict_engine(cch + h,
                                               with_act=(p == HP - 1))
                            eng.tensor_scalar(
                                usb[cch][:, h * 64:(h + 1) * 64],
                                av[:, hb + cch * VW:hb + cch * VW + 64],
                                r_t[h][:, cch:cch + 1], None, AL.mult)
                        urgent.append((("t", bi),
                                       gen_transp_chunk(p, jb, cch, usb[cch]),
                                       avtick[0] + 3 + cch))
                        if p == HP - 1:
                            # drain each block's proj within the following
                            # pair-3 block (last: by end)
                            dl = {0: 132, 1: 144, 2: 160, 3: 160}[jb]
                            background.append(
                                (("proj", jb),
                                 gen_proj_tile(NJB * jb + cch, jb == NJB - 1),
                                 avtick[0] + 4 + cch, dl, [9]))

            for t in range(n_tk):
                if len(pend) > LAG:
                    emit_av()
                o = t - NJB * jb
                psS = ps_tile("pss", "psS")
                lo = max(o, 0) * 128
                for h in range(2):
                    hb = h * 512
                    kc = kT[p][h * 64:(h + 1) * 64, t * 128:(t + 1) * 128]
                    if o < 0:
                        nc.tensor.matmul(
                            psS[:, hb:hb + 512], kc,
                            qT[p][h * 64:(h + 1) * 64,
                                  jb * TQB:(jb + 1) * TQB],
                            start=True, stop=True)
                    else:
                        # diagonal tile: -1e30 mask bias folded into the
                        # accumulation, then scores on top. One group per
                        # bank: start only on the first matmul (the rest
                        # region is lazily zeroed on write), stop on last.
                        nc.tensor.matmul(
                            psS[:, hb + lo:hb + lo + 128],
                            ident_sb[:], mask_sb[:],
                            start=True, stop=False)
                        nc.tensor.matmul(
                            psS[:, hb + lo:hb + lo + 128], kc,
                            qT[p][h * 64:(h + 1) * 64,
                                  jb * TQB + lo:jb * TQB + lo + 128],
                            start=False, stop=(o == NJB - 1))
                        clock["pe"] += 256 * PE_C
                        if o < NJB - 1:
                            nc.tensor.matmul(
                                psS[:, hb + lo + 128:hb + 512], kc,
                                qT[p][h * 64:(h + 1) * 64,
                                      jb * TQB + lo + 128:(jb + 1) * TQB],
                                start=False, stop=True)
                    clock["pe"] += (512 - lo - (128 if o >= 0 else 0)) * PE_C
                ee = ee_pool.tile([128, 1024], BF16, tag="ee", name="ee")
                if o <= 0:
                    nc.scalar.activation(ee[:], psS[:], AF.Exp, scale=SCALE)
                    clock["act"] += _exp_ns(1024)
                else:
                    for h in range(2):
                        nc.scalar.activation(
                            ee[:, h * 512 + lo:h * 512 + 512],
                            psS[:, h * 512 + lo:h * 512 + 512],
                            AF.Exp, scale=SCALE)
                    clock["act"] += _exp_ns(2 * (512 - lo), nops=2)
                pend.append((t, ee))
                pace()
            while pend:
                emit_av()

        # ---------------- emission ----------------------------------------
        emit_v_pass_a()
        emit_qk_block(wqt, qT, 0, 0, evict_engine(0, True))
        emit_qk_block(wqt, qT, 0, 2, evict_engine(1, True))
        emit_qk_block(wkt, kT, 0, 0, evict_engine(0, True))
        emit_qk_block(wkt, kT, 0, 2, evict_engine(1, True))

        for tg in range(8, TT):
            jbn = tg // 4
            dl = max(2 * jbn * (jbn + 1) - 2, 1)
            background.append((("vb", tg), gen_v_group_b(tg), 0, dl, [9]))
        for p in range(1, HP):
            for jb in range(NJB):
                s_blk = 40 * p + 2 * jb * (jb + 1)
                background.append(
                    (("qk", p, jb), gen_qk_fill(wqt, qT, p, jb), 0,
                     max(s_blk - 2, 1), [9]))
                background.append(
                    (("qk", p, jb), gen_qk_fill(wkt, kT, p, jb), 0,
                     max(s_blk + 4 * jb - 2, 1), [9]))

        bi = 0
        for p in range(HP):
            for jb in range(NJB):
                # correctness: everything this block consumes must already
                # be emitted (Tile deps follow emission order) — vaug tiles
                # for its key range, q/k of this pair; plus recycle old
                # transposes (usb pool depth) before new norms allocate.
                n_tk = NJB * (jb + 1)
                force_drain(background, lambda k, n=n_tk, p=p: (
                    (k[0] == "vb" and k[1] < n)
                    or (k[0] == "qk" and k[1] <= p)))
                force_drain(urgent, lambda k, bi=bi: (
                    k[0] == "t" and k[1] <= bi - 2))
                emit_attn_block(p, jb, bi)
                bi += 1
        drain_fillers()
    return nc


def make_host_inputs():
    import ml_dtypes
    ntri = np.where(np.arange(128)[None, :] < np.arange(128)[:, None],
                    -1.0e30, 0.0).astype(np.float32)
    masks = ntri.astype(ml_dtypes.bfloat16)
    ident = np.eye(128, dtype=np.float32).astype(ml_dtypes.bfloat16)
    return masks, ident


def shard_inputs(data, Wq, Wk, Wv, Wp):
    """Build the 8 per-core input maps from full inputs."""
    import ml_dtypes
    BF = ml_dtypes.bfloat16
    data = np.asarray(data, np.float32)
    Wq = np.asarray(Wq, np.float32)
    Wk = np.asarray(Wk, np.float32)
    Wv = np.asarray(Wv, np.float32)
    Wp = np.asarray(Wp, np.float32)
    masks, ident = make_host_inputs()
    in_maps = []
    for c in range(N_CORES):
        b, g = c // 2, c % 2
        hs = slice(g * H_LOC, (g + 1) * H_LOC)
        in_maps.append({
            "xT": np.ascontiguousarray(data[b].T).astype(BF),
            "wq": np.ascontiguousarray(
                Wq[hs].transpose(1, 0, 2).reshape(E, H_LOC * D)).astype(BF),
            "wk": np.ascontiguousarray(
                Wk[hs].transpose(1, 0, 2).reshape(E, H_LOC * D)).astype(BF),
            "wv": np.ascontiguousarray(
                Wv[hs].transpose(1, 0, 2).reshape(E, H_LOC * D)).astype(BF),
            "wp": np.ascontiguousarray(
                Wp[g * H_LOC * D:(g + 1) * H_LOC * D, :]).astype(BF),
            "masks": masks,
            "ident": ident,
        })
    return in_maps


_NC_CACHE = {}


def legalize_single_wait(nc):
    """This toolchain's walrus accepts at most ONE sync wait per engine
    instruction; Tile freely emits more. Split extra waits onto preceding
    same-engine NoOps (engine FIFOs make that equivalent)."""
    import bass_rust
    cnt = 0
    for f in nc.m.functions:
        for blk in f.blocks:
            new = []
            changed = False
            for inst in blk.instructions:
                si = inst.sync_info
                if si is not None and len(si.on_wait) > 1:
                    waits = list(si.on_wait)
                    for w in waits[:-1]:
                        nop = bass_rust.InstNoOp(name=f"legal_nop_{cnt}")
                        cnt += 1
                        nop.engine = inst.engine
                        nop.sync_info = bass_rust.SyncInfo(on_wait=[w],
                                                           on_update=[])
                        new.append(nop)
                    inst.sync_info = bass_rust.SyncInfo(
                        on_wait=[waits[-1]], on_update=list(si.on_update))
                    changed = True
                new.append(inst)
            if changed:
                blk.instructions = new
    return cnt


def get_nc():
    if "nc" not in _NC_CACHE:
        nc = bass.Bass("TRN2", target_bir_lowering=False, debug=False,
                       num_devices=N_CORES)
        build_program(nc)
        legalize_single_wait(nc)
        _NC_CACHE["nc"] = nc
    return _NC_CACHE["nc"]


def run(inputs, trace=False, **kw):
    """Run on the 8 NeuronCores; returns (full_output, BassKernelResults)."""
    from concourse.bass_utils import run_bass_kernel_spmd
    nc = get_nc()
    in_maps = shard_inputs(inputs["data"], inputs["Wq"], inputs["Wk"],
                           inputs["Wv"], inputs["Wp"])
    res = run_bass_kernel_spmd(nc, in_maps, core_ids=list(range(N_CORES)),
                               trace=trace, **kw)
    bp = np.asarray(inputs["bp"], np.float32)
    outf = np.empty((B, T, E), np.float32)
    for b in range(B):
        outf[b] = res.results[2 * b]["out"] + res.results[2 * b + 1]["out"] + bp
    return outf, res


def kernel(**inputs):
    out, _ = run(inputs)
    return out


# revision 68
# speedup vs baseline: 1.3768x; 1.0043x over previous
"""Trainium2 Bass kernel for multi-head causal attention + output projection.

Problem (hardcoded): B=4, T=2048, E=1024, H=16, D=64, float32.
  q = einsum('bte,hed->bhtd', data, Wq)   (same k, v)
  scores = q@k.T / sqrt(D), causal mask, softmax
  out = (attn @ v) concat-heads @ Wp + bp

Sharding across 8 NeuronCores: core c -> (batch b=c//2, head-group g=c%2).
Each core computes 8 heads of one batch and a partial projection with its
512-row slice of Wp; host sums the two partials per batch and adds bias.

Kernel structure (v2 — transposed attn@V):
  - all matmul operands bf16 (validated ~5e-3 rel err end to end)
  - scores kept transposed: psS[key, query] per key tile, 2 heads side by
    side; diagonal key tiles only compute query cols >= o*128, with the
    -1e30 causal mask folded into the PSUM accumulation as an extra
    ident.T@ntri matmul (no DVE masking, one PE->ACT->PE hop per tile)
  - attn@V with queries on PSUM partitions: stationary = exp weights
    [k, q-chunk], moving = V-augmented [k, 64+1] -> out [q, 65] per chunk
    (65-col moving beats the 512-col orientation ~2x in PE time); the ones
    column of V yields sum(exp) at col 64
  - PSUM zero-region rule: start=True lazily zeroes the whole 2KB bank, so
    each (bank, block) accumulation round has exactly one start (first
    matmul) and one stop (last matmul); PSUM is only read after the stop
  - normalization at block end: DVE reciprocal + per-partition
    tensor_scalar eviction, then a PE transpose ([q,c]->[c,q]) rebuilds
    olt[c, t] for the output projection
  - the PE is the overall bottleneck (~213us busy): remaining v/q/k
    projections, transposes and the output projection are drip-fed
    between key tiles by an EDF-rationed emission pacer so filler work
    interleaves with the ACT-paced attention cadence instead of bursting

PSUM discipline (8 banks):
  tag "pss"  x2 [128,1024]: score tiles (ping-pong) + head-phase groups
  tag "av"   x1 [128,1024]: per-block attn@V (A: cols 0:260, B: 512:772)
  tag "misc" x2 [128,512]:  fillers (v pass B, q/k blocks, transposes,
                            projection rounds), ping-pong hides WAR
"""

from collections import deque

import numpy as np

import concourse.bass as bass
import concourse.mybir as mybir
import concourse.tile as tile
from contextlib import ExitStack

F32 = mybir.dt.float32
BF16 = mybir.dt.bfloat16

# Full-problem constants
B, T, E, H, D = 4, 2048, 1024, 16, 64
N_CORES = 8
H_LOC = H // 2          # heads per core
HP = H_LOC // 2         # head pairs per core
SCALE = float(D) ** -0.5

C = H_LOC * D           # local concat width (512)
ET = E // 128           # embedding 128-tiles (8)
TT = T // 128           # token 128-tiles (16)
TQB = 512               # query-block width
NJB = T // TQB          # query blocks (4)
VW = 65                 # vaug per-head width (64 + ones col)
LAG = 3                 # tiles between exp and attn@V consumption (the
                        # loop emits AV one tile later -> effective 4)

# pacing constants (ns estimates mirroring the cost model)
PE_C = 1.0 / 2.4
ACT_C = 1.0 / 1.2


def _exp_ns(cols, nops=1):
    return cols * ACT_C + 185.0 * nops


def build_program(nc):
    AF = mybir.ActivationFunctionType
    AL = mybir.AluOpType

    xTd = nc.dram_tensor("xT", [E, T], BF16, kind="ExternalInput").ap()
    wqd = nc.dram_tensor("wq", [E, C], BF16, kind="ExternalInput").ap()
    wkd = nc.dram_tensor("wk", [E, C], BF16, kind="ExternalInput").ap()
    wvd = nc.dram_tensor("wv", [E, C], BF16, kind="ExternalInput").ap()
    wpd = nc.dram_tensor("wp", [C, E], BF16, kind="ExternalInput").ap()
    # ntri[r, g] = -1e30 where g < r else 0 (strict lower triangle)
    maskd = nc.dram_tensor("masks", [128, 128], BF16, kind="ExternalInput").ap()
    identd = nc.dram_tensor("ident", [128, 128], BF16, kind="ExternalInput").ap()
    out = nc.dram_tensor("out", [T, E], F32, kind="ExternalOutput").ap()

    with tile.TileContext(nc) as tc, ExitStack() as ctx:
        const = ctx.enter_context(tc.tile_pool(name="const", bufs=1))
        xt_pool = ctx.enter_context(tc.tile_pool(name="xt", bufs=ET))
        wv_pool = ctx.enter_context(tc.tile_pool(name="wvp", bufs=ET))
        wq_pool = ctx.enter_context(tc.tile_pool(name="wqp", bufs=ET))
        wk_pool = ctx.enter_context(tc.tile_pool(name="wkp", bufs=ET))
        vaug_pool = ctx.enter_context(tc.tile_pool(name="vaugp", bufs=1))
        qk_pool = ctx.enter_context(tc.tile_pool(name="qkp", bufs=2 * HP))
        ee_pool = ctx.enter_context(tc.tile_pool(name="eep", bufs=8))
        usb_pool = ctx.enter_context(tc.tile_pool(name="usbp", bufs=8))
        r_pool = ctx.enter_context(tc.tile_pool(name="rp", bufs=4))
        olt_pool = ctx.enter_context(tc.tile_pool(name="oltp", bufs=HP))
        wp_pool = ctx.enter_context(tc.tile_pool(name="wpp", bufs=HP))
        out_pool = ctx.enter_context(tc.tile_pool(name="outp", bufs=4))
        psum = ctx.enter_context(tc.tile_pool(name="ps", bufs=4, space="PSUM"))

        mask_sb = const.tile([128, 128], BF16, name="mask_sb")
        ident_sb = const.tile([128, 128], BF16, name="ident_sb")

        vaug = vaug_pool.tile([128, TT * H_LOC * VW], BF16, name="vaug")
        xt = [xt_pool.tile([128, T], BF16, tag="xt", name=f"xt{e}")
              for e in range(ET)]
        wvt = [wv_pool.tile([128, C], BF16, tag="wv", name=f"wvt{e}")
               for e in range(ET)]
        wqt = [wq_pool.tile([128, C], BF16, tag="wq", name=f"wqt{e}")
               for e in range(ET)]
        wkt = [wk_pool.tile([128, C], BF16, tag="wk", name=f"wkt{e}")
               for e in range(ET)]
        qT = [qk_pool.tile([128, T], BF16, tag="qk", name=f"qT{p}")
              for p in range(HP)]
        kT = [qk_pool.tile([128, T], BF16, tag="qk", name=f"kT{p}")
              for p in range(HP)]
        olt = [olt_pool.tile([128, T], BF16, tag="olt", name=f"olt{c}")
               for c in range(HP)]
        wpt = [wp_pool.tile([128, E], BF16, tag="wp", name=f"wpt{c}")
               for c in range(HP)]

        TAG_BUFS = {"pss": 2, "av": 1, "misc": 2}

        def ps_tile(tag, name):
            # pss/av slots are 2 banks ([128,1024] f32); misc slots 1 bank
            shape = [128, 512] if tag == "misc" else [128, 1024]
            return psum.tile(shape, F32, tag=tag, name=name,
                             bufs=TAG_BUFS[tag])

        # ---------------- input DMAs (order = consumption order) ----------
        # first v matmul needs only xt0[:,0:128]+wvt0: tiny first transfers
        nc.sync.dma_start(xt[0][:, 0:128], xTd[0:128, 0:128])
        nc.sync.dma_start(wvt[0][:], wvd[0:128, :])
        nc.sync.dma_start(xt[0][:, 128:T // 2], xTd[0:128, 128:T // 2])
        for e in range(1, ET):
            nc.sync.dma_start(wvt[e][:], wvd[e * 128:(e + 1) * 128, :])
            nc.sync.dma_start(xt[e][:, 0:T // 2],
                              xTd[e * 128:(e + 1) * 128, 0:T // 2])
        for e in range(ET):
            nc.sync.dma_start(wqt[e][:], wqd[e * 128:(e + 1) * 128, :])
        for e in range(ET):
            nc.sync.dma_start(xt[e][:, T // 2:T],
                              xTd[e * 128:(e + 1) * 128, T // 2:T])
        for e in range(ET):
            nc.sync.dma_start(wkt[e][:], wkd[e * 128:(e + 1) * 128, :])
        nc.sync.dma_start(mask_sb[:], maskd)
        nc.sync.dma_start(ident_sb[:], identd)
        for c in range(HP):
            nc.sync.dma_start(wpt[c][:], wpd[c * 128:(c + 1) * 128, :])

        # ones columns of vaug (data cols are fully overwritten by evicts)
        nc.vector.memset(vaug[:, 64:TT * H_LOC * VW:VW], 1.0)

        def evict_engine(i, with_act=False):
            # GPSIMD cannot access PSUM on this target: evictions are
            # DVE-only during attention, DVE/ACT alternating in phases
            # where the ACT (exp) is idle.
            engs = [nc.vector, nc.scalar] if with_act else [nc.vector]
            eng = engs[i % len(engs)]

            def copy(out_ap, in_ap, _eng=eng):
                if _eng is nc.scalar:
                    return _eng.copy(out_ap, in_ap)
                return _eng.tensor_copy(out_ap, in_ap)

            def ts(out_ap, in_ap, scalar, _unused, _op, _eng=eng):
                if _eng is nc.scalar:
                    return _eng.mul(out_ap, in_ap, scalar)
                return _eng.tensor_scalar(out_ap, in_ap, scalar, None, _op)
            return type("E", (), {"tensor_copy": staticmethod(copy),
                                  "tensor_scalar": staticmethod(ts)})

        # ---------------- v projection ------------------------------------
        def v_mm(pv, slot, e, t):
            nc.tensor.matmul(
                pv[:, slot * C:(slot + 1) * C],
                xt[e][:, t * 128:(t + 1) * 128],
                wvt[e][:],
                start=(e == 0), stop=(e == ET - 1))

        def v_evict(pv, slot, t, eng):
            base = t * H_LOC * VW
            dst = vaug[:, base:base + H_LOC * VW].rearrange(
                "p (h c) -> p h c", c=VW)[:, :, 0:64]
            src = pv[:, slot * C:(slot + 1) * C].rearrange(
                "p (h c) -> p h c", c=64)
            eng.tensor_copy(dst, src)

        # head phase: t 0..5 in 4 interleaved groups, then t 6..7
        def emit_v_pass_a():
            g0 = ps_tile("pss", "psv_a0")
            g1 = ps_tile("pss", "psv_a1")
            g2 = ps_tile("misc", "psv_a2")
            g3 = ps_tile("misc", "psv_a3")
            gs = [(g0, 0), (g0, 1), (g1, 0), (g1, 1), (g2, 0), (g3, 0)]
            for e in range(ET):
                for t in range(6):
                    v_mm(gs[t][0], gs[t][1], e, t)
            for t in range(6):
                v_evict(gs[t][0], gs[t][1], t, evict_engine(t, with_act=True))
            g4 = ps_tile("av", "psv_a4")
            for e in range(ET):
                for t in (6, 7):
                    v_mm(g4, t % 2, e, t)
            for t in (6, 7):
                v_evict(g4, t % 2, t, evict_engine(t, with_act=True))

        # ---------------- q/k projections (head phase, pss tag) -----------
        def emit_qk_block(wlist, dst, p, jbp, eng, tag="pss"):
            pq = ps_tile(tag, "psqk")
            for e in range(ET):
                for j in range(2):
                    jb = jbp + j
                    nc.tensor.matmul(
                        pq[:, j * TQB:(j + 1) * TQB],
                        wlist[e][:, p * 128:(p + 1) * 128],
                        xt[e][:, jb * TQB:(jb + 1) * TQB],
                        start=(e == 0), stop=(e == ET - 1))
            eng.tensor_copy(dst[p][:, jbp * TQB:(jbp + 2) * TQB], pq[:])

        # ---------------- pacer / filler machinery ------------------------
        # Two queues: `urgent` (per-chunk transposes — tiny, gate olt) and
        # `background` (v pass B, q/k projections, output projection —
        # clock-paced against the ACT (exp) bottleneck). Entries:
        # (key, gen, min_tick): min_tick delays emission until the DVE work
        # they depend on has had time to execute (avtick = AV emissions).
        clock = {"pe": 0.0, "act": 0.0}
        avtick = [0]
        urgent = deque()
        background = deque()

        def gen_v_group_b(t):
            pv = ps_tile("misc", f"psv_b{t}")
            for e in range(ET):
                v_mm(pv, 0, e, t)
                yield TQB * PE_C
            v_evict(pv, 0, t, evict_engine(t))

        def gen_qk_fill(wlist, dst, p, jb):
            pq = ps_tile("misc", "psqkf")
            for e in range(ET):
                nc.tensor.matmul(
                    pq[:],
                    wlist[e][:, p * 128:(p + 1) * 128],
                    xt[e][:, jb * TQB:(jb + 1) * TQB],
                    start=(e == 0), stop=(e == ET - 1))
                yield TQB * PE_C
            evict_engine(p + jb).tensor_copy(
                dst[p][:, jb * TQB:(jb + 1) * TQB], pq[:])

        def gen_transp_chunk(p, jb, c, usb_c):
            mt = psum.tile([128, 128], BF16, tag="misc", name="pstr",
                           bufs=TAG_BUFS["misc"])
            nc.tensor.transpose(mt[:], usb_c[:], ident_sb[:])
            evict_engine(c).tensor_copy(
                olt[p][:, jb * TQB + c * 128:jb * TQB + (c + 1) * 128],
                mt[:])
            yield 128 * PE_C

        def gen_proj_tile(t, act_evict=False):
            ot = out_pool.tile([128, E], F32, tag="out", name=f"ot{t}")
            for nb in range(2):
                mp = ps_tile("misc", "psproj")
                h = nb * TQB
                for cc in range(HP):
                    nc.tensor.matmul(
                        mp[:],
                        olt[cc][:, t * 128:(t + 1) * 128],
                        wpt[cc][:, h:h + TQB],
                        start=(cc == 0), stop=(cc == HP - 1))
                    yield TQB * PE_C
                # ACT is idle at the very end: offload half the evictions
                if act_evict and nb == 1:
                    nc.scalar.copy(ot[:, h:h + TQB], mp[:])
                else:
                    nc.vector.tensor_copy(ot[:, h:h + TQB], mp[:])
                nc.sync.dma_start(out[t * 128:(t + 1) * 128, h:h + TQB],
                                  ot[:, h:h + TQB])

        def step_q(q):
            entry = q[0]
            try:
                clock["pe"] += next(entry[1])
                if len(entry) == 5:
                    entry[4][0] -= 1
                return True
            except StopIteration:
                q.popleft()
                return False

        quota = [0.0]

        def pace():
            while urgent and urgent[0][2] <= avtick[0]:
                step_q(urgent)
            if not background:
                return
            # EDF rationing: per tick emit just enough background steps that
            # every entry finishes by its deadline, spread uniformly
            tick = avtick[0]
            cum, rate = 0.0, 0.0
            for entry in background:
                cum += entry[4][0]
                rate = max(rate, cum / max(entry[3] - tick, 1.0))
            quota[0] = min(quota[0] + max(rate, 1.5), 8.0)
            while (background and quota[0] >= 1.0
                   and background[0][2] <= avtick[0]):
                if step_q(background):
                    quota[0] -= 1.0

        def force_drain(q, pred):
            """Fully emit all entries of q matching pred (FIFO order, so
            everything queued before them drains too)."""
            while any(pred(e[0]) for e in q):
                step_q(q)

        def drain_fillers():
            while urgent:
                step_q(urgent)
            while background:
                step_q(background)

        # ---------------- attention block ---------------------------------
        def emit_attn_block(p, jb, bi):
            n_tk = NJB * (jb + 1)
            av = ps_tile("av", "psav")
            r_t = [r_pool.tile([128, NJB], F32, tag="r", name=f"r{h}")
                   for h in range(2)]
            usb = [None] * NJB
            pend = deque()

            def emit_av():
                avtick[0] += 1
                t, ee = pend.popleft()
                o = t - NJB * jb
                for h in range(2):
                    hb = h * 512
                    vcol = t * H_LOC * VW + (2 * p + h) * VW
                    for cch in range(max(o, 0), NJB):
                        # one accumulation group per PSUM bank per block:
                        # start only on the round's first matmul (start
                        # marks the whole 2KB zero-region pending; later
                        # chunks are lazily zeroed on first write), stop
                        # only on the last (tile n_tk-1 touches chunk 3
                        # alone)
                        nc.tensor.matmul(
                            av[:, hb + cch * VW:hb + cch * VW + VW],
                            ee[:, hb + cch * 128:hb + (cch + 1) * 128],
                            vaug[:, vcol:vcol + VW],
                            start=(t == 0 and cch == max(o, 0)),
                            stop=(t == n_tk - 1 and cch == NJB - 1))
                        clock["pe"] += VW * PE_C
                # after the bank groups stop (last tile): normalize + evict
                # all chunks, then queue their transposes / projections
                if t == n_tk - 1:
                    for h in range(2):
                        hb = h * 512
                        nc.vector.reciprocal(
                            r_t[h][:],
                            av[:, hb + 64:hb + NJB * VW:VW])
                    for cch in range(NJB):
                        usb[cch] = usb_pool.tile(
                            [128, 128], BF16, tag="usb", name=f"usb{cch}")
                        for h in range(2):
                            hb = h * 512
                            eng = evict_engine(cch + h,
                                               with_act=(p == HP - 1))
                            eng.tensor_scalar(
                                usb[cch][:, h * 64:(h + 1) * 64],
                                av[:, hb + cch * VW:hb + cch * VW + 64],
                                r_t[h][:, cch:cch + 1], None, AL.mult)
                        urgent.append((("t", bi),
                                       gen_transp_chunk(p, jb, cch, usb[cch]),
                                       avtick[0] + 3 + cch))
                        if p == HP - 1:
                            # drain each block's proj within the following
                            # pair-3 block (last: by end)
                            dl = {0: 132, 1: 144, 2: 160, 3: 160}[jb]
                            background.append(
                                (("proj", jb),
                                 gen_proj_tile(NJB * jb + cch, jb == NJB - 1),
                                 avtick[0] + 4 + cch, dl, [9]))

            for t in range(n_tk):
                if len(pend) > LAG:
                    emit_av()
                o = t - NJB * jb
                psS = ps_tile("pss", "psS")
                lo = max(o, 0) * 128
                for h in range(2):
                    hb = h * 512
                    kc = kT[p][h * 64:(h + 1) * 64, t * 128:(t + 1) * 128]
                    if o < 0:
                        nc.tensor.matmul(
                            psS[:, hb:hb + 512], kc,
                            qT[p][h * 64:(h + 1) * 64,
                                  jb * TQB:(jb + 1) * TQB],
                            start=True, stop=True)
                    else:
                        # diagonal tile: -1e30 mask bias folded into the
                        # accumulation, then scores on top. One group per
                        # bank: start only on the first matmul (the rest
                        # region is lazily zeroed on write), stop on last.
                        nc.tensor.matmul(
                            psS[:, hb + lo:hb + lo + 128],
                            ident_sb[:], mask_sb[:],
                            start=True, stop=False)
                        nc.tensor.matmul(
                            psS[:, hb + lo:hb + lo + 128], kc,
                            qT[p][h * 64:(h + 1) * 64,
                                  jb * TQB + lo:jb * TQB + lo + 128],
                            start=False, stop=(o == NJB - 1))
                        clock["pe"] += 256 * PE_C
                        if o < NJB - 1:
                            nc.tensor.matmul(
                                psS[:, hb + lo + 128:hb + 512], kc,
                                qT[p][h * 64:(h + 1) * 64,
                                      jb * TQB + lo + 128:(jb + 1) * TQB],
                                start=False, stop=True)
                    clock["pe"] += (512 - lo - (128 if o >= 0 else 0)) * PE_C
                ee = ee_pool.tile([128, 1024], BF16, tag="ee", name="ee")
                if o <= 0:
                    nc.scalar.activation(ee[:], psS[:], AF.Exp, scale=SCALE)
                    clock["act"] += _exp_ns(1024)
                else:
                    for h in range(2):
                        nc.scalar.activation(
                            ee[:, h * 512 + lo:h * 512 + 512],
                            psS[:, h * 512 + lo:h * 512 + 512],
                            AF.Exp, scale=SCALE)
                    clock["act"] += _exp_ns(2 * (512 - lo), nops=2)
                pend.append((t, ee))
                pace()
            while pend:
                emit_av()

        # ---------------- emission ----------------------------------------
        emit_v_pass_a()
        emit_qk_block(wqt, qT, 0, 0, evict_engine(0, True))
        emit_qk_block(wqt, qT, 0, 2, evict_engine(1, True))
        emit_qk_block(wkt, kT, 0, 0, evict_engine(0, True))
        emit_qk_block(wkt, kT, 0, 2, evict_engine(1, True))

        for tg in range(8, TT):
            jbn = tg // 4
            dl = max(2 * jbn * (jbn + 1) - 2, 1)
            background.append((("vb", tg), gen_v_group_b(tg), 0, dl, [9]))
        for p in range(1, HP):
            for jb in range(NJB):
                s_blk = 40 * p + 2 * jb * (jb + 1)
                background.append(
                    (("qk", p, jb), gen_qk_fill(wqt, qT, p, jb), 0,
                     max(s_blk - 2, 1), [9]))
                background.append(
                    (("qk", p, jb), gen_qk_fill(wkt, kT, p, jb), 0,
                     max(s_blk + 4 * jb - 2, 1), [9]))

        bi = 0
        for p in range(HP):
            for jb in range(NJB):
                # correctness: everything this block consumes must already
                # be emitted (Tile deps follow emission order) — vaug tiles
                # for its key range, q/k of this pair; plus recycle old
                # transposes (usb pool depth) before new norms allocate.
                n_tk = NJB * (jb + 1)
                force_drain(background, lambda k, n=n_tk, p=p: (
                    (k[0] == "vb" and k[1] < n)
                    or (k[0] == "qk" and k[1] <= p)))
                force_drain(urgent, lambda k, bi=bi: (
                    k[0] == "t" and k[1] <= bi - 2))
                emit_attn_block(p, jb, bi)
                bi += 1
        drain_fillers()
    return nc


def make_host_inputs():
    import ml_dtypes
    ntri = np.where(np.arange(128)[None, :] < np.arange(128)[:, None],
                    -1.0e30, 0.0).astype(np.float32)
    masks = ntri.astype(ml_dtypes.bfloat16)
    ident = np.eye(128, dtype=np.float32).astype(ml_dtypes.bfloat16)
    return masks, ident


def shard_inputs(data, Wq, Wk, Wv, Wp):
    """Build the 8 per-core input maps from full inputs."""
    import ml_dtypes
    BF = ml_dtypes.bfloat16
    data = np.asarray(data, np.float32)
    Wq = np.asarray(Wq, np.float32)
    Wk = np.asarray(Wk, np.float32)
    Wv = np.asarray(Wv, np.float32)
    Wp = np.asarray(Wp, np.float32)
    masks, ident = make_host_inputs()
    in_maps = []
    for c in range(N_CORES):
        b, g = c // 2, c % 2
        hs = slice(g * H_LOC, (g + 1) * H_LOC)
        in_maps.append({
            "xT": np.ascontiguousarray(data[b].T).astype(BF),
            "wq": np.ascontiguousarray(
                Wq[hs].transpose(1, 0, 2).reshape(E, H_LOC * D)).astype(BF),
            "wk": np.ascontiguousarray(
                Wk[hs].transpose(1, 0, 2).reshape(E, H_LOC * D)).astype(BF),
            "wv": np.ascontiguousarray(
                Wv[hs].transpose(1, 0, 2).reshape(E, H_LOC * D)).astype(BF),
            "wp": np.ascontiguousarray(
                Wp[g * H_LOC * D:(g + 1) * H_LOC * D, :]).astype(BF),
            "masks": masks,
            "ident": ident,
        })
    return in_maps


_NC_CACHE = {}


def legalize_single_wait(nc):
    """This toolchain's walrus accepts at most ONE sync wait per engine
    instruction; Tile freely emits more. Split extra waits onto preceding
    same-engine NoOps (engine FIFOs make that equivalent)."""
    import bass_rust
    cnt = 0
    for f in nc.m.functions:
        for blk in f.blocks:
            new = []
            changed = False
            for inst in blk.instructions:
                si = inst.sync_info
                if si is not None and len(si.on_wait) > 1:
                    waits = list(si.on_wait)
                    for w in waits[:-1]:
                        nop = bass_rust.InstNoOp(name=f"legal_nop_{cnt}")
                        cnt += 1
                        nop.engine = inst.engine
                        nop.sync_info = bass_rust.SyncInfo(on_wait=[w],
                                                           on_update=[])
                        new.append(nop)
                    inst.sync_info = bass_rust.SyncInfo(
                        on_wait=[waits[-1]], on_update=list(si.on_update))
                    changed = True
                new.append(inst)
            if changed:
                blk.instructions = new
    return cnt


def get_nc():
    if "nc" not in _NC_CACHE:
        nc = bass.Bass("TRN2", target_bir_lowering=False, debug=False,
                       num_devices=N_CORES)
        build_program(nc)
        legalize_single_wait(nc)
        _NC_CACHE["nc"] = nc
    return _NC_CACHE["nc"]


def run(inputs, trace=False, **kw):
    """Run on the 8 NeuronCores; returns (full_output, BassKernelResults)."""
    from concourse.bass_utils import run_bass_kernel_spmd
    nc = get_nc()
    in_maps = shard_inputs(inputs["data"], inputs["Wq"], inputs["Wk"],
                           inputs["Wv"], inputs["Wp"])
    res = run_bass_kernel_spmd(nc, in_maps, core_ids=list(range(N_CORES)),
                               trace=trace, **kw)
    bp = np.asarray(inputs["bp"], np.float32)
    outf = np.empty((B, T, E), np.float32)
    for b in range(B):
        outf[b] = res.results[2 * b]["out"] + res.results[2 * b + 1]["out"] + bp
    return outf, res


def kernel(**inputs):
    out, _ = run(inputs)
    return out


# revision 69
# speedup vs baseline: 1.3908x; 1.0102x over previous
"""Trainium2 Bass kernel for multi-head causal attention + output projection.

Problem (hardcoded): B=4, T=2048, E=1024, H=16, D=64, float32.
  q = einsum('bte,hed->bhtd', data, Wq)   (same k, v)
  scores = q@k.T / sqrt(D), causal mask, softmax
  out = (attn @ v) concat-heads @ Wp + bp

Sharding across 8 NeuronCores: core c -> (batch b=c//2, head-group g=c%2).
Each core computes 8 heads of one batch and a partial projection with its
512-row slice of Wp; host sums the two partials per batch and adds bias.

Kernel structure (v2 — transposed attn@V):
  - all matmul operands bf16 (validated ~5e-3 rel err end to end)
  - scores kept transposed: psS[key, query] per key tile, 2 heads side by
    side; diagonal key tiles only compute query cols >= o*128, with the
    -1e30 causal mask folded into the PSUM accumulation as an extra
    ident.T@ntri matmul (no DVE masking, one PE->ACT->PE hop per tile)
  - attn@V with queries on PSUM partitions: stationary = exp weights
    [k, q-chunk], moving = V-augmented [k, 64+1] -> out [q, 65] per chunk
    (65-col moving beats the 512-col orientation ~2x in PE time); the ones
    column of V yields sum(exp) at col 64
  - PSUM zero-region rule: start=True lazily zeroes the whole 2KB bank, so
    each (bank, block) accumulation round has exactly one start (first
    matmul) and one stop (last matmul); PSUM is only read after the stop
  - normalization at block end: DVE reciprocal + per-partition
    tensor_scalar eviction, then a PE transpose ([q,c]->[c,q]) rebuilds
    olt[c, t] for the output projection
  - the PE is the overall bottleneck (~213us busy): remaining v/q/k
    projections, transposes and the output projection are drip-fed
    between key tiles by an EDF-rationed emission pacer so filler work
    interleaves with the ACT-paced attention cadence instead of bursting

PSUM discipline (8 banks):
  tag "pss"  x2 [128,1024]: score tiles (ping-pong) + head-phase groups
  tag "av"   x1 [128,1024]: per-block attn@V (A: cols 0:260, B: 512:772)
  tag "misc" x2 [128,512]:  fillers (v pass B, q/k blocks, transposes,
                            projection rounds), ping-pong hides WAR
"""

from collections import deque

import numpy as np

import concourse.bass as bass
import concourse.mybir as mybir
import concourse.tile as tile
from contextlib import ExitStack

F32 = mybir.dt.float32
BF16 = mybir.dt.bfloat16

# Full-problem constants
B, T, E, H, D = 4, 2048, 1024, 16, 64
N_CORES = 8
H_LOC = H // 2          # heads per core
HP = H_LOC // 2         # head pairs per core
SCALE = float(D) ** -0.5

C = H_LOC * D           # local concat width (512)
ET = E // 128           # embedding 128-tiles (8)
TT = T // 128           # token 128-tiles (16)
TQB = 512               # query-block width
NJB = T // TQB          # query blocks (4)
VW = 65                 # vaug per-head width (64 + ones col)
LAG = 3                 # tiles between exp and attn@V consumption (the
                        # loop emits AV one tile later -> effective 4)

# pacing constants (ns estimates mirroring the cost model)
PE_C = 1.0 / 2.4
ACT_C = 1.0 / 1.2


def _exp_ns(cols, nops=1):
    return cols * ACT_C + 185.0 * nops


def build_program(nc):
    AF = mybir.ActivationFunctionType
    AL = mybir.AluOpType

    xTd = nc.dram_tensor("xT", [E, T], BF16, kind="ExternalInput").ap()
    wqd = nc.dram_tensor("wq", [E, C], BF16, kind="ExternalInput").ap()
    wkd = nc.dram_tensor("wk", [E, C], BF16, kind="ExternalInput").ap()
    wvd = nc.dram_tensor("wv", [E, C], BF16, kind="ExternalInput").ap()
    wpd = nc.dram_tensor("wp", [C, E], BF16, kind="ExternalInput").ap()
    # [128,256]: two copies of tri[r, g] = (g >= r)
    maskd = nc.dram_tensor("masks", [128, 256], BF16, kind="ExternalInput").ap()
    identd = nc.dram_tensor("ident", [128, 128], BF16, kind="ExternalInput").ap()
    out = nc.dram_tensor("out", [T, E], F32, kind="ExternalOutput").ap()

    with tile.TileContext(nc) as tc, ExitStack() as ctx:
        const = ctx.enter_context(tc.tile_pool(name="const", bufs=1))
        xt_pool = ctx.enter_context(tc.tile_pool(name="xt", bufs=ET))
        wv_pool = ctx.enter_context(tc.tile_pool(name="wvp", bufs=ET))
        wq_pool = ctx.enter_context(tc.tile_pool(name="wqp", bufs=ET))
        wk_pool = ctx.enter_context(tc.tile_pool(name="wkp", bufs=ET))
        vaug_pool = ctx.enter_context(tc.tile_pool(name="vaugp", bufs=1))
        qk_pool = ctx.enter_context(tc.tile_pool(name="qkp", bufs=2 * HP))
        ee_pool = ctx.enter_context(tc.tile_pool(name="eep", bufs=8))
        usb_pool = ctx.enter_context(tc.tile_pool(name="usbp", bufs=8))
        r_pool = ctx.enter_context(tc.tile_pool(name="rp", bufs=4))
        olt_pool = ctx.enter_context(tc.tile_pool(name="oltp", bufs=HP))
        wp_pool = ctx.enter_context(tc.tile_pool(name="wpp", bufs=HP))
        out_pool = ctx.enter_context(tc.tile_pool(name="outp", bufs=4))
        psum = ctx.enter_context(tc.tile_pool(name="ps", bufs=4, space="PSUM"))

        mask_sb = const.tile([128, 256], BF16, name="mask_sb")
        ident_sb = const.tile([128, 128], BF16, name="ident_sb")

        vaug = vaug_pool.tile([128, TT * H_LOC * VW], BF16, name="vaug")
        xt = [xt_pool.tile([128, T], BF16, tag="xt", name=f"xt{e}")
              for e in range(ET)]
        wvt = [wv_pool.tile([128, C], BF16, tag="wv", name=f"wvt{e}")
               for e in range(ET)]
        wqt = [wq_pool.tile([128, C], BF16, tag="wq", name=f"wqt{e}")
               for e in range(ET)]
        wkt = [wk_pool.tile([128, C], BF16, tag="wk", name=f"wkt{e}")
               for e in range(ET)]
        qT = [qk_pool.tile([128, T], BF16, tag="qk", name=f"qT{p}")
              for p in range(HP)]
        kT = [qk_pool.tile([128, T], BF16, tag="qk", name=f"kT{p}")
              for p in range(HP)]
        olt = [olt_pool.tile([128, T], BF16, tag="olt", name=f"olt{c}")
               for c in range(HP)]
        wpt = [wp_pool.tile([128, E], BF16, tag="wp", name=f"wpt{c}")
               for c in range(HP)]

        TAG_BUFS = {"pss": 2, "av": 1, "misc": 2}

        def ps_tile(tag, name):
            # pss/av slots are 2 banks ([128,1024] f32); misc slots 1 bank
            shape = [128, 512] if tag == "misc" else [128, 1024]
            return psum.tile(shape, F32, tag=tag, name=name,
                             bufs=TAG_BUFS[tag])

        # ---------------- input DMAs (order = consumption order) ----------
        # first v matmul needs only xt0[:,0:128]+wvt0: tiny first transfers
        nc.sync.dma_start(xt[0][:, 0:128], xTd[0:128, 0:128])
        nc.sync.dma_start(wvt[0][:], wvd[0:128, :])
        nc.sync.dma_start(xt[0][:, 128:T // 2], xTd[0:128, 128:T // 2])
        for e in range(1, ET):
            nc.sync.dma_start(wvt[e][:], wvd[e * 128:(e + 1) * 128, :])
            nc.sync.dma_start(xt[e][:, 0:T // 2],
                              xTd[e * 128:(e + 1) * 128, 0:T // 2])
        for e in range(ET):
            nc.sync.dma_start(wqt[e][:], wqd[e * 128:(e + 1) * 128, :])
        for e in range(ET):
            nc.sync.dma_start(xt[e][:, T // 2:T],
                              xTd[e * 128:(e + 1) * 128, T // 2:T])
        for e in range(ET):
            nc.sync.dma_start(wkt[e][:], wkd[e * 128:(e + 1) * 128, :])
        nc.sync.dma_start(mask_sb[:], maskd)
        nc.sync.dma_start(ident_sb[:], identd)
        for c in range(HP):
            nc.sync.dma_start(wpt[c][:], wpd[c * 128:(c + 1) * 128, :])

        # ones columns of vaug (data cols are fully overwritten by evicts)
        nc.vector.memset(vaug[:, 64:TT * H_LOC * VW:VW], 1.0)

        def evict_engine(i, with_act=False):
            # GPSIMD cannot access PSUM on this target: evictions are
            # DVE-only during attention, DVE/ACT alternating in phases
            # where the ACT (exp) is idle.
            engs = [nc.vector, nc.scalar] if with_act else [nc.vector]
            eng = engs[i % len(engs)]

            def copy(out_ap, in_ap, _eng=eng):
                if _eng is nc.scalar:
                    return _eng.copy(out_ap, in_ap)
                return _eng.tensor_copy(out_ap, in_ap)

            def ts(out_ap, in_ap, scalar, _unused, _op, _eng=eng):
                if _eng is nc.scalar:
                    return _eng.mul(out_ap, in_ap, scalar)
                return _eng.tensor_scalar(out_ap, in_ap, scalar, None, _op)
            return type("E", (), {"tensor_copy": staticmethod(copy),
                                  "tensor_scalar": staticmethod(ts)})

        # ---------------- v projection ------------------------------------
        def v_mm(pv, slot, e, t):
            nc.tensor.matmul(
                pv[:, slot * C:(slot + 1) * C],
                xt[e][:, t * 128:(t + 1) * 128],
                wvt[e][:],
                start=(e == 0), stop=(e == ET - 1))

        def v_evict(pv, slot, t, eng):
            base = t * H_LOC * VW
            dst = vaug[:, base:base + H_LOC * VW].rearrange(
                "p (h c) -> p h c", c=VW)[:, :, 0:64]
            src = pv[:, slot * C:(slot + 1) * C].rearrange(
                "p (h c) -> p h c", c=64)
            eng.tensor_copy(dst, src)

        # head phase: t 0..5 in 4 interleaved groups, then t 6..7
        def emit_v_pass_a():
            g0 = ps_tile("pss", "psv_a0")
            g1 = ps_tile("pss", "psv_a1")
            g2 = ps_tile("misc", "psv_a2")
            g3 = ps_tile("misc", "psv_a3")
            gs = [(g0, 0), (g0, 1), (g1, 0), (g1, 1), (g2, 0), (g3, 0)]
            for e in range(ET):
                for t in range(6):
                    v_mm(gs[t][0], gs[t][1], e, t)
            for t in range(6):
                v_evict(gs[t][0], gs[t][1], t, evict_engine(t, with_act=True))
            g4 = ps_tile("av", "psv_a4")
            for e in range(ET):
                for t in (6, 7):
                    v_mm(g4, t % 2, e, t)
            for t in (6, 7):
                v_evict(g4, t % 2, t, evict_engine(t, with_act=True))

        # ---------------- q/k projections (head phase, pss tag) -----------
        def emit_qk_block(wlist, dst, p, jbp, eng, tag="pss"):
            pq = ps_tile(tag, "psqk")
            for e in range(ET):
                for j in range(2):
                    jb = jbp + j
                    nc.tensor.matmul(
                        pq[:, j * TQB:(j + 1) * TQB],
                        wlist[e][:, p * 128:(p + 1) * 128],
                        xt[e][:, jb * TQB:(jb + 1) * TQB],
                        start=(e == 0), stop=(e == ET - 1))
            eng.tensor_copy(dst[p][:, jbp * TQB:(jbp + 2) * TQB], pq[:])

        # ---------------- pacer / filler machinery ------------------------
        # Two queues: `urgent` (per-chunk transposes — tiny, gate olt) and
        # `background` (v pass B, q/k projections, output projection —
        # clock-paced against the ACT (exp) bottleneck). Entries:
        # (key, gen, min_tick): min_tick delays emission until the DVE work
        # they depend on has had time to execute (avtick = AV emissions).
        clock = {"pe": 0.0, "act": 0.0}
        avtick = [0]
        urgent = deque()
        background = deque()

        def gen_v_group_b(t):
            pv = ps_tile("misc", f"psv_b{t}")
            for e in range(ET):
                v_mm(pv, 0, e, t)
                yield TQB * PE_C
            v_evict(pv, 0, t, evict_engine(t))

        def gen_qk_fill(wlist, dst, p, jb):
            pq = ps_tile("misc", "psqkf")
            for e in range(ET):
                nc.tensor.matmul(
                    pq[:],
                    wlist[e][:, p * 128:(p + 1) * 128],
                    xt[e][:, jb * TQB:(jb + 1) * TQB],
                    start=(e == 0), stop=(e == ET - 1))
                yield TQB * PE_C
            evict_engine(p + jb).tensor_copy(
                dst[p][:, jb * TQB:(jb + 1) * TQB], pq[:])

        def gen_transp_chunk(p, jb, c, usb_c):
            mt = psum.tile([128, 128], BF16, tag="misc", name="pstr",
                           bufs=TAG_BUFS["misc"])
            nc.tensor.transpose(mt[:], usb_c[:], ident_sb[:])
            evict_engine(c).tensor_copy(
                olt[p][:, jb * TQB + c * 128:jb * TQB + (c + 1) * 128],
                mt[:])
            yield 128 * PE_C

        def gen_proj_tile(t, act_evict=False):
            ot = out_pool.tile([128, E], F32, tag="out", name=f"ot{t}")
            for nb in range(2):
                mp = ps_tile("misc", "psproj")
                h = nb * TQB
                for cc in range(HP):
                    nc.tensor.matmul(
                        mp[:],
                        olt[cc][:, t * 128:(t + 1) * 128],
                        wpt[cc][:, h:h + TQB],
                        start=(cc == 0), stop=(cc == HP - 1))
                    yield TQB * PE_C
                # ACT is idle at the very end: offload half the evictions
                if act_evict and nb == 1:
                    nc.scalar.copy(ot[:, h:h + TQB], mp[:])
                else:
                    nc.vector.tensor_copy(ot[:, h:h + TQB], mp[:])
                nc.sync.dma_start(out[t * 128:(t + 1) * 128, h:h + TQB],
                                  ot[:, h:h + TQB])

        def step_q(q):
            entry = q[0]
            try:
                clock["pe"] += next(entry[1])
                if len(entry) == 5:
                    entry[4][0] -= 1
                return True
            except StopIteration:
                q.popleft()
                return False

        quota = [0.0]

        def pace():
            while urgent and urgent[0][2] <= avtick[0]:
                step_q(urgent)
            if not background:
                return
            # EDF rationing: per tick emit just enough background steps that
            # every entry finishes by its deadline, spread uniformly
            tick = avtick[0]
            cum, rate = 0.0, 0.0
            for entry in background:
                cum += entry[4][0]
                rate = max(rate, cum / max(entry[3] - tick, 1.0))
            quota[0] = min(quota[0] + max(rate, 1.5), 8.0)
            while (background and quota[0] >= 1.0
                   and background[0][2] <= avtick[0]):
                if step_q(background):
                    quota[0] -= 1.0

        def force_drain(q, pred):
            """Fully emit all entries of q matching pred (FIFO order, so
            everything queued before them drains too)."""
            while any(pred(e[0]) for e in q):
                step_q(q)

        def drain_fillers():
            while urgent:
                step_q(urgent)
            while background:
                step_q(background)

        # ---------------- attention block ---------------------------------
        def emit_attn_block(p, jb, bi):
            n_tk = NJB * (jb + 1)
            av = ps_tile("av", "psav")
            r_t = [r_pool.tile([128, NJB], F32, tag="r", name=f"r{h}")
                   for h in range(2)]
            usb = [None] * NJB
            pend = deque()

            def emit_av():
                avtick[0] += 1
                t, ee = pend.popleft()
                o = t - NJB * jb
                for h in range(2):
                    hb = h * 512
                    vcol = t * H_LOC * VW + (2 * p + h) * VW
                    for cch in range(max(o, 0), NJB):
                        # one accumulation group per PSUM bank per block:
                        # start only on the round's first matmul (start
                        # marks the whole 2KB zero-region pending; later
                        # chunks are lazily zeroed on first write), stop
                        # only on the last (tile n_tk-1 touches chunk 3
                        # alone)
                        nc.tensor.matmul(
                            av[:, hb + cch * VW:hb + cch * VW + VW],
                            ee[:, hb + cch * 128:hb + (cch + 1) * 128],
                            vaug[:, vcol:vcol + VW],
                            start=(t == 0 and cch == max(o, 0)),
                            stop=(t == n_tk - 1 and cch == NJB - 1))
                        clock["pe"] += VW * PE_C
                # after the bank groups stop (last tile): normalize + evict
                # all chunks, then queue their transposes / projections
                if t == n_tk - 1:
                    for h in range(2):
                        hb = h * 512
                        nc.vector.reciprocal(
                            r_t[h][:],
                            av[:, hb + 64:hb + NJB * VW:VW])
                    for cch in range(NJB):
                        usb[cch] = usb_pool.tile(
                            [128, 128], BF16, tag="usb", name=f"usb{cch}")
                        for h in range(2):
                            hb = h * 512
                            eng = evict_engine(cch + h,
                                               with_act=(p == HP - 1))
                            eng.tensor_scalar(
                                usb[cch][:, h * 64:(h + 1) * 64],
                                av[:, hb + cch * VW:hb + cch * VW + 64],
                                r_t[h][:, cch:cch + 1], None, AL.mult)
                        urgent.append((("t", bi),
                                       gen_transp_chunk(p, jb, cch, usb[cch]),
                                       avtick[0] + 3 + cch))
                        if p == HP - 1:
                            # drain each block's proj within the following
                            # pair-3 block (last: by end)
                            dl = {0: 132, 1: 144, 2: 160, 3: 160}[jb]
                            background.append(
                                (("proj", jb),
                                 gen_proj_tile(NJB * jb + cch, jb == NJB - 1),
                                 avtick[0] + 4 + cch, dl, [9]))

            for t in range(n_tk):
                if len(pend) > LAG:
                    emit_av()
                o = t - NJB * jb
                psS = ps_tile("pss", "psS")
                lo = max(o, 0) * 128
                for h in range(2):
                    hb = h * 512
                    kc = kT[p][h * 64:(h + 1) * 64, t * 128:(t + 1) * 128]
                    if o < 0:
                        nc.tensor.matmul(
                            psS[:, hb:hb + 512], kc,
                            qT[p][h * 64:(h + 1) * 64,
                                  jb * TQB:(jb + 1) * TQB],
                            start=True, stop=True)
                    else:
                        # diagonal tile: restricted query columns only;
                        # the within-tile triangle is masked on DVE after
                        # the exp (LAG absorbs the extra hop)
                        nc.tensor.matmul(
                            psS[:, hb + lo:hb + 512], kc,
                            qT[p][h * 64:(h + 1) * 64,
                                  jb * TQB + lo:(jb + 1) * TQB],
                            start=True, stop=True)
                    clock["pe"] += (512 - lo) * PE_C
                ee = ee_pool.tile([128, 1024], BF16, tag="ee", name="ee")
                if o <= 0:
                    nc.scalar.activation(ee[:], psS[:], AF.Exp, scale=SCALE)
                    clock["act"] += _exp_ns(1024)
                else:
                    for h in range(2):
                        nc.scalar.activation(
                            ee[:, h * 512 + lo:h * 512 + 512],
                            psS[:, h * 512 + lo:h * 512 + 512],
                            AF.Exp, scale=SCALE)
                    clock["act"] += _exp_ns(2 * (512 - lo), nops=2)
                if o >= 0:
                    band = ee[:].rearrange("p (h q) -> p h q", q=512)[
                        :, :, o * 128:(o + 1) * 128]
                    mband = mask_sb[:].rearrange("p (h q) -> p h q", q=128)
                    nc.vector.tensor_tensor(band, band, mband, AL.mult)
                pend.append((t, ee))
                pace()
            while pend:
                emit_av()

        # ---------------- emission ----------------------------------------
        emit_v_pass_a()
        emit_qk_block(wqt, qT, 0, 0, evict_engine(0, True))
        emit_qk_block(wqt, qT, 0, 2, evict_engine(1, True))
        emit_qk_block(wkt, kT, 0, 0, evict_engine(0, True))
        emit_qk_block(wkt, kT, 0, 2, evict_engine(1, True))

        for tg in range(8, TT):
            jbn = tg // 4
            dl = max(2 * jbn * (jbn + 1) - 2, 1)
            background.append((("vb", tg), gen_v_group_b(tg), 0, dl, [9]))
        for p in range(1, HP):
            for jb in range(NJB):
                s_blk = 40 * p + 2 * jb * (jb + 1)
                background.append(
                    (("qk", p, jb), gen_qk_fill(wqt, qT, p, jb), 0,
                     max(s_blk - 2, 1), [9]))
                background.append(
                    (("qk", p, jb), gen_qk_fill(wkt, kT, p, jb), 0,
                     max(s_blk + 4 * jb - 2, 1), [9]))

        bi = 0
        for p in range(HP):
            for jb in range(NJB):
                # correctness: everything this block consumes must already
                # be emitted (Tile deps follow emission order) — vaug tiles
                # for its key range, q/k of this pair; plus recycle old
                # transposes (usb pool depth) before new norms allocate.
                n_tk = NJB * (jb + 1)
                force_drain(background, lambda k, n=n_tk, p=p: (
                    (k[0] == "vb" and k[1] < n)
                    or (k[0] == "qk" and k[1] <= p)))
                force_drain(urgent, lambda k, bi=bi: (
                    k[0] == "t" and k[1] <= bi - 2))
                emit_attn_block(p, jb, bi)
                bi += 1
        drain_fillers()
    return nc


def make_host_inputs():
    import ml_dtypes
    tri = np.where(np.arange(128)[None, :] >= np.arange(128)[:, None],
                   1.0, 0.0).astype(np.float32)
    masks = np.concatenate([tri, tri], axis=1).astype(ml_dtypes.bfloat16)
    ident = np.eye(128, dtype=np.float32).astype(ml_dtypes.bfloat16)
    return masks, ident


def shard_inputs(data, Wq, Wk, Wv, Wp):
    """Build the 8 per-core input maps from full inputs."""
    import ml_dtypes
    BF = ml_dtypes.bfloat16
    data = np.asarray(data, np.float32)
    Wq = np.asarray(Wq, np.float32)
    Wk = np.asarray(Wk, np.float32)
    Wv = np.asarray(Wv, np.float32)
    Wp = np.asarray(Wp, np.float32)
    masks, ident = make_host_inputs()
    in_maps = []
    for c in range(N_CORES):
        b, g = c // 2, c % 2
        hs = slice(g * H_LOC, (g + 1) * H_LOC)
        in_maps.append({
            "xT": np.ascontiguousarray(data[b].T).astype(BF),
            "wq": np.ascontiguousarray(
                Wq[hs].transpose(1, 0, 2).reshape(E, H_LOC * D)).astype(BF),
            "wk": np.ascontiguousarray(
                Wk[hs].transpose(1, 0, 2).reshape(E, H_LOC * D)).astype(BF),
            "wv": np.ascontiguousarray(
                Wv[hs].transpose(1, 0, 2).reshape(E, H_LOC * D)).astype(BF),
            "wp": np.ascontiguousarray(
                Wp[g * H_LOC * D:(g + 1) * H_LOC * D, :]).astype(BF),
            "masks": masks,
            "ident": ident,
        })
    return in_maps


_NC_CACHE = {}


def legalize_single_wait(nc):
    """This toolchain's walrus accepts at most ONE sync wait per engine
    instruction; Tile freely emits more. Split extra waits onto preceding
    same-engine NoOps (engine FIFOs make that equivalent)."""
    import bass_rust
    cnt = 0
    for f in nc.m.functions:
        for blk in f.blocks:
            new = []
            changed = False
            for inst in blk.instructions:
                si = inst.sync_info
                if si is not None and len(si.on_wait) > 1:
                    waits = list(si.on_wait)
                    for w in waits[:-1]:
                        nop = bass_rust.InstNoOp(name=f"legal_nop_{cnt}")
                        cnt += 1
                        nop.engine = inst.engine
                        nop.sync_info = bass_rust.SyncInfo(on_wait=[w],
                                                           on_update=[])
                        new.append(nop)
                    inst.sync_info = bass_rust.SyncInfo(
                        on_wait=[waits[-1]], on_update=list(si.on_update))
                    changed = True
                new.append(inst)
            if changed:
                blk.instructions = new
    return cnt


def get_nc():
    if "nc" not in _NC_CACHE:
        nc = bass.Bass("TRN2", target_bir_lowering=False, debug=False,
                       num_devices=N_CORES)
        build_program(nc)
        legalize_single_wait(nc)
        _NC_CACHE["nc"] = nc
    return _NC_CACHE["nc"]


def run(inputs, trace=False, **kw):
    """Run on the 8 NeuronCores; returns (full_output, BassKernelResults)."""
    from concourse.bass_utils import run_bass_kernel_spmd
    nc = get_nc()
    in_maps = shard_inputs(inputs["data"], inputs["Wq"], inputs["Wk"],
                           inputs["Wv"], inputs["Wp"])
    res = run_bass_kernel_spmd(nc, in_maps, core_ids=list(range(N_CORES)),
                               trace=trace, **kw)
    bp = np.asarray(inputs["bp"], np.float32)
    outf = np.empty((B, T, E), np.float32)
    for b in range(B):
        outf[b] = res.results[2 * b]["out"] + res.results[2 * b + 1]["out"] + bp
    return outf, res


def kernel(**inputs):
    out, _ = run(inputs)
    return out


# revision 72
# speedup vs baseline: 1.4140x; 1.0167x over previous
"""Trainium2 Bass kernel for multi-head causal attention + output projection.

Problem (hardcoded): B=4, T=2048, E=1024, H=16, D=64, float32.
  q = einsum('bte,hed->bhtd', data, Wq)   (same k, v)
  scores = q@k.T / sqrt(D), causal mask, softmax
  out = (attn @ v) concat-heads @ Wp + bp

Sharding across 8 NeuronCores: core c -> (batch b=c//2, head-group g=c%2).
Each core computes 8 heads of one batch and a partial projection with its
512-row slice of Wp; host sums the two partials per batch and adds bias.

Kernel structure (v2 — transposed attn@V):
  - all matmul operands bf16 (validated ~5e-3 rel err end to end)
  - scores kept transposed: psS[key, query] per key tile, 2 heads side by
    side; diagonal key tiles only compute query cols >= o*128, with the
    -1e30 causal mask folded into the PSUM accumulation as an extra
    ident.T@ntri matmul (no DVE masking, one PE->ACT->PE hop per tile)
  - attn@V with queries on PSUM partitions: stationary = exp weights
    [k, q-chunk], moving = V-augmented [k, 64+1] -> out [q, 65] per chunk
    (65-col moving beats the 512-col orientation ~2x in PE time); the ones
    column of V yields sum(exp) at col 64
  - PSUM zero-region rule: start=True lazily zeroes the whole 2KB bank, so
    each (bank, block) accumulation round has exactly one start (first
    matmul) and one stop (last matmul); PSUM is only read after the stop
  - normalization at block end: DVE reciprocal + per-partition
    tensor_scalar eviction, then a PE transpose ([q,c]->[c,q]) rebuilds
    olt[c, t] for the output projection
  - the PE is the overall bottleneck (~213us busy): remaining v/q/k
    projections, transposes and the output projection are drip-fed
    between key tiles by an EDF-rationed emission pacer so filler work
    interleaves with the ACT-paced attention cadence instead of bursting

PSUM discipline (8 banks):
  tag "pss"  x2 [128,1024]: score tiles (ping-pong) + head-phase groups
  tag "av"   x1 [128,1024]: per-block attn@V (A: cols 0:260, B: 512:772)
  tag "misc" x2 [128,512]:  fillers (v pass B, q/k blocks, transposes,
                            projection rounds), ping-pong hides WAR
"""

from collections import deque

import numpy as np

import concourse.bass as bass
import concourse.mybir as mybir
import concourse.tile as tile
from contextlib import ExitStack

F32 = mybir.dt.float32
BF16 = mybir.dt.bfloat16

# Full-problem constants
B, T, E, H, D = 4, 2048, 1024, 16, 64
N_CORES = 8
H_LOC = H // 2          # heads per core
HP = H_LOC // 2         # head pairs per core
SCALE = float(D) ** -0.5

C = H_LOC * D           # local concat width (512)
ET = E // 128           # embedding 128-tiles (8)
TT = T // 128           # token 128-tiles (16)
TQB = 512               # query-block width
NJB = T // TQB          # query blocks (4)
VW = 65                 # vaug per-head width (64 + ones col)
LAG = 3                 # tiles between exp and attn@V consumption (the
                        # loop emits AV one tile later -> effective 4)

# pacing constants (ns estimates mirroring the cost model)
PE_C = 1.0 / 2.4
ACT_C = 1.0 / 1.2


def _exp_ns(cols, nops=1):
    return cols * ACT_C + 185.0 * nops


def build_program(nc):
    AF = mybir.ActivationFunctionType
    AL = mybir.AluOpType

    xTd = nc.dram_tensor("xT", [E, T], BF16, kind="ExternalInput").ap()
    wqd = nc.dram_tensor("wq", [E, C], BF16, kind="ExternalInput").ap()
    wkd = nc.dram_tensor("wk", [E, C], BF16, kind="ExternalInput").ap()
    wvd = nc.dram_tensor("wv", [E, C], BF16, kind="ExternalInput").ap()
    wpd = nc.dram_tensor("wp", [C, E], BF16, kind="ExternalInput").ap()
    # [128,256]: two copies of tri[r, g] = (g >= r)
    maskd = nc.dram_tensor("masks", [128, 256], BF16, kind="ExternalInput").ap()
    identd = nc.dram_tensor("ident", [128, 128], BF16, kind="ExternalInput").ap()
    out = nc.dram_tensor("out", [T, E], F32, kind="ExternalOutput").ap()

    with tile.TileContext(nc) as tc, ExitStack() as ctx:
        const = ctx.enter_context(tc.tile_pool(name="const", bufs=1))
        xt_pool = ctx.enter_context(tc.tile_pool(name="xt", bufs=ET))
        wv_pool = ctx.enter_context(tc.tile_pool(name="wvp", bufs=ET))
        wq_pool = ctx.enter_context(tc.tile_pool(name="wqp", bufs=ET))
        wk_pool = ctx.enter_context(tc.tile_pool(name="wkp", bufs=ET))
        vaug_pool = ctx.enter_context(tc.tile_pool(name="vaugp", bufs=1))
        qk_pool = ctx.enter_context(tc.tile_pool(name="qkp", bufs=2 * HP))
        ee_pool = ctx.enter_context(tc.tile_pool(name="eep", bufs=8))
        usb_pool = ctx.enter_context(tc.tile_pool(name="usbp", bufs=8))
        r_pool = ctx.enter_context(tc.tile_pool(name="rp", bufs=4))
        olt_pool = ctx.enter_context(tc.tile_pool(name="oltp", bufs=HP))
        wp_pool = ctx.enter_context(tc.tile_pool(name="wpp", bufs=HP))
        out_pool = ctx.enter_context(tc.tile_pool(name="outp", bufs=4))
        psum = ctx.enter_context(tc.tile_pool(name="ps", bufs=4, space="PSUM"))

        mask_sb = const.tile([128, 256], BF16, name="mask_sb")
        ident_sb = const.tile([128, 128], BF16, name="ident_sb")

        vaug = vaug_pool.tile([128, TT * H_LOC * VW], BF16, name="vaug")
        xt = [xt_pool.tile([128, T], BF16, tag="xt", name=f"xt{e}")
              for e in range(ET)]
        wvt = [wv_pool.tile([128, C], BF16, tag="wv", name=f"wvt{e}")
               for e in range(ET)]
        wqt = [wq_pool.tile([128, C], BF16, tag="wq", name=f"wqt{e}")
               for e in range(ET)]
        wkt = [wk_pool.tile([128, C], BF16, tag="wk", name=f"wkt{e}")
               for e in range(ET)]
        qT = [qk_pool.tile([128, T], BF16, tag="qk", name=f"qT{p}")
              for p in range(HP)]
        kT = [qk_pool.tile([128, T], BF16, tag="qk", name=f"kT{p}")
              for p in range(HP)]
        olt = [olt_pool.tile([128, T], BF16, tag="olt", name=f"olt{c}")
               for c in range(HP)]
        wpt = [wp_pool.tile([128, E], BF16, tag="wp", name=f"wpt{c}")
               for c in range(HP)]

        TAG_BUFS = {"pss": 2, "av": 1, "misc": 2}

        def ps_tile(tag, name):
            # pss/av slots are 2 banks ([128,1024] f32); misc slots 1 bank
            shape = [128, 512] if tag == "misc" else [128, 1024]
            return psum.tile(shape, F32, tag=tag, name=name,
                             bufs=TAG_BUFS[tag])

        # ---------------- input DMAs (order = consumption order) ----------
        # first v matmul needs only xt0[:,0:128]+wvt0: tiny first transfers
        nc.sync.dma_start(xt[0][:, 0:128], xTd[0:128, 0:128])
        nc.sync.dma_start(wvt[0][:], wvd[0:128, :])
        nc.sync.dma_start(xt[0][:, 128:T // 2], xTd[0:128, 128:T // 2])
        for e in range(1, ET):
            nc.sync.dma_start(wvt[e][:], wvd[e * 128:(e + 1) * 128, :])
            nc.sync.dma_start(xt[e][:, 0:T // 2],
                              xTd[e * 128:(e + 1) * 128, 0:T // 2])
        for e in range(ET):
            nc.sync.dma_start(wqt[e][:], wqd[e * 128:(e + 1) * 128, :])
        for e in range(ET):
            nc.sync.dma_start(xt[e][:, T // 2:T],
                              xTd[e * 128:(e + 1) * 128, T // 2:T])
        for e in range(ET):
            nc.sync.dma_start(wkt[e][:], wkd[e * 128:(e + 1) * 128, :])
        nc.sync.dma_start(mask_sb[:], maskd)
        nc.sync.dma_start(ident_sb[:], identd)
        for c in range(HP):
            nc.sync.dma_start(wpt[c][:], wpd[c * 128:(c + 1) * 128, :])

        # ones columns of vaug (data cols are fully overwritten by evicts)
        nc.vector.memset(vaug[:, 64:TT * H_LOC * VW:VW], 1.0)

        def evict_engine(i, with_act=False):
            # GPSIMD cannot access PSUM on this target: evictions are
            # DVE-only during attention, DVE/ACT alternating in phases
            # where the ACT (exp) is idle.
            engs = [nc.vector, nc.scalar] if with_act else [nc.vector]
            eng = engs[i % len(engs)]

            def copy(out_ap, in_ap, _eng=eng):
                if _eng is nc.scalar:
                    return _eng.copy(out_ap, in_ap)
                return _eng.tensor_copy(out_ap, in_ap)

            def ts(out_ap, in_ap, scalar, _unused, _op, _eng=eng):
                if _eng is nc.scalar:
                    return _eng.mul(out_ap, in_ap, scalar)
                return _eng.tensor_scalar(out_ap, in_ap, scalar, None, _op)
            return type("E", (), {"tensor_copy": staticmethod(copy),
                                  "tensor_scalar": staticmethod(ts)})

        # ---------------- v projection ------------------------------------
        def v_mm(pv, slot, e, t):
            nc.tensor.matmul(
                pv[:, slot * C:(slot + 1) * C],
                xt[e][:, t * 128:(t + 1) * 128],
                wvt[e][:],
                start=(e == 0), stop=(e == ET - 1))

        def v_evict(pv, slot, t, eng):
            base = t * H_LOC * VW
            dst = vaug[:, base:base + H_LOC * VW].rearrange(
                "p (h c) -> p h c", c=VW)[:, :, 0:64]
            src = pv[:, slot * C:(slot + 1) * C].rearrange(
                "p (h c) -> p h c", c=64)
            eng.tensor_copy(dst, src)

        # head phase: t 0..5 in 4 interleaved groups, then t 6..7
        def emit_v_pass_a():
            g0 = ps_tile("pss", "psv_a0")
            g1 = ps_tile("pss", "psv_a1")
            g2 = ps_tile("misc", "psv_a2")
            g3 = ps_tile("misc", "psv_a3")
            gs = [(g0, 0), (g0, 1), (g1, 0), (g1, 1), (g2, 0), (g3, 0)]
            for e in range(ET):
                for t in range(6):
                    v_mm(gs[t][0], gs[t][1], e, t)
            for t in range(6):
                v_evict(gs[t][0], gs[t][1], t, evict_engine(t, with_act=True))
            g4 = ps_tile("av", "psv_a4")
            for e in range(ET):
                for t in (6, 7):
                    v_mm(g4, t % 2, e, t)
            for t in (6, 7):
                v_evict(g4, t % 2, t, evict_engine(t, with_act=True))

        # ---------------- q/k projections (head phase, pss tag) -----------
        def emit_qk_block(wlist, dst, p, jbp, eng, tag="pss"):
            pq = ps_tile(tag, "psqk")
            for e in range(ET):
                for j in range(2):
                    jb = jbp + j
                    nc.tensor.matmul(
                        pq[:, j * TQB:(j + 1) * TQB],
                        wlist[e][:, p * 128:(p + 1) * 128],
                        xt[e][:, jb * TQB:(jb + 1) * TQB],
                        start=(e == 0), stop=(e == ET - 1))
            eng.tensor_copy(dst[p][:, jbp * TQB:(jbp + 2) * TQB], pq[:])

        # ---------------- pacer / filler machinery ------------------------
        # Two queues: `urgent` (per-chunk transposes — tiny, gate olt) and
        # `background` (v pass B, q/k projections, output projection —
        # clock-paced against the ACT (exp) bottleneck). Entries:
        # (key, gen, min_tick): min_tick delays emission until the DVE work
        # they depend on has had time to execute (avtick = AV emissions).
        clock = {"pe": 0.0, "act": 0.0}
        avtick = [0]
        urgent = deque()
        background = deque()

        def gen_v_group_b(t):
            pv = ps_tile("misc", f"psv_b{t}")
            for e in range(ET):
                v_mm(pv, 0, e, t)
                yield TQB * PE_C
            v_evict(pv, 0, t, evict_engine(t))

        def gen_qk_fill(wlist, dst, p, jb):
            pq = ps_tile("misc", "psqkf")
            for e in range(ET):
                nc.tensor.matmul(
                    pq[:],
                    wlist[e][:, p * 128:(p + 1) * 128],
                    xt[e][:, jb * TQB:(jb + 1) * TQB],
                    start=(e == 0), stop=(e == ET - 1))
                yield TQB * PE_C
            evict_engine(p + jb).tensor_copy(
                dst[p][:, jb * TQB:(jb + 1) * TQB], pq[:])

        def gen_transp_chunk(p, jb, c, usb_c):
            mt = psum.tile([128, 128], BF16, tag="misc", name="pstr",
                           bufs=TAG_BUFS["misc"])
            nc.tensor.transpose(mt[:], usb_c[:], ident_sb[:])
            evict_engine(c).tensor_copy(
                olt[p][:, jb * TQB + c * 128:jb * TQB + (c + 1) * 128],
                mt[:])
            yield 128 * PE_C

        def gen_proj_tile(t, act_evict=False):
            ot = out_pool.tile([128, E], F32, tag="out", name=f"ot{t}")
            for nb in range(2):
                mp = ps_tile("misc", "psproj")
                h = nb * TQB
                for cc in range(HP):
                    nc.tensor.matmul(
                        mp[:],
                        olt[cc][:, t * 128:(t + 1) * 128],
                        wpt[cc][:, h:h + TQB],
                        start=(cc == 0), stop=(cc == HP - 1))
                    yield TQB * PE_C
                # ACT is idle at the very end: offload half the evictions
                if act_evict and nb == 1:
                    nc.scalar.copy(ot[:, h:h + TQB], mp[:])
                else:
                    nc.vector.tensor_copy(ot[:, h:h + TQB], mp[:])
                nc.sync.dma_start(out[t * 128:(t + 1) * 128, h:h + TQB],
                                  ot[:, h:h + TQB])

        def step_q(q):
            entry = q[0]
            try:
                clock["pe"] += next(entry[1])
                if len(entry) == 5:
                    entry[4][0] -= 1
                return True
            except StopIteration:
                q.popleft()
                return False

        quota = [0.0]

        def pace():
            while urgent and urgent[0][2] <= avtick[0]:
                step_q(urgent)
            if not background:
                return
            # EDF rationing: per tick emit just enough background steps that
            # every entry finishes by its deadline, spread uniformly
            tick = avtick[0]
            cum, rate = 0.0, 0.0
            for entry in background:
                cum += entry[4][0]
                rate = max(rate, cum / max(entry[3] - tick, 1.0))
            quota[0] = min(quota[0] + max(rate, 1.5), 8.0)
            while (background and quota[0] >= 1.0
                   and background[0][2] <= avtick[0]):
                if step_q(background):
                    quota[0] -= 1.0

        def force_drain(q, pred):
            """Fully emit all entries of q matching pred (FIFO order, so
            everything queued before them drains too)."""
            while any(pred(e[0]) for e in q):
                step_q(q)

        def drain_fillers():
            while urgent:
                step_q(urgent)
            while background:
                step_q(background)

        # ---------------- attention block ---------------------------------
        def emit_attn_block(p, jb, bi):
            n_tk = NJB * (jb + 1)
            av = ps_tile("av", "psav")
            r_t = [r_pool.tile([128, NJB], F32, tag="r", name=f"r{h}")
                   for h in range(2)]
            usb = [None] * NJB
            pend = deque()

            def emit_av():
                avtick[0] += 1
                t, ee = pend.popleft()
                o = t - NJB * jb
                for h in range(2):
                    hb = h * 512
                    vcol = t * H_LOC * VW + (2 * p + h) * VW
                    for cch in range(max(o, 0), NJB):
                        # one accumulation group per PSUM bank per block:
                        # start only on the round's first matmul (start
                        # marks the whole 2KB zero-region pending; later
                        # chunks are lazily zeroed on first write), stop
                        # only on the last (tile n_tk-1 touches chunk 3
                        # alone)
                        nc.tensor.matmul(
                            av[:, hb + cch * VW:hb + cch * VW + VW],
                            ee[:, hb + cch * 128:hb + (cch + 1) * 128],
                            vaug[:, vcol:vcol + VW],
                            start=(t == 0 and cch == max(o, 0)),
                            stop=(t == n_tk - 1 and cch == NJB - 1))
                        clock["pe"] += VW * PE_C
                # after the bank groups stop (last tile): normalize + evict
                # all chunks, then queue their transposes / projections
                if t == n_tk - 1:
                    for h in range(2):
                        hb = h * 512
                        nc.vector.reciprocal(
                            r_t[h][:],
                            av[:, hb + 64:hb + NJB * VW:VW])
                    for cch in range(NJB):
                        usb[cch] = usb_pool.tile(
                            [128, 128], BF16, tag="usb", name=f"usb{cch}")
                        for h in range(2):
                            hb = h * 512
                            eng = evict_engine(cch + h,
                                               with_act=(p == HP - 1))
                            eng.tensor_scalar(
                                usb[cch][:, h * 64:(h + 1) * 64],
                                av[:, hb + cch * VW:hb + cch * VW + 64],
                                r_t[h][:, cch:cch + 1], None, AL.mult)
                        urgent.append((("t", bi),
                                       gen_transp_chunk(p, jb, cch, usb[cch]),
                                       avtick[0] + 3 + cch))
                        if p == HP - 1:
                            # drain each block's proj within the following
                            # pair-3 block (last: by end)
                            dl = {0: 132, 1: 144, 2: 160, 3: 160}[jb]
                            background.append(
                                (("proj", jb),
                                 gen_proj_tile(NJB * jb + cch, jb == NJB - 1),
                                 avtick[0] + 4 + cch, dl, [9]))

            for t in range(n_tk):
                if len(pend) > LAG:
                    emit_av()
                o = t - NJB * jb
                psS = ps_tile("pss", "psS")
                lo = max(o, 0) * 128
                for h in range(2):
                    hb = h * 512
                    kc = kT[p][h * 64:(h + 1) * 64, t * 128:(t + 1) * 128]
                    if o < 0:
                        nc.tensor.matmul(
                            psS[:, hb:hb + 512], kc,
                            qT[p][h * 64:(h + 1) * 64,
                                  jb * TQB:(jb + 1) * TQB],
                            start=True, stop=True)
                    else:
                        # diagonal tile: restricted query columns only;
                        # the within-tile triangle is masked on DVE after
                        # the exp (LAG absorbs the extra hop)
                        nc.tensor.matmul(
                            psS[:, hb + lo:hb + 512], kc,
                            qT[p][h * 64:(h + 1) * 64,
                                  jb * TQB + lo:(jb + 1) * TQB],
                            start=True, stop=True)
                    clock["pe"] += (512 - lo) * PE_C
                ee = ee_pool.tile([128, 1024], BF16, tag="ee", name="ee")
                if o <= 0:
                    nc.scalar.activation(ee[:], psS[:], AF.Exp, scale=SCALE)
                    clock["act"] += _exp_ns(1024)
                else:
                    for h in range(2):
                        nc.scalar.activation(
                            ee[:, h * 512 + lo:h * 512 + 512],
                            psS[:, h * 512 + lo:h * 512 + 512],
                            AF.Exp, scale=SCALE)
                    clock["act"] += _exp_ns(2 * (512 - lo), nops=2)
                if o >= 0:
                    band = ee[:].rearrange("p (h q) -> p h q", q=512)[
                        :, :, o * 128:(o + 1) * 128]
                    mband = mask_sb[:].rearrange("p (h q) -> p h q", q=128)
                    nc.vector.tensor_tensor(band, band, mband, AL.mult)
                pend.append((t, ee))
                pace()
            while pend:
                emit_av()

        # ---------------- emission ----------------------------------------
        emit_v_pass_a()
        emit_qk_block(wqt, qT, 0, 0, evict_engine(0, True))
        emit_qk_block(wqt, qT, 0, 2, evict_engine(1, True))
        emit_qk_block(wkt, kT, 0, 0, evict_engine(0, True))
        emit_qk_block(wkt, kT, 0, 2, evict_engine(1, True))

        for tg in range(8, TT):
            jbn = tg // 4
            dl = max(2 * jbn * (jbn + 1) - 2, 1)
            background.append((("vb", tg), gen_v_group_b(tg), 0, dl, [9]))
        for p in range(1, HP):
            for jb in range(NJB):
                s_blk = 40 * p + 2 * jb * (jb + 1)
                background.append(
                    (("qk", p, jb), gen_qk_fill(wqt, qT, p, jb), 0,
                     max(s_blk - 2, 1), [9]))
                background.append(
                    (("qk", p, jb), gen_qk_fill(wkt, kT, p, jb), 0,
                     max(s_blk + 4 * jb - 2, 1), [9]))
        assert [e[0] for e in background if e[0][0] == "qk"] == [
            ("qk", p, jb) for p in range(1, HP) for jb in range(NJB)
            for _ in range(2)]

        bi = 0
        for p in range(HP):
            for jb in range(NJB):
                # correctness: everything this block consumes must already
                # be emitted (Tile deps follow emission order) — vaug tiles
                # for its key range, q/k of this pair; plus recycle old
                # transposes (usb pool depth) before new norms allocate.
                n_tk = NJB * (jb + 1)
                force_drain(background, lambda k, n=n_tk, p=p, jb=jb: (
                    (k[0] == "vb" and k[1] < n)
                    or (k[0] == "qk" and (k[1] < p
                                          or (k[1] == p and k[2] <= jb)))))
                force_drain(urgent, lambda k, bi=bi: (
                    k[0] == "t" and k[1] <= bi - 2))
                emit_attn_block(p, jb, bi)
                bi += 1
        drain_fillers()
    return nc


def make_host_inputs():
    import ml_dtypes
    tri = np.where(np.arange(128)[None, :] >= np.arange(128)[:, None],
                   1.0, 0.0).astype(np.float32)
    masks = np.concatenate([tri, tri], axis=1).astype(ml_dtypes.bfloat16)
    ident = np.eye(128, dtype=np.float32).astype(ml_dtypes.bfloat16)
    return masks, ident


def shard_inputs(data, Wq, Wk, Wv, Wp):
    """Build the 8 per-core input maps from full inputs."""
    import ml_dtypes
    BF = ml_dtypes.bfloat16
    data = np.asarray(data, np.float32)
    Wq = np.asarray(Wq, np.float32)
    Wk = np.asarray(Wk, np.float32)
    Wv = np.asarray(Wv, np.float32)
    Wp = np.asarray(Wp, np.float32)
    masks, ident = make_host_inputs()
    in_maps = []
    for c in range(N_CORES):
        b, g = c // 2, c % 2
        hs = slice(g * H_LOC, (g + 1) * H_LOC)
        in_maps.append({
            "xT": np.ascontiguousarray(data[b].T).astype(BF),
            "wq": np.ascontiguousarray(
                Wq[hs].transpose(1, 0, 2).reshape(E, H_LOC * D)).astype(BF),
            "wk": np.ascontiguousarray(
                Wk[hs].transpose(1, 0, 2).reshape(E, H_LOC * D)).astype(BF),
            "wv": np.ascontiguousarray(
                Wv[hs].transpose(1, 0, 2).reshape(E, H_LOC * D)).astype(BF),
            "wp": np.ascontiguousarray(
                Wp[g * H_LOC * D:(g + 1) * H_LOC * D, :]).astype(BF),
            "masks": masks,
            "ident": ident,
        })
    return in_maps


_NC_CACHE = {}


def legalize_single_wait(nc):
    """This toolchain's walrus accepts at most ONE sync wait per engine
    instruction; Tile freely emits more. Split extra waits onto preceding
    same-engine NoOps (engine FIFOs make that equivalent)."""
    import bass_rust
    cnt = 0
    for f in nc.m.functions:
        for blk in f.blocks:
            new = []
            changed = False
            for inst in blk.instructions:
                si = inst.sync_info
                if si is not None and len(si.on_wait) > 1:
                    waits = list(si.on_wait)
                    for w in waits[:-1]:
                        nop = bass_rust.InstNoOp(name=f"legal_nop_{cnt}")
                        cnt += 1
                        nop.engine = inst.engine
                        nop.sync_info = bass_rust.SyncInfo(on_wait=[w],
                                                           on_update=[])
                        new.append(nop)
                    inst.sync_info = bass_rust.SyncInfo(
                        on_wait=[waits[-1]], on_update=list(si.on_update))
                    changed = True
                new.append(inst)
            if changed:
                blk.instructions = new
    return cnt


def get_nc():
    if "nc" not in _NC_CACHE:
        nc = bass.Bass("TRN2", target_bir_lowering=False, debug=False,
                       num_devices=N_CORES)
        build_program(nc)
        legalize_single_wait(nc)
        _NC_CACHE["nc"] = nc
    return _NC_CACHE["nc"]


def run(inputs, trace=False, **kw):
    """Run on the 8 NeuronCores; returns (full_output, BassKernelResults)."""
    from concourse.bass_utils import run_bass_kernel_spmd
    nc = get_nc()
    in_maps = shard_inputs(inputs["data"], inputs["Wq"], inputs["Wk"],
                           inputs["Wv"], inputs["Wp"])
    res = run_bass_kernel_spmd(nc, in_maps, core_ids=list(range(N_CORES)),
                               trace=trace, **kw)
    bp = np.asarray(inputs["bp"], np.float32)
    outf = np.empty((B, T, E), np.float32)
    for b in range(B):
        outf[b] = res.results[2 * b]["out"] + res.results[2 * b + 1]["out"] + bp
    return outf, res


def kernel(**inputs):
    out, _ = run(inputs)
    return out


# revision 73
# speedup vs baseline: 1.4278x; 1.0098x over previous
"""Trainium2 Bass kernel for multi-head causal attention + output projection.

Problem (hardcoded): B=4, T=2048, E=1024, H=16, D=64, float32.
  q = einsum('bte,hed->bhtd', data, Wq)   (same k, v)
  scores = q@k.T / sqrt(D), causal mask, softmax
  out = (attn @ v) concat-heads @ Wp + bp

Sharding across 8 NeuronCores: core c -> (batch b=c//2, head-group g=c%2).
Each core computes 8 heads of one batch and a partial projection with its
512-row slice of Wp; host sums the two partials per batch and adds bias.

Kernel structure (v2 — transposed attn@V):
  - all matmul operands bf16 (validated ~5e-3 rel err end to end)
  - scores kept transposed: psS[key, query] per key tile, 2 heads side by
    side; diagonal key tiles only compute query cols >= o*128, with the
    -1e30 causal mask folded into the PSUM accumulation as an extra
    ident.T@ntri matmul (no DVE masking, one PE->ACT->PE hop per tile)
  - attn@V with queries on PSUM partitions: stationary = exp weights
    [k, q-chunk], moving = V-augmented [k, 64+1] -> out [q, 65] per chunk
    (65-col moving beats the 512-col orientation ~2x in PE time); the ones
    column of V yields sum(exp) at col 64
  - PSUM zero-region rule: start=True lazily zeroes the whole 2KB bank, so
    each (bank, block) accumulation round has exactly one start (first
    matmul) and one stop (last matmul); PSUM is only read after the stop
  - normalization at block end: DVE reciprocal + per-partition
    tensor_scalar eviction, then a PE transpose ([q,c]->[c,q]) rebuilds
    olt[c, t] for the output projection
  - the PE is the overall bottleneck (~213us busy): remaining v/q/k
    projections, transposes and the output projection are drip-fed
    between key tiles by an EDF-rationed emission pacer so filler work
    interleaves with the ACT-paced attention cadence instead of bursting

PSUM discipline (8 banks):
  tag "pss"  x2 [128,1024]: score tiles (ping-pong) + head-phase groups
  tag "av"   x1 [128,1024]: per-block attn@V (A: cols 0:260, B: 512:772)
  tag "misc" x2 [128,512]:  fillers (v pass B, q/k blocks, transposes,
                            projection rounds), ping-pong hides WAR
"""

from collections import deque

import numpy as np

import concourse.bass as bass
import concourse.mybir as mybir
import concourse.tile as tile
from contextlib import ExitStack

F32 = mybir.dt.float32
BF16 = mybir.dt.bfloat16

# Full-problem constants
B, T, E, H, D = 4, 2048, 1024, 16, 64
N_CORES = 8
H_LOC = H // 2          # heads per core
HP = H_LOC // 2         # head pairs per core
SCALE = float(D) ** -0.5

C = H_LOC * D           # local concat width (512)
ET = E // 128           # embedding 128-tiles (8)
TT = T // 128           # token 128-tiles (16)
TQB = 512               # query-block width
NJB = T // TQB          # query blocks (4)
VW = 65                 # vaug per-head width (64 + ones col)
LAG = 4                 # tiles between exp and attn@V consumption (the
                        # loop emits AV one tile later -> effective 5)

# pacing constants (ns estimates mirroring the cost model)
PE_C = 1.0 / 2.4
ACT_C = 1.0 / 1.2


def _exp_ns(cols, nops=1):
    return cols * ACT_C + 185.0 * nops


def build_program(nc):
    AF = mybir.ActivationFunctionType
    AL = mybir.AluOpType

    xTd = nc.dram_tensor("xT", [E, T], BF16, kind="ExternalInput").ap()
    wqd = nc.dram_tensor("wq", [E, C], BF16, kind="ExternalInput").ap()
    wkd = nc.dram_tensor("wk", [E, C], BF16, kind="ExternalInput").ap()
    wvd = nc.dram_tensor("wv", [E, C], BF16, kind="ExternalInput").ap()
    wpd = nc.dram_tensor("wp", [C, E], BF16, kind="ExternalInput").ap()
    # [128,256]: two copies of tri[r, g] = (g >= r)
    maskd = nc.dram_tensor("masks", [128, 256], BF16, kind="ExternalInput").ap()
    identd = nc.dram_tensor("ident", [128, 128], BF16, kind="ExternalInput").ap()
    out = nc.dram_tensor("out", [T, E], F32, kind="ExternalOutput").ap()

    with tile.TileContext(nc) as tc, ExitStack() as ctx:
        const = ctx.enter_context(tc.tile_pool(name="const", bufs=1))
        xt_pool = ctx.enter_context(tc.tile_pool(name="xt", bufs=ET))
        wv_pool = ctx.enter_context(tc.tile_pool(name="wvp", bufs=ET))
        wq_pool = ctx.enter_context(tc.tile_pool(name="wqp", bufs=ET))
        wk_pool = ctx.enter_context(tc.tile_pool(name="wkp", bufs=ET))
        vaug_pool = ctx.enter_context(tc.tile_pool(name="vaugp", bufs=1))
        qk_pool = ctx.enter_context(tc.tile_pool(name="qkp", bufs=2 * HP))
        ee_pool = ctx.enter_context(tc.tile_pool(name="eep", bufs=8))
        usb_pool = ctx.enter_context(tc.tile_pool(name="usbp", bufs=8))
        r_pool = ctx.enter_context(tc.tile_pool(name="rp", bufs=4))
        olt_pool = ctx.enter_context(tc.tile_pool(name="oltp", bufs=HP))
        wp_pool = ctx.enter_context(tc.tile_pool(name="wpp", bufs=HP))
        out_pool = ctx.enter_context(tc.tile_pool(name="outp", bufs=4))
        psum = ctx.enter_context(tc.tile_pool(name="ps", bufs=4, space="PSUM"))

        mask_sb = const.tile([128, 256], BF16, name="mask_sb")
        ident_sb = const.tile([128, 128], BF16, name="ident_sb")

        vaug = vaug_pool.tile([128, TT * H_LOC * VW], BF16, name="vaug")
        xt = [xt_pool.tile([128, T], BF16, tag="xt", name=f"xt{e}")
              for e in range(ET)]
        wvt = [wv_pool.tile([128, C], BF16, tag="wv", name=f"wvt{e}")
               for e in range(ET)]
        wqt = [wq_pool.tile([128, C], BF16, tag="wq", name=f"wqt{e}")
               for e in range(ET)]
        wkt = [wk_pool.tile([128, C], BF16, tag="wk", name=f"wkt{e}")
               for e in range(ET)]
        qT = [qk_pool.tile([128, T], BF16, tag="qk", name=f"qT{p}")
              for p in range(HP)]
        kT = [qk_pool.tile([128, T], BF16, tag="qk", name=f"kT{p}")
              for p in range(HP)]
        olt = [olt_pool.tile([128, T], BF16, tag="olt", name=f"olt{c}")
               for c in range(HP)]
        wpt = [wp_pool.tile([128, E], BF16, tag="wp", name=f"wpt{c}")
               for c in range(HP)]

        TAG_BUFS = {"pss": 2, "av": 1, "misc": 2}

        def ps_tile(tag, name):
            # pss/av slots are 2 banks ([128,1024] f32); misc slots 1 bank
            shape = [128, 512] if tag == "misc" else [128, 1024]
            return psum.tile(shape, F32, tag=tag, name=name,
                             bufs=TAG_BUFS[tag])

        # ---------------- input DMAs (order = consumption order) ----------
        # first v matmul needs only xt0[:,0:128]+wvt0: tiny first transfers
        nc.sync.dma_start(xt[0][:, 0:128], xTd[0:128, 0:128])
        nc.sync.dma_start(wvt[0][:], wvd[0:128, :])
        nc.sync.dma_start(xt[0][:, 128:T // 2], xTd[0:128, 128:T // 2])
        for e in range(1, ET):
            nc.sync.dma_start(wvt[e][:], wvd[e * 128:(e + 1) * 128, :])
            nc.sync.dma_start(xt[e][:, 0:T // 2],
                              xTd[e * 128:(e + 1) * 128, 0:T // 2])
        for e in range(ET):
            nc.sync.dma_start(wqt[e][:], wqd[e * 128:(e + 1) * 128, :])
        for e in range(ET):
            nc.sync.dma_start(xt[e][:, T // 2:T],
                              xTd[e * 128:(e + 1) * 128, T // 2:T])
        for e in range(ET):
            nc.sync.dma_start(wkt[e][:], wkd[e * 128:(e + 1) * 128, :])
        nc.sync.dma_start(mask_sb[:], maskd)
        nc.sync.dma_start(ident_sb[:], identd)
        for c in range(HP):
            nc.sync.dma_start(wpt[c][:], wpd[c * 128:(c + 1) * 128, :])

        # ones columns of vaug (data cols are fully overwritten by evicts)
        nc.vector.memset(vaug[:, 64:TT * H_LOC * VW:VW], 1.0)

        def evict_engine(i, with_act=False):
            # GPSIMD cannot access PSUM on this target: evictions are
            # DVE-only during attention, DVE/ACT alternating in phases
            # where the ACT (exp) is idle.
            engs = [nc.vector, nc.scalar] if with_act else [nc.vector]
            eng = engs[i % len(engs)]

            def copy(out_ap, in_ap, _eng=eng):
                if _eng is nc.scalar:
                    return _eng.copy(out_ap, in_ap)
                return _eng.tensor_copy(out_ap, in_ap)

            def ts(out_ap, in_ap, scalar, _unused, _op, _eng=eng):
                if _eng is nc.scalar:
                    return _eng.mul(out_ap, in_ap, scalar)
                return _eng.tensor_scalar(out_ap, in_ap, scalar, None, _op)
            return type("E", (), {"tensor_copy": staticmethod(copy),
                                  "tensor_scalar": staticmethod(ts)})

        # ---------------- v projection ------------------------------------
        def v_mm(pv, slot, e, t):
            nc.tensor.matmul(
                pv[:, slot * C:(slot + 1) * C],
                xt[e][:, t * 128:(t + 1) * 128],
                wvt[e][:],
                start=(e == 0), stop=(e == ET - 1))

        def v_evict(pv, slot, t, eng):
            base = t * H_LOC * VW
            dst = vaug[:, base:base + H_LOC * VW].rearrange(
                "p (h c) -> p h c", c=VW)[:, :, 0:64]
            src = pv[:, slot * C:(slot + 1) * C].rearrange(
                "p (h c) -> p h c", c=64)
            eng.tensor_copy(dst, src)

        # head phase: t 0..5 in 4 interleaved groups, then t 6..7
        def emit_v_pass_a():
            g0 = ps_tile("pss", "psv_a0")
            g1 = ps_tile("pss", "psv_a1")
            g2 = ps_tile("misc", "psv_a2")
            g3 = ps_tile("misc", "psv_a3")
            gs = [(g0, 0), (g0, 1), (g1, 0), (g1, 1), (g2, 0), (g3, 0)]
            for e in range(ET):
                for t in range(6):
                    v_mm(gs[t][0], gs[t][1], e, t)
            for t in range(6):
                v_evict(gs[t][0], gs[t][1], t, evict_engine(t, with_act=True))
            g4 = ps_tile("av", "psv_a4")
            for e in range(ET):
                for t in (6, 7):
                    v_mm(g4, t % 2, e, t)
            for t in (6, 7):
                v_evict(g4, t % 2, t, evict_engine(t, with_act=True))

        # ---------------- q/k projections (head phase, pss tag) -----------
        def emit_qk_block(wlist, dst, p, jbp, eng, tag="pss"):
            pq = ps_tile(tag, "psqk")
            for e in range(ET):
                for j in range(2):
                    jb = jbp + j
                    nc.tensor.matmul(
                        pq[:, j * TQB:(j + 1) * TQB],
                        wlist[e][:, p * 128:(p + 1) * 128],
                        xt[e][:, jb * TQB:(jb + 1) * TQB],
                        start=(e == 0), stop=(e == ET - 1))
            eng.tensor_copy(dst[p][:, jbp * TQB:(jbp + 2) * TQB], pq[:])

        # ---------------- pacer / filler machinery ------------------------
        # Two queues: `urgent` (per-chunk transposes — tiny, gate olt) and
        # `background` (v pass B, q/k projections, output projection —
        # clock-paced against the ACT (exp) bottleneck). Entries:
        # (key, gen, min_tick): min_tick delays emission until the DVE work
        # they depend on has had time to execute (avtick = AV emissions).
        clock = {"pe": 0.0, "act": 0.0}
        avtick = [0]
        urgent = deque()
        background = deque()

        def gen_v_group_b(t):
            pv = ps_tile("misc", f"psv_b{t}")
            for e in range(ET):
                v_mm(pv, 0, e, t)
                yield TQB * PE_C
            v_evict(pv, 0, t, evict_engine(t))

        def gen_qk_fill(wlist, dst, p, jb):
            pq = ps_tile("misc", "psqkf")
            for e in range(ET):
                nc.tensor.matmul(
                    pq[:],
                    wlist[e][:, p * 128:(p + 1) * 128],
                    xt[e][:, jb * TQB:(jb + 1) * TQB],
                    start=(e == 0), stop=(e == ET - 1))
                yield TQB * PE_C
            evict_engine(p + jb).tensor_copy(
                dst[p][:, jb * TQB:(jb + 1) * TQB], pq[:])

        def gen_transp_chunk(p, jb, c, usb_c):
            mt = psum.tile([128, 128], BF16, tag="misc", name="pstr",
                           bufs=TAG_BUFS["misc"])
            nc.tensor.transpose(mt[:], usb_c[:], ident_sb[:])
            evict_engine(c).tensor_copy(
                olt[p][:, jb * TQB + c * 128:jb * TQB + (c + 1) * 128],
                mt[:])
            yield 128 * PE_C

        def gen_proj_tile(t, act_evict=False):
            ot = out_pool.tile([128, E], F32, tag="out", name=f"ot{t}")
            for nb in range(2):
                mp = ps_tile("misc", "psproj")
                h = nb * TQB
                for cc in range(HP):
                    nc.tensor.matmul(
                        mp[:],
                        olt[cc][:, t * 128:(t + 1) * 128],
                        wpt[cc][:, h:h + TQB],
                        start=(cc == 0), stop=(cc == HP - 1))
                    yield TQB * PE_C
                # ACT is idle at the very end: offload half the evictions
                if act_evict and nb == 1:
                    nc.scalar.copy(ot[:, h:h + TQB], mp[:])
                else:
                    nc.vector.tensor_copy(ot[:, h:h + TQB], mp[:])
                nc.sync.dma_start(out[t * 128:(t + 1) * 128, h:h + TQB],
                                  ot[:, h:h + TQB])

        def step_q(q):
            entry = q[0]
            try:
                clock["pe"] += next(entry[1])
                if len(entry) == 5:
                    entry[4][0] -= 1
                return True
            except StopIteration:
                q.popleft()
                return False

        quota = [0.0]

        def pace():
            while urgent and urgent[0][2] <= avtick[0]:
                step_q(urgent)
            if not background:
                return
            # EDF rationing: per tick emit just enough background steps that
            # every entry finishes by its deadline, spread uniformly
            tick = avtick[0]
            cum, rate = 0.0, 0.0
            for entry in background:
                cum += entry[4][0]
                rate = max(rate, cum / max(entry[3] - tick, 1.0))
            quota[0] = min(quota[0] + max(rate, 1.5), 8.0)
            while (background and quota[0] >= 1.0
                   and background[0][2] <= avtick[0]):
                if step_q(background):
                    quota[0] -= 1.0

        def force_drain(q, pred):
            """Fully emit all entries of q matching pred (FIFO order, so
            everything queued before them drains too)."""
            while any(pred(e[0]) for e in q):
                step_q(q)

        def drain_fillers():
            while urgent:
                step_q(urgent)
            while background:
                step_q(background)

        # ---------------- attention block ---------------------------------
        def emit_attn_block(p, jb, bi):
            n_tk = NJB * (jb + 1)
            av = ps_tile("av", "psav")
            r_t = [r_pool.tile([128, NJB], F32, tag="r", name=f"r{h}")
                   for h in range(2)]
            usb = [None] * NJB
            pend = deque()

            def emit_av():
                avtick[0] += 1
                t, ee = pend.popleft()
                o = t - NJB * jb
                for h in range(2):
                    hb = h * 512
                    vcol = t * H_LOC * VW + (2 * p + h) * VW
                    for cch in range(max(o, 0), NJB):
                        # one accumulation group per PSUM bank per block:
                        # start only on the round's first matmul (start
                        # marks the whole 2KB zero-region pending; later
                        # chunks are lazily zeroed on first write), stop
                        # only on the last (tile n_tk-1 touches chunk 3
                        # alone)
                        nc.tensor.matmul(
                            av[:, hb + cch * VW:hb + cch * VW + VW],
                            ee[:, hb + cch * 128:hb + (cch + 1) * 128],
                            vaug[:, vcol:vcol + VW],
                            start=(t == 0 and cch == max(o, 0)),
                            stop=(t == n_tk - 1 and cch == NJB - 1))
                        clock["pe"] += VW * PE_C
                # after the bank groups stop (last tile): normalize + evict
                # all chunks, then queue their transposes / projections
                if t == n_tk - 1:
                    for h in range(2):
                        hb = h * 512
                        nc.vector.reciprocal(
                            r_t[h][:],
                            av[:, hb + 64:hb + NJB * VW:VW])
                    for cch in range(NJB):
                        usb[cch] = usb_pool.tile(
                            [128, 128], BF16, tag="usb", name=f"usb{cch}")
                        for h in range(2):
                            hb = h * 512
                            eng = evict_engine(cch + h,
                                               with_act=(p == HP - 1))
                            eng.tensor_scalar(
                                usb[cch][:, h * 64:(h + 1) * 64],
                                av[:, hb + cch * VW:hb + cch * VW + 64],
                                r_t[h][:, cch:cch + 1], None, AL.mult)
                        urgent.append((("t", bi),
                                       gen_transp_chunk(p, jb, cch, usb[cch]),
                                       avtick[0] + 3 + cch))
                        if p == HP - 1:
                            # drain each block's proj within the following
                            # pair-3 block (last: by end)
                            dl = {0: 132, 1: 144, 2: 160, 3: 160}[jb]
                            background.append(
                                (("proj", jb),
                                 gen_proj_tile(NJB * jb + cch, jb == NJB - 1),
                                 avtick[0] + 4 + cch, dl, [9]))

            for t in range(n_tk):
                if len(pend) > LAG:
                    emit_av()
                o = t - NJB * jb
                psS = ps_tile("pss", "psS")
                lo = max(o, 0) * 128
                for h in range(2):
                    hb = h * 512
                    kc = kT[p][h * 64:(h + 1) * 64, t * 128:(t + 1) * 128]
                    if o < 0:
                        nc.tensor.matmul(
                            psS[:, hb:hb + 512], kc,
                            qT[p][h * 64:(h + 1) * 64,
                                  jb * TQB:(jb + 1) * TQB],
                            start=True, stop=True)
                    else:
                        # diagonal tile: restricted query columns only;
                        # the within-tile triangle is masked on DVE after
                        # the exp (LAG absorbs the extra hop)
                        nc.tensor.matmul(
                            psS[:, hb + lo:hb + 512], kc,
                            qT[p][h * 64:(h + 1) * 64,
                                  jb * TQB + lo:(jb + 1) * TQB],
                            start=True, stop=True)
                    clock["pe"] += (512 - lo) * PE_C
                ee = ee_pool.tile([128, 1024], BF16, tag="ee", name="ee")
                if o <= 0:
                    nc.scalar.activation(ee[:], psS[:], AF.Exp, scale=SCALE)
                    clock["act"] += _exp_ns(1024)
                else:
                    for h in range(2):
                        nc.scalar.activation(
                            ee[:, h * 512 + lo:h * 512 + 512],
                            psS[:, h * 512 + lo:h * 512 + 512],
                            AF.Exp, scale=SCALE)
                    clock["act"] += _exp_ns(2 * (512 - lo), nops=2)
                if o >= 0:
                    band = ee[:].rearrange("p (h q) -> p h q", q=512)[
                        :, :, o * 128:(o + 1) * 128]
                    mband = mask_sb[:].rearrange("p (h q) -> p h q", q=128)
                    nc.vector.tensor_tensor(band, band, mband, AL.mult)
                pend.append((t, ee))
                pace()
            while pend:
                emit_av()

        # ---------------- emission ----------------------------------------
        emit_v_pass_a()
        emit_qk_block(wqt, qT, 0, 0, evict_engine(0, True))
        emit_qk_block(wqt, qT, 0, 2, evict_engine(1, True))
        emit_qk_block(wkt, kT, 0, 0, evict_engine(0, True))
        emit_qk_block(wkt, kT, 0, 2, evict_engine(1, True))

        for tg in range(8, TT):
            jbn = tg // 4
            dl = max(2 * jbn * (jbn + 1) - 2, 1)
            background.append((("vb", tg), gen_v_group_b(tg), 0, dl, [9]))
        for p in range(1, HP):
            for jb in range(NJB):
                s_blk = 40 * p + 2 * jb * (jb + 1)
                background.append(
                    (("qk", p, jb), gen_qk_fill(wqt, qT, p, jb), 0,
                     max(s_blk - 2, 1), [9]))
                background.append(
                    (("qk", p, jb), gen_qk_fill(wkt, kT, p, jb), 0,
                     max(s_blk + 4 * jb - 2, 1), [9]))
        assert [e[0] for e in background if e[0][0] == "qk"] == [
            ("qk", p, jb) for p in range(1, HP) for jb in range(NJB)
            for _ in range(2)]

        bi = 0
        for p in range(HP):
            for jb in range(NJB):
                # correctness: everything this block consumes must already
                # be emitted (Tile deps follow emission order) — vaug tiles
                # for its key range, q/k of this pair; plus recycle old
                # transposes (usb pool depth) before new norms allocate.
                n_tk = NJB * (jb + 1)
                force_drain(background, lambda k, n=n_tk, p=p, jb=jb: (
                    (k[0] == "vb" and k[1] < n)
                    or (k[0] == "qk" and (k[1] < p
                                          or (k[1] == p and k[2] <= jb)))))
                force_drain(urgent, lambda k, bi=bi: (
                    k[0] == "t" and k[1] <= bi - 2))
                emit_attn_block(p, jb, bi)
                bi += 1
        drain_fillers()
    return nc


def make_host_inputs():
    import ml_dtypes
    tri = np.where(np.arange(128)[None, :] >= np.arange(128)[:, None],
                   1.0, 0.0).astype(np.float32)
    masks = np.concatenate([tri, tri], axis=1).astype(ml_dtypes.bfloat16)
    ident = np.eye(128, dtype=np.float32).astype(ml_dtypes.bfloat16)
    return masks, ident


def shard_inputs(data, Wq, Wk, Wv, Wp):
    """Build the 8 per-core input maps from full inputs."""
    import ml_dtypes
    BF = ml_dtypes.bfloat16
    data = np.asarray(data, np.float32)
    Wq = np.asarray(Wq, np.float32)
    Wk = np.asarray(Wk, np.float32)
    Wv = np.asarray(Wv, np.float32)
    Wp = np.asarray(Wp, np.float32)
    masks, ident = make_host_inputs()
    in_maps = []
    for c in range(N_CORES):
        b, g = c // 2, c % 2
        hs = slice(g * H_LOC, (g + 1) * H_LOC)
        in_maps.append({
            "xT": np.ascontiguousarray(data[b].T).astype(BF),
            "wq": np.ascontiguousarray(
                Wq[hs].transpose(1, 0, 2).reshape(E, H_LOC * D)).astype(BF),
            "wk": np.ascontiguousarray(
                Wk[hs].transpose(1, 0, 2).reshape(E, H_LOC * D)).astype(BF),
            "wv": np.ascontiguousarray(
                Wv[hs].transpose(1, 0, 2).reshape(E, H_LOC * D)).astype(BF),
            "wp": np.ascontiguousarray(
                Wp[g * H_LOC * D:(g + 1) * H_LOC * D, :]).astype(BF),
            "masks": masks,
            "ident": ident,
        })
    return in_maps


_NC_CACHE = {}


def legalize_single_wait(nc):
    """This toolchain's walrus accepts at most ONE sync wait per engine
    instruction; Tile freely emits more. Split extra waits onto preceding
    same-engine NoOps (engine FIFOs make that equivalent)."""
    import bass_rust
    cnt = 0
    for f in nc.m.functions:
        for blk in f.blocks:
            new = []
            changed = False
            for inst in blk.instructions:
                si = inst.sync_info
                if si is not None and len(si.on_wait) > 1:
                    waits = list(si.on_wait)
                    for w in waits[:-1]:
                        nop = bass_rust.InstNoOp(name=f"legal_nop_{cnt}")
                        cnt += 1
                        nop.engine = inst.engine
                        nop.sync_info = bass_rust.SyncInfo(on_wait=[w],
                                                           on_update=[])
                        new.append(nop)
                    inst.sync_info = bass_rust.SyncInfo(
                        on_wait=[waits[-1]], on_update=list(si.on_update))
                    changed = True
                new.append(inst)
            if changed:
                blk.instructions = new
    return cnt


def get_nc():
    if "nc" not in _NC_CACHE:
        nc = bass.Bass("TRN2", target_bir_lowering=False, debug=False,
                       num_devices=N_CORES)
        build_program(nc)
        legalize_single_wait(nc)
        _NC_CACHE["nc"] = nc
    return _NC_CACHE["nc"]


def run(inputs, trace=False, **kw):
    """Run on the 8 NeuronCores; returns (full_output, BassKernelResults)."""
    from concourse.bass_utils import run_bass_kernel_spmd
    nc = get_nc()
    in_maps = shard_inputs(inputs["data"], inputs["Wq"], inputs["Wk"],
                           inputs["Wv"], inputs["Wp"])
    res = run_bass_kernel_spmd(nc, in_maps, core_ids=list(range(N_CORES)),
                               trace=trace, **kw)
    bp = np.asarray(inputs["bp"], np.float32)
    outf = np.empty((B, T, E), np.float32)
    for b in range(B):
        outf[b] = res.results[2 * b]["out"] + res.results[2 * b + 1]["out"] + bp
    return outf, res


def kernel(**inputs):
    out, _ = run(inputs)
    return out


# revision 78
# speedup vs baseline: 1.4290x; 1.0008x over previous
"""Trainium2 Bass kernel for multi-head causal attention + output projection.

Problem (hardcoded): B=4, T=2048, E=1024, H=16, D=64, float32.
  q = einsum('bte,hed->bhtd', data, Wq)   (same k, v)
  scores = q@k.T / sqrt(D), causal mask, softmax
  out = (attn @ v) concat-heads @ Wp + bp

Sharding across 8 NeuronCores: core c -> (batch b=c//2, head-group g=c%2).
Each core computes 8 heads of one batch and a partial projection with its
512-row slice of Wp; host sums the two partials per batch and adds bias.

Kernel structure (v2 — transposed attn@V):
  - all matmul operands bf16 (validated ~5e-3 rel err end to end)
  - scores kept transposed: psS[key, query] per key tile, 2 heads side by
    side; diagonal key tiles only compute query cols >= o*128, with the
    -1e30 causal mask folded into the PSUM accumulation as an extra
    ident.T@ntri matmul (no DVE masking, one PE->ACT->PE hop per tile)
  - attn@V with queries on PSUM partitions: stationary = exp weights
    [k, q-chunk], moving = V-augmented [k, 64+1] -> out [q, 65] per chunk
    (65-col moving beats the 512-col orientation ~2x in PE time); the ones
    column of V yields sum(exp) at col 64
  - PSUM zero-region rule: start=True lazily zeroes the whole 2KB bank, so
    each (bank, block) accumulation round has exactly one start (first
    matmul) and one stop (last matmul); PSUM is only read after the stop
  - normalization at block end: DVE reciprocal + per-partition
    tensor_scalar eviction, then a PE transpose ([q,c]->[c,q]) rebuilds
    olt[c, t] for the output projection
  - the PE is the overall bottleneck (~213us busy): remaining v/q/k
    projections, transposes and the output projection are drip-fed
    between key tiles by an EDF-rationed emission pacer so filler work
    interleaves with the ACT-paced attention cadence instead of bursting

PSUM discipline (8 banks):
  tag "pss"  x2 [128,1024]: score tiles (ping-pong) + head-phase groups
  tag "av"   x1 [128,1024]: per-block attn@V (A: cols 0:260, B: 512:772)
  tag "misc" x2 [128,512]:  fillers (v pass B, q/k blocks, transposes,
                            projection rounds), ping-pong hides WAR
"""

from collections import deque

import numpy as np

import concourse.bass as bass
import concourse.mybir as mybir
import concourse.tile as tile
from contextlib import ExitStack

F32 = mybir.dt.float32
BF16 = mybir.dt.bfloat16

# Full-problem constants
B, T, E, H, D = 4, 2048, 1024, 16, 64
N_CORES = 8
H_LOC = H // 2          # heads per core
HP = H_LOC // 2         # head pairs per core
SCALE = float(D) ** -0.5

C = H_LOC * D           # local concat width (512)
ET = E // 128           # embedding 128-tiles (8)
TT = T // 128           # token 128-tiles (16)
TQB = 512               # query-block width
NJB = T // TQB          # query blocks (4)
VW = 65                 # vaug per-head width (64 + ones col)
LAG = 4                 # tiles between exp and attn@V consumption (the
                        # loop emits AV one tile later -> effective 5)

# pacing constants (ns estimates mirroring the cost model)
PE_C = 1.0 / 2.4
ACT_C = 1.0 / 1.2


def _exp_ns(cols, nops=1):
    return cols * ACT_C + 185.0 * nops


def build_program(nc):
    AF = mybir.ActivationFunctionType
    AL = mybir.AluOpType

    xTd = nc.dram_tensor("xT", [E, T], BF16, kind="ExternalInput").ap()
    wqd = nc.dram_tensor("wq", [E, C], BF16, kind="ExternalInput").ap()
    wkd = nc.dram_tensor("wk", [E, C], BF16, kind="ExternalInput").ap()
    wvd = nc.dram_tensor("wv", [E, C], BF16, kind="ExternalInput").ap()
    wpd = nc.dram_tensor("wp", [C, E], BF16, kind="ExternalInput").ap()
    # [128,256]: two copies of tri[r, g] = (g >= r)
    maskd = nc.dram_tensor("masks", [128, 256], BF16, kind="ExternalInput").ap()
    identd = nc.dram_tensor("ident", [128, 128], BF16, kind="ExternalInput").ap()
    out = nc.dram_tensor("out", [T, E], F32, kind="ExternalOutput").ap()

    with tile.TileContext(nc) as tc, ExitStack() as ctx:
        const = ctx.enter_context(tc.tile_pool(name="const", bufs=1))
        xt_pool = ctx.enter_context(tc.tile_pool(name="xt", bufs=ET))
        wv_pool = ctx.enter_context(tc.tile_pool(name="wvp", bufs=ET))
        wq_pool = ctx.enter_context(tc.tile_pool(name="wqp", bufs=ET))
        wk_pool = ctx.enter_context(tc.tile_pool(name="wkp", bufs=ET))
        vaug_pool = ctx.enter_context(tc.tile_pool(name="vaugp", bufs=1))
        qk_pool = ctx.enter_context(tc.tile_pool(name="qkp", bufs=2 * HP))
        ee_pool = ctx.enter_context(tc.tile_pool(name="eep", bufs=8))
        usb_pool = ctx.enter_context(tc.tile_pool(name="usbp", bufs=8))
        r_pool = ctx.enter_context(tc.tile_pool(name="rp", bufs=4))
        olt_pool = ctx.enter_context(tc.tile_pool(name="oltp", bufs=HP))
        wp_pool = ctx.enter_context(tc.tile_pool(name="wpp", bufs=HP))
        out_pool = ctx.enter_context(tc.tile_pool(name="outp", bufs=4))
        psum = ctx.enter_context(tc.tile_pool(name="ps", bufs=4, space="PSUM"))

        mask_sb = const.tile([128, 256], BF16, name="mask_sb")
        ident_sb = const.tile([128, 128], BF16, name="ident_sb")

        vaug = vaug_pool.tile([128, TT * H_LOC * VW], BF16, name="vaug")
        xt = [xt_pool.tile([128, T], BF16, tag="xt", name=f"xt{e}")
              for e in range(ET)]
        wvt = [wv_pool.tile([128, C], BF16, tag="wv", name=f"wvt{e}")
               for e in range(ET)]
        wqt = [wq_pool.tile([128, C], BF16, tag="wq", name=f"wqt{e}")
               for e in range(ET)]
        wkt = [wk_pool.tile([128, C], BF16, tag="wk", name=f"wkt{e}")
               for e in range(ET)]
        qT = [qk_pool.tile([128, T], BF16, tag="qk", name=f"qT{p}")
              for p in range(HP)]
        kT = [qk_pool.tile([128, T], BF16, tag="qk", name=f"kT{p}")
              for p in range(HP)]
        olt = [olt_pool.tile([128, T], BF16, tag="olt", name=f"olt{c}")
               for c in range(HP)]
        wpt = [wp_pool.tile([128, E], BF16, tag="wp", name=f"wpt{c}")
               for c in range(HP)]

        TAG_BUFS = {"pss": 2, "av": 1, "misc": 2}

        def ps_tile(tag, name):
            # pss/av slots are 2 banks ([128,1024] f32); misc slots 1 bank
            shape = [128, 512] if tag == "misc" else [128, 1024]
            return psum.tile(shape, F32, tag=tag, name=name,
                             bufs=TAG_BUFS[tag])

        # ---------------- input DMAs (order = consumption order) ----------
        # first v matmul needs only xt0[:,0:128]+wvt0: tiny first transfers
        nc.sync.dma_start(xt[0][:, 0:128], xTd[0:128, 0:128])
        nc.sync.dma_start(wvt[0][:], wvd[0:128, :])
        nc.sync.dma_start(xt[0][:, 128:T // 2], xTd[0:128, 128:T // 2])
        for e in range(1, ET):
            nc.sync.dma_start(wvt[e][:], wvd[e * 128:(e + 1) * 128, :])
            nc.sync.dma_start(xt[e][:, 0:T // 2],
                              xTd[e * 128:(e + 1) * 128, 0:T // 2])
        for e in range(ET):
            nc.sync.dma_start(wqt[e][:], wqd[e * 128:(e + 1) * 128, :])
        for e in range(ET):
            nc.sync.dma_start(xt[e][:, T // 2:T],
                              xTd[e * 128:(e + 1) * 128, T // 2:T])
        for e in range(ET):
            nc.sync.dma_start(wkt[e][:], wkd[e * 128:(e + 1) * 128, :])
        nc.sync.dma_start(mask_sb[:], maskd)
        nc.sync.dma_start(ident_sb[:], identd)
        for c in range(HP):
            nc.sync.dma_start(wpt[c][:], wpd[c * 128:(c + 1) * 128, :])

        # ones columns of vaug (data cols are fully overwritten by evicts)
        nc.vector.memset(vaug[:, 64:TT * H_LOC * VW:VW], 1.0)

        def evict_engine(i, with_act=False):
            # GPSIMD cannot access PSUM on this target: evictions are
            # DVE-only during attention, DVE/ACT alternating in phases
            # where the ACT (exp) is idle.
            engs = [nc.vector, nc.scalar] if with_act else [nc.vector]
            eng = engs[i % len(engs)]

            def copy(out_ap, in_ap, _eng=eng):
                if _eng is nc.scalar:
                    return _eng.copy(out_ap, in_ap)
                return _eng.tensor_copy(out_ap, in_ap)

            def ts(out_ap, in_ap, scalar, _unused, _op, _eng=eng):
                if _eng is nc.scalar:
                    return _eng.mul(out_ap, in_ap, scalar)
                return _eng.tensor_scalar(out_ap, in_ap, scalar, None, _op)
            return type("E", (), {"tensor_copy": staticmethod(copy),
                                  "tensor_scalar": staticmethod(ts)})

        # ---------------- v projection ------------------------------------
        def v_mm(pv, slot, e, t):
            nc.tensor.matmul(
                pv[:, slot * C:(slot + 1) * C],
                xt[e][:, t * 128:(t + 1) * 128],
                wvt[e][:],
                start=(e == 0), stop=(e == ET - 1))

        def v_evict(pv, slot, t, eng):
            base = t * H_LOC * VW
            dst = vaug[:, base:base + H_LOC * VW].rearrange(
                "p (h c) -> p h c", c=VW)[:, :, 0:64]
            src = pv[:, slot * C:(slot + 1) * C].rearrange(
                "p (h c) -> p h c", c=64)
            eng.tensor_copy(dst, src)

        # head phase: t 0..5 in 4 interleaved groups, then t 6..7
        def emit_v_pass_a():
            g0 = ps_tile("pss", "psv_a0")
            g1 = ps_tile("pss", "psv_a1")
            g2 = ps_tile("misc", "psv_a2")
            g3 = ps_tile("misc", "psv_a3")
            gs = [(g0, 0), (g0, 1), (g1, 0), (g1, 1), (g2, 0), (g3, 0)]
            for e in range(ET):
                for t in range(6):
                    v_mm(gs[t][0], gs[t][1], e, t)
            for t in range(6):
                v_evict(gs[t][0], gs[t][1], t, evict_engine(t, with_act=True))
            g4 = ps_tile("av", "psv_a4")
            for e in range(ET):
                for t in (6, 7):
                    v_mm(g4, t % 2, e, t)
            for t in (6, 7):
                v_evict(g4, t % 2, t, evict_engine(t, with_act=True))

        # ---------------- q/k projections (head phase, pss tag) -----------
        def emit_qk_block(wlist, dst, p, jbp, eng, tag="pss"):
            pq = ps_tile(tag, "psqk")
            for e in range(ET):
                for j in range(2):
                    jb = jbp + j
                    nc.tensor.matmul(
                        pq[:, j * TQB:(j + 1) * TQB],
                        wlist[e][:, p * 128:(p + 1) * 128],
                        xt[e][:, jb * TQB:(jb + 1) * TQB],
                        start=(e == 0), stop=(e == ET - 1))
            eng.tensor_copy(dst[p][:, jbp * TQB:(jbp + 2) * TQB], pq[:])

        # ---------------- pacer / filler machinery ------------------------
        # Two queues: `urgent` (per-chunk transposes — tiny, gate olt) and
        # `background` (v pass B, q/k projections, output projection —
        # clock-paced against the ACT (exp) bottleneck). Entries:
        # (key, gen, min_tick): min_tick delays emission until the DVE work
        # they depend on has had time to execute (avtick = AV emissions).
        clock = {"pe": 0.0, "act": 0.0}
        avtick = [0]
        urgent = deque()
        background = deque()

        def gen_v_group_b(t):
            pv = ps_tile("misc", f"psv_b{t}")
            for e in range(ET):
                v_mm(pv, 0, e, t)
                yield TQB * PE_C
            v_evict(pv, 0, t, evict_engine(t))

        def gen_qk_fill(wlist, dst, p, jb):
            pq = ps_tile("misc", "psqkf")
            for e in range(ET):
                nc.tensor.matmul(
                    pq[:],
                    wlist[e][:, p * 128:(p + 1) * 128],
                    xt[e][:, jb * TQB:(jb + 1) * TQB],
                    start=(e == 0), stop=(e == ET - 1))
                yield TQB * PE_C
            evict_engine(p + jb).tensor_copy(
                dst[p][:, jb * TQB:(jb + 1) * TQB], pq[:])

        def gen_transp_chunk(p, jb, c, usb_c):
            mt = psum.tile([128, 128], BF16, tag="misc", name="pstr",
                           bufs=TAG_BUFS["misc"])
            nc.tensor.transpose(mt[:], usb_c[:], ident_sb[:])
            evict_engine(c).tensor_copy(
                olt[p][:, jb * TQB + c * 128:jb * TQB + (c + 1) * 128],
                mt[:])
            yield 128 * PE_C

        def gen_proj_tile(t, act_evict=False):
            ot = out_pool.tile([128, E], F32, tag="out", name=f"ot{t}")
            for nb in range(2):
                mp = ps_tile("misc", "psproj")
                h = nb * TQB
                for cc in range(HP):
                    nc.tensor.matmul(
                        mp[:],
                        olt[cc][:, t * 128:(t + 1) * 128],
                        wpt[cc][:, h:h + TQB],
                        start=(cc == 0), stop=(cc == HP - 1))
                    yield TQB * PE_C
                # ACT is idle at the very end: offload the evictions
                if act_evict:
                    nc.scalar.copy(ot[:, h:h + TQB], mp[:])
                else:
                    nc.vector.tensor_copy(ot[:, h:h + TQB], mp[:])
                nc.sync.dma_start(out[t * 128:(t + 1) * 128, h:h + TQB],
                                  ot[:, h:h + TQB])

        def step_q(q):
            entry = q[0]
            try:
                clock["pe"] += next(entry[1])
                if len(entry) == 5:
                    entry[4][0] -= 1
                return True
            except StopIteration:
                q.popleft()
                return False

        quota = [0.0]

        def pace():
            while urgent and urgent[0][2] <= avtick[0]:
                step_q(urgent)
            if not background:
                return
            # EDF rationing: per tick emit just enough background steps that
            # every entry finishes by its deadline, spread uniformly
            tick = avtick[0]
            cum, rate = 0.0, 0.0
            for entry in background:
                cum += entry[4][0]
                rate = max(rate, cum / max(entry[3] - tick, 1.0))
            quota[0] = min(quota[0] + max(rate, 1.5), 8.0)
            while (background and quota[0] >= 1.0
                   and background[0][2] <= avtick[0]):
                if step_q(background):
                    quota[0] -= 1.0

        def force_drain(q, pred):
            """Fully emit all entries of q matching pred (FIFO order, so
            everything queued before them drains too)."""
            while any(pred(e[0]) for e in q):
                step_q(q)

        def drain_fillers():
            while urgent:
                step_q(urgent)
            while background:
                step_q(background)

        # ---------------- attention block ---------------------------------
        def emit_attn_block(p, jb, bi):
            n_tk = NJB * (jb + 1)
            av = ps_tile("av", "psav")
            r_t = [r_pool.tile([128, NJB], F32, tag="r", name=f"r{h}")
                   for h in range(2)]
            usb = [None] * NJB
            pend = deque()

            def emit_av():
                avtick[0] += 1
                t, ee = pend.popleft()
                o = t - NJB * jb
                for h in range(2):
                    hb = h * 512
                    vcol = t * H_LOC * VW + (2 * p + h) * VW
                    for cch in range(max(o, 0), NJB):
                        # one accumulation group per PSUM bank per block:
                        # start only on the round's first matmul (start
                        # marks the whole 2KB zero-region pending; later
                        # chunks are lazily zeroed on first write), stop
                        # only on the last (tile n_tk-1 touches chunk 3
                        # alone)
                        nc.tensor.matmul(
                            av[:, hb + cch * VW:hb + cch * VW + VW],
                            ee[:, hb + cch * 128:hb + (cch + 1) * 128],
                            vaug[:, vcol:vcol + VW],
                            start=(t == 0 and cch == max(o, 0)),
                            stop=(t == n_tk - 1 and cch == NJB - 1))
                        clock["pe"] += VW * PE_C
                # after the bank groups stop (last tile): normalize + evict
                # all chunks, then queue their transposes / projections
                if t == n_tk - 1:
                    for h in range(2):
                        hb = h * 512
                        nc.vector.reciprocal(
                            r_t[h][:],
                            av[:, hb + 64:hb + NJB * VW:VW])
                    for cch in range(NJB):
                        usb[cch] = usb_pool.tile(
                            [128, 128], BF16, tag="usb", name=f"usb{cch}")
                        for h in range(2):
                            hb = h * 512
                            eng = evict_engine(cch + h,
                                               with_act=(p == HP - 1))
                            eng.tensor_scalar(
                                usb[cch][:, h * 64:(h + 1) * 64],
                                av[:, hb + cch * VW:hb + cch * VW + 64],
                                r_t[h][:, cch:cch + 1], None, AL.mult)
                        urgent.append((("t", bi),
                                       gen_transp_chunk(p, jb, cch, usb[cch]),
                                       avtick[0] + 3 + cch))
                        if p == HP - 1:
                            # drain each block's proj within the following
                            # pair-3 block (last: by end)
                            dl = {0: 132, 1: 144, 2: 160, 3: 160}[jb]
                            background.append(
                                (("proj", jb),
                                 gen_proj_tile(NJB * jb + cch, jb == NJB - 1),
                                 avtick[0] + 4 + cch, dl, [9]))

            for t in range(n_tk):
                if len(pend) > LAG:
                    emit_av()
                o = t - NJB * jb
                psS = ps_tile("pss", "psS")
                lo = max(o, 0) * 128
                for h in range(2):
                    hb = h * 512
                    kc = kT[p][h * 64:(h + 1) * 64, t * 128:(t + 1) * 128]
                    if o < 0:
                        nc.tensor.matmul(
                            psS[:, hb:hb + 512], kc,
                            qT[p][h * 64:(h + 1) * 64,
                                  jb * TQB:(jb + 1) * TQB],
                            start=True, stop=True)
                    else:
                        # diagonal tile: restricted query columns only;
                        # the within-tile triangle is masked on DVE after
                        # the exp (LAG absorbs the extra hop)
                        nc.tensor.matmul(
                            psS[:, hb + lo:hb + 512], kc,
                            qT[p][h * 64:(h + 1) * 64,
                                  jb * TQB + lo:(jb + 1) * TQB],
                            start=True, stop=True)
                    clock["pe"] += (512 - lo) * PE_C
                ee = ee_pool.tile([128, 1024], BF16, tag="ee", name="ee")
                if o <= 0:
                    nc.scalar.activation(ee[:], psS[:], AF.Exp, scale=SCALE)
                    clock["act"] += _exp_ns(1024)
                else:
                    for h in range(2):
                        nc.scalar.activation(
                            ee[:, h * 512 + lo:h * 512 + 512],
                            psS[:, h * 512 + lo:h * 512 + 512],
                            AF.Exp, scale=SCALE)
                    clock["act"] += _exp_ns(2 * (512 - lo), nops=2)
                if o >= 0:
                    band = ee[:].rearrange("p (h q) -> p h q", q=512)[
                        :, :, o * 128:(o + 1) * 128]
                    mband = mask_sb[:].rearrange("p (h q) -> p h q", q=128)
                    nc.vector.tensor_tensor(band, band, mband, AL.mult)
                pend.append((t, ee))
                pace()
            while pend:
                emit_av()

        # ---------------- emission ----------------------------------------
        emit_v_pass_a()
        emit_qk_block(wqt, qT, 0, 0, evict_engine(0, True))
        emit_qk_block(wqt, qT, 0, 2, evict_engine(1, True))
        emit_qk_block(wkt, kT, 0, 0, evict_engine(0, True))
        emit_qk_block(wkt, kT, 0, 2, evict_engine(1, True))

        for tg in range(8, TT):
            jbn = tg // 4
            dl = max(2 * jbn * (jbn + 1) - 2, 1)
            background.append((("vb", tg), gen_v_group_b(tg), 0, dl, [9]))
        for p in range(1, HP):
            for jb in range(NJB):
                s_blk = 40 * p + 2 * jb * (jb + 1)
                background.append(
                    (("qk", p, jb), gen_qk_fill(wqt, qT, p, jb), 0,
                     max(s_blk - 2, 1), [9]))
                background.append(
                    (("qk", p, jb), gen_qk_fill(wkt, kT, p, jb), 0,
                     max(s_blk + 4 * jb - 2, 1), [9]))
        assert [e[0] for e in background if e[0][0] == "qk"] == [
            ("qk", p, jb) for p in range(1, HP) for jb in range(NJB)
            for _ in range(2)]

        bi = 0
        for p in range(HP):
            for jb in range(NJB):
                # correctness: everything this block consumes must already
                # be emitted (Tile deps follow emission order) — vaug tiles
                # for its key range, q/k of this pair; plus recycle old
                # transposes (usb pool depth) before new norms allocate.
                n_tk = NJB * (jb + 1)
                force_drain(background, lambda k, n=n_tk, p=p, jb=jb: (
                    (k[0] == "vb" and k[1] < n)
                    or (k[0] == "qk" and (k[1] < p
                                          or (k[1] == p and k[2] <= jb)))))
                force_drain(urgent, lambda k, bi=bi: (
                    k[0] == "t" and k[1] <= bi - 2))
                emit_attn_block(p, jb, bi)
                bi += 1
        drain_fillers()
    return nc


def make_host_inputs():
    import ml_dtypes
    tri = np.where(np.arange(128)[None, :] >= np.arange(128)[:, None],
                   1.0, 0.0).astype(np.float32)
    masks = np.concatenate([tri, tri], axis=1).astype(ml_dtypes.bfloat16)
    ident = np.eye(128, dtype=np.float32).astype(ml_dtypes.bfloat16)
    return masks, ident


def shard_inputs(data, Wq, Wk, Wv, Wp):
    """Build the 8 per-core input maps from full inputs."""
    import ml_dtypes
    BF = ml_dtypes.bfloat16
    data = np.asarray(data, np.float32)
    Wq = np.asarray(Wq, np.float32)
    Wk = np.asarray(Wk, np.float32)
    Wv = np.asarray(Wv, np.float32)
    Wp = np.asarray(Wp, np.float32)
    masks, ident = make_host_inputs()
    in_maps = []
    for c in range(N_CORES):
        b, g = c // 2, c % 2
        hs = slice(g * H_LOC, (g + 1) * H_LOC)
        in_maps.append({
            "xT": np.ascontiguousarray(data[b].T).astype(BF),
            "wq": np.ascontiguousarray(
                Wq[hs].transpose(1, 0, 2).reshape(E, H_LOC * D)).astype(BF),
            "wk": np.ascontiguousarray(
                Wk[hs].transpose(1, 0, 2).reshape(E, H_LOC * D)).astype(BF),
            "wv": np.ascontiguousarray(
                Wv[hs].transpose(1, 0, 2).reshape(E, H_LOC * D)).astype(BF),
            "wp": np.ascontiguousarray(
                Wp[g * H_LOC * D:(g + 1) * H_LOC * D, :]).astype(BF),
            "masks": masks,
            "ident": ident,
        })
    return in_maps


_NC_CACHE = {}


def legalize_single_wait(nc):
    """This toolchain's walrus accepts at most ONE sync wait per engine
    instruction; Tile freely emits more. Split extra waits onto preceding
    same-engine NoOps (engine FIFOs make that equivalent)."""
    import bass_rust
    cnt = 0
    for f in nc.m.functions:
        for blk in f.blocks:
            new = []
            changed = False
            for inst in blk.instructions:
                si = inst.sync_info
                if si is not None and len(si.on_wait) > 1:
                    waits = list(si.on_wait)
                    for w in waits[:-1]:
                        nop = bass_rust.InstNoOp(name=f"legal_nop_{cnt}")
                        cnt += 1
                        nop.engine = inst.engine
                        nop.sync_info = bass_rust.SyncInfo(on_wait=[w],
                                                           on_update=[])
                        new.append(nop)
                    inst.sync_info = bass_rust.SyncInfo(
                        on_wait=[waits[-1]], on_update=list(si.on_update))
                    changed = True
                new.append(inst)
            if changed:
                blk.instructions = new
    return cnt


def get_nc():
    if "nc" not in _NC_CACHE:
        nc = bass.Bass("TRN2", target_bir_lowering=False, debug=False,
                       num_devices=N_CORES)
        build_program(nc)
        legalize_single_wait(nc)
        _NC_CACHE["nc"] = nc
    return _NC_CACHE["nc"]


def run(inputs, trace=False, **kw):
    """Run on the 8 NeuronCores; returns (full_output, BassKernelResults)."""
    from concourse.bass_utils import run_bass_kernel_spmd
    nc = get_nc()
    in_maps = shard_inputs(inputs["data"], inputs["Wq"], inputs["Wk"],
                           inputs["Wv"], inputs["Wp"])
    res = run_bass_kernel_spmd(nc, in_maps, core_ids=list(range(N_CORES)),
                               trace=trace, **kw)
    bp = np.asarray(inputs["bp"], np.float32)
    outf = np.empty((B, T, E), np.float32)
    for b in range(B):
        outf[b] = res.results[2 * b]["out"] + res.results[2 * b + 1]["out"] + bp
    return outf, res


def kernel(**inputs):
    out, _ = run(inputs)
    return out
